# revision 4
# baseline (speedup 1.0000x reference)
"""DescrptSeA descriptor on 8 Trainium2 NeuronCores via a hand-written
Bass/Tile kernel (SPMD over the nloc axis, 512 atoms per core).

Per-core device kernel (see _build_kernel): neighbor-coordinate gather via
indirect DMA, smoothed env matrix, 3-layer embedding net (resnet skips folded
into PE matmuls via PSUM accumulation), per-atom contraction to the
[M*AXIS]=1600 descriptor. Host only casts/reshapes inputs and reassembles the
output; the compiled executable is cached across calls.
"""
import numpy as np
import jax
import ml_dtypes

NF = 2
NLOC, NALL = 4096, 8192
S = 512
NNEI = 138
SEL = [46, 92]
M = 100
AXIS = 16
PROT = 1e-6
RMIN, RMAX = 0.5, 6.0
COLS = 552
CH = 368
NCORES = 8


# ---------------------------------------------------------------------------
# device kernel builder
# ---------------------------------------------------------------------------
def _build_kernel():
    import concourse.bass as bass
    import concourse.bacc as bacc
    import concourse.tile as tile
    from concourse import mybir
    from concourse.masks import make_identity

    F32 = mybir.dt.float32
    I32 = mybir.dt.int32
    BF16 = mybir.dt.bfloat16
    AF = mybir.ActivationFunctionType
    ALU = mybir.AluOpType
    AX = mybir.AxisListType

    def ap_of(t):
        return t[:] if not isinstance(t, bass.AP) else t

    def mkap(t, offset_elems, free_dims, parts=None, part_off=0):
        a = ap_of(t)
        pitch = a.ap[0][0]
        p = [pitch, parts if parts is not None else a.ap[0][1]]
        return bass.AP(a.tensor, a.offset + part_off * pitch + offset_elems,
                       [p] + [list(d) for d in free_dims])

    nc = bacc.Bacc(None, target_bir_lowering=False, debug=False)

    nl_d = nc.dram_tensor("nl", [NF, 128, COLS], I32, kind="ExternalInput")
    coord_ds = [nc.dram_tensor(f"coord4_{f}", [NALL, 4], F32, kind="ExternalInput")
                for f in range(NF)]
    ctr_d = nc.dram_tensor("catype", [NF, 128, 16], F32, kind="ExternalInput")
    nrm_d = nc.dram_tensor("nrm", [4, COLS, 4], BF16, kind="ExternalInput")
    w0_d = nc.dram_tensor("w0bd", [2, 4, 128], F32, kind="ExternalInput")
    b0_d = nc.dram_tensor("b0s", [2, 128, 1], F32, kind="ExternalInput")
    w1_d = nc.dram_tensor("w1r", [2, 121, 50], F32, kind="ExternalInput")
    b1_d = nc.dram_tensor("b1s", [2, 128, 1], F32, kind="ExternalInput")
    w2_d = nc.dram_tensor("w2r", [2, 114, M], F32, kind="ExternalInput")
    b2_d = nc.dram_tensor("b2s", [2, M, 1], F32, kind="ExternalInput")
    w2s_d = nc.dram_tensor("w2sr", [2, 121, M], F32, kind="ExternalInput")
    cc2_d = nc.dram_tensor("cc2r", [114, M], F32, kind="ExternalInput")
    cc4_d = nc.dram_tensor("cc4r", [121, M], F32, kind="ExternalInput")
    res_d = nc.dram_tensor("res", [NF, S, M * AXIS], F32, kind="ExternalOutput")

    with tile.TileContext(nc) as tc:
        with (
            tc.tile_pool(name="const", bufs=1) as constp,
            tc.tile_pool(name="frame", bufs=1) as framep,
            tc.tile_pool(name="envt", bufs=1) as envp,
            tc.tile_pool(name="ssb", bufs=2) as ssbp,
            tc.tile_pool(name="mlp", bufs=2) as mlpp,
            tc.tile_pool(name="ggp", bufs=1) as ggp,
            tc.tile_pool(name="tsb", bufs=4) as tsbp,
            tc.tile_pool(name="t0p", bufs=1) as t0p,
            tc.tile_pool(name="outp", bufs=2) as outp_pool,
            tc.tile_pool(name="ps_mlp", bufs=3, space="PSUM") as ps_mlp,
            tc.tile_pool(name="ps_b", bufs=1, space="PSUM") as ps_b,
            tc.tile_pool(name="ps_t", bufs=2, space="PSUM") as ps_t,
            tc.tile_pool(name="ps_small", bufs=2, space="PSUM") as ps_small,
        ):
            ident = constp.tile([128, 128], F32)
            make_identity(nc, ident)
            nrm_sb = []
            for i in range(4):
                t = constp.tile([128, COLS, 4], BF16, tag=f"nrm{i}", name=f"nrm{i}")
                src = bass.AP(nrm_d, i * COLS * 4, [[0, 128], [4, COLS], [1, 4]])
                nc.sync.dma_start(out=t[:], in_=src)
                nrm_sb.append(t)
            nsc0_sb, nscd_sb, nsh0_sb, nshd_sb = nrm_sb
            WS = {}
            for t in range(2):
                for nm, d, shp in (("w0", w0_d, [4, 128]), ("b0", b0_d, [128, 1]),
                                   ("w1", w1_d, [121, 50]), ("b1", b1_d, [128, 1]),
                                   ("w2", w2_d, [114, M]), ("b2", b2_d, [M, 1]),
                                   ("w2s", w2s_d, [121, M])):
                    tl = constp.tile(shp, F32, tag=f"{nm}_{t}", name=f"{nm}_{t}")
                    nc.sync.dma_start(out=tl[:], in_=d[t])
                    WS[(nm, t)] = tl
            cc2_sb = constp.tile([114, M], F32, tag="cc2", name="cc2")
            nc.sync.dma_start(out=cc2_sb[:], in_=cc2_d[:])
            cc4_sb = constp.tile([121, M], F32, tag="cc4", name="cc4")
            nc.sync.dma_start(out=cc4_sb[:], in_=cc4_d[:])

            for f in range(NF):
                # ---- Phase E: env matrix (chunked layout) ----
                it = framep.tile([128, COLS], I32, tag="it", name="it")
                nc.sync.dma_start(out=it[:], in_=nl_d[f])
                mask = envp.tile([128, COLS], F32, tag="mask", name="mask")
                nc.vector.tensor_scalar(out=mask[:], in0=it[:], scalar1=0,
                                        scalar2=None, op0=ALU.is_ge)
                itc = envp.tile([128, COLS], I32, tag="itc", name="itc")
                nc.vector.tensor_scalar(out=itc[:], in0=it[:], scalar1=0,
                                        scalar2=None, op0=ALU.max)
                gt = framep.tile([128, COLS, 4], F32, tag="gt", name="gt")
                for k in range(COLS):
                    nc.gpsimd.indirect_dma_start(
                        out=gt[:, k, :], out_offset=None, in_=coord_ds[f][:],
                        in_offset=bass.IndirectOffsetOnAxis(ap=itc[:, k:k + 1],
                                                            axis=0),
                    )
                ctr = framep.tile([128, 4, 4], F32, tag="ctr", name="ctr")
                nc.sync.dma_start(out=ctr[:],
                                  in_=ctr_d[f].rearrange("p (q c) -> p q c", q=4))

                diff = envp.tile([128, COLS, 3], F32, tag="diff", name="diff")
                ctr_b = mkap(ctr, 0, [[4, 4], [0, NNEI], [1, 3]])
                nc.vector.tensor_tensor(out=diff[:], in0=gt[:, :, 0:3], in1=ctr_b,
                                        op=ALU.subtract)
                sq = envp.tile([128, COLS, 3], F32, tag="sq", name="sq")
                nc.vector.tensor_tensor(out=sq[:], in0=diff[:], in1=diff[:],
                                        op=ALU.mult)
                r2 = envp.tile([128, COLS, 1], F32, tag="r2", name="r2")
                nc.vector.tensor_reduce(out=r2[:], in_=sq[:], axis=AX.X, op=ALU.add)
                r = envp.tile([128, COLS], F32, tag="r", name="r")
                nc.scalar.activation(out=r[:], in_=r2[:, :, 0], func=AF.Sqrt)
                sr = envp.tile([128, COLS], F32, tag="sr", name="sr")
                nc.vector.tensor_scalar(out=sr[:], in0=r[:], scalar1=PROT,
                                        scalar2=None, op0=ALU.add)
                nc.vector.reciprocal(out=sr[:], in_=sr[:])
                sr2 = envp.tile([128, COLS], F32, tag="sr2", name="sr2")
                nc.vector.tensor_tensor(out=sr2[:], in0=sr[:], in1=sr[:], op=ALU.mult)
                uu = envp.tile([128, COLS], F32, tag="uu", name="uu")
                nc.vector.tensor_scalar(out=uu[:], in0=r[:], scalar1=-RMIN,
                                        scalar2=1.0 / (RMAX - RMIN),
                                        op0=ALU.add, op1=ALU.mult)
                nc.vector.tensor_scalar(out=uu[:], in0=uu[:], scalar1=0.0,
                                        scalar2=1.0, op0=ALU.max, op1=ALU.min)
                u2 = envp.tile([128, COLS], F32, tag="u2", name="u2")
                nc.vector.tensor_tensor(out=u2[:], in0=uu[:], in1=uu[:], op=ALU.mult)
                nc.vector.tensor_tensor(out=u2[:], in0=u2[:], in1=uu[:], op=ALU.mult)
                p1 = envp.tile([128, COLS], F32, tag="p1", name="p1")
                nc.vector.tensor_scalar(out=p1[:], in0=uu[:], scalar1=-6.0,
                                        scalar2=15.0, op0=ALU.mult, op1=ALU.add)
                nc.vector.tensor_tensor(out=p1[:], in0=p1[:], in1=uu[:], op=ALU.mult)
                nc.vector.tensor_scalar(out=p1[:], in0=p1[:], scalar1=-10.0,
                                        scalar2=None, op0=ALU.add)
                nc.vector.tensor_tensor(out=p1[:], in0=p1[:], in1=u2[:], op=ALU.mult)
                nc.vector.tensor_scalar(out=p1[:], in0=p1[:], scalar1=1.0,
                                        scalar2=None, op0=ALU.add)
                wm = envp.tile([128, COLS], F32, tag="wm", name="wm")
                nc.vector.tensor_tensor(out=wm[:], in0=p1[:], in1=mask[:],
                                        op=ALU.mult)

                envw = framep.tile([128, COLS, 4], F32, tag="gt", name="envw")
                nc.vector.tensor_copy(out=envw[:, :, 0], in_=sr[:])
                sr2_b = mkap(sr2, 0, [[1, COLS], [0, 3]])
                nc.vector.tensor_tensor(out=envw[:, :, 1:4], in0=diff[:], in1=sr2_b,
                                        op=ALU.mult)
                tpt = envp.tile([128, COLS], F32, tag="r2", name="tpt")
                tpt_src = mkap(ctr, 3, [[4, 4], [0, NNEI]])
                nc.vector.tensor_copy(out=tpt[:], in_=tpt_src)
                tpt_b = mkap(tpt, 0, [[1, COLS], [0, 4]])
                wm_b = mkap(wm, 0, [[1, COLS], [0, 4]])
                x1 = envp.tile([128, COLS, 4], F32, tag="sq", name="x1")
                nc.vector.tensor_tensor(out=x1[:], in0=nscd_sb[:], in1=tpt_b,
                                        op=ALU.mult)
                nc.vector.tensor_tensor(out=x1[:], in0=x1[:], in1=nsc0_sb[:],
                                        op=ALU.add)
                nc.vector.tensor_tensor(out=x1[:], in0=x1[:], in1=wm_b, op=ALU.mult)
                dm = framep.tile([128, COLS, 4], F32, tag="dm", name="dm")
                nc.vector.tensor_tensor(out=dm[:], in0=envw[:], in1=x1[:],
                                        op=ALU.mult)
                y1 = envp.tile([128, COLS, 4], F32, tag="diff", name="y1")
                nc.vector.tensor_tensor(out=y1[:], in0=nshd_sb[:], in1=tpt_b,
                                        op=ALU.mult)
                nc.vector.tensor_tensor(out=y1[:], in0=y1[:], in1=nsh0_sb[:],
                                        op=ALU.add)
                nc.vector.tensor_tensor(out=dm[:], in0=dm[:], in1=y1[:], op=ALU.add)

                # ---- Phase T: rr to slot-major [sel, 4, S] ----
                rr0 = framep.tile([SEL[0], 4, S], F32, tag="rr0", name="rr0")
                rr1 = framep.tile([SEL[1], 4, S], F32, tag="rr1", name="rr1")
                for q in range(4):
                    for ch in range(4):
                        for rr_sb, j0, sel in ((rr0, 0, SEL[0]),
                                               (rr1, SEL[0], SEL[1])):
                            src = mkap(dm, (q * NNEI + j0) * 4 + ch, [[4, sel]])
                            tp = ps_t.tile([128, 128], F32, tag="tpt", name="tpq",
                                           space="PSUM")
                            nc.tensor.transpose(out=tp[:sel, :], in_=src,
                                                identity=ident[:])
                            dst = mkap(rr_sb, ch * S + q, [[4, 128]])
                            nc.vector.tensor_copy(out=dst, in_=tp[:sel, :])

                ssc = framep.tile([128, COLS], F32, tag="ssc", name="ssc")
                nc.vector.tensor_copy(out=ssc[:], in_=dm[:, :, 0])

                # ---- per 64-atom block: MLP + contraction ----
                for blk in range(8):
                    ss_t = {}
                    for seg, (sel, ngrp) in enumerate(((SEL[0], 2), (SEL[1], 4))):
                        sst = ssbp.tile([4, ngrp * CH], F32, tag=f"ss{seg}",
                                        name=f"ss{seg}")
                        j0 = 0 if seg == 0 else SEL[0]
                        src = mkap(ssc, j0, [[NNEI, 4], [1, sel]],
                                   parts=16, part_off=16 * blk)
                        dst = mkap(sst, 0, [[CH, ngrp], [1, CH]])
                        nc.sync.dma_start(out=dst, in_=src)
                        ss_t[seg] = (sst, ngrp, sel)

                    gg_blk = {}
                    for seg in (0, 1):
                        sst, ngrp, sel = ss_t[seg]
                        gg = ggp.tile([M, 64 * sel], F32, tag=f"gg{seg}",
                                      name=f"gg{seg}")
                        gg_blk[seg] = gg
                        for g in range(ngrp):
                            ps0 = ps_mlp.tile([128, CH], F32, tag="psA", name="ps0",
                                              space="PSUM")
                            nc.tensor.matmul(out=ps0[:], lhsT=WS[("w0", seg)][:],
                                             rhs=sst[:, g * CH:(g + 1) * CH],
                                             start=True, stop=True,
                                             tile_position=(0, 0))
                            y0s = mlpp.tile([128, CH], F32, tag="y0s", name="y0s")
                            nc.scalar.activation(out=y0s[:], in_=ps0[:],
                                                 func=AF.Tanh,
                                                 bias=WS[("b0", seg)][:])
                            th1s = []
                            for half in range(2):
                                ps1 = ps_mlp.tile([128, CH], F32, tag="psA",
                                                  name="ps1", space="PSUM")
                                for ci in range(2):
                                    c = half * 2 + ci
                                    nc.tensor.matmul(
                                        out=ps1[64 * ci:64 * ci + 50, :],
                                        lhsT=WS[("w1", seg)][32 * c:32 * c + 25, :],
                                        rhs=y0s[32 * c:32 * c + 25, :],
                                        start=True, stop=True,
                                        tile_position=(32 * c, 64 * ci))
                                th1 = mlpp.tile([128, CH], F32, tag="y1s",
                                                name="th1")
                                nc.scalar.activation(out=th1[:], in_=ps1[:],
                                                     func=AF.Tanh,
                                                     bias=WS[("b1", seg)][:])
                                th1s.append(th1)
                            for c in range(4):
                                th1 = th1s[c // 2]
                                pb = 64 * (c % 2)
                                ps2 = ps_mlp.tile([128, CH], F32, tag="psA",
                                                  name="ps2", space="PSUM")
                                nc.tensor.matmul(out=ps2[:M, :],
                                                 lhsT=WS[("w2", seg)][pb:pb + 50, :],
                                                 rhs=th1[pb:pb + 50, :],
                                                 start=True, stop=False,
                                                 tile_position=(pb, 0))
                                nc.tensor.matmul(
                                    out=ps2[:M, :],
                                    lhsT=WS[("w2s", seg)][32 * c:32 * c + 25, :],
                                    rhs=y0s[32 * c:32 * c + 25, :],
                                    start=False, stop=True,
                                    tile_position=(32 * c, 0))
                                ps3 = ps_b.tile([128, CH], F32, tag="psB",
                                                name="ps3", space="PSUM")
                                nc.tensor.matmul(out=ps3[:M, :],
                                                 lhsT=cc2_sb[pb:pb + 50, :],
                                                 rhs=th1[pb:pb + 50, :],
                                                 start=True, stop=False,
                                                 tile_position=(pb, 0))
                                nc.tensor.matmul(
                                    out=ps3[:M, :],
                                    lhsT=cc4_sb[32 * c:32 * c + 25, :],
                                    rhs=y0s[32 * c:32 * c + 25, :],
                                    start=False, stop=True,
                                    tile_position=(32 * c, 0))
                                o = (c * ngrp + g) * CH
                                nc.scalar.activation(out=gg[:, o:o + CH],
                                                     in_=ps2[:M, :], func=AF.Tanh,
                                                     bias=WS[("b2", seg)][:])
                                nc.vector.tensor_tensor(out=gg[:, o:o + CH],
                                                        in0=gg[:, o:o + CH],
                                                        in1=ps3[:M, :], op=ALU.add)

                    # contraction
                    t0all = t0p.tile([46, 64, M], F32, tag="t0all", name="t0all")
                    for a0 in range(64):
                        tp = ps_t.tile([128, 128], F32, tag="tpt", name="tp0",
                                       space="PSUM")
                        nc.tensor.transpose(out=tp[:46, :M],
                                            in_=gg_blk[0][:, a0 * 46:(a0 + 1) * 46],
                                            identity=ident[0:M, 0:M])
                        nc.vector.tensor_copy(out=t0all[:, a0, :], in_=tp[0:46, :M])
                    obuf = outp_pool.tile([M, 64, AXIS], F32, tag="obuf",
                                          name="obuf")
                    for a in range(64):
                        tp = ps_t.tile([128, 128], F32, tag="tpt", name="tp1",
                                       space="PSUM")
                        nc.tensor.transpose(out=tp[:92, :M],
                                            in_=gg_blk[1][:, a * 92:(a + 1) * 92],
                                            identity=ident[0:M, 0:M])
                        t1 = tsbp.tile([92, M], F32, tag="t1", name="t1")
                        nc.vector.tensor_copy(out=t1[:], in_=tp[:92, :M])

                        A = blk * 64 + a
                        xyz_ps = ps_small.tile([4, M], F32, tag="small",
                                               name="xyzp", space="PSUM")
                        lhs0 = mkap(rr0, A, [[S, 4]])
                        nc.tensor.matmul(out=xyz_ps[:], lhsT=lhs0,
                                         rhs=t0all[:, a, :], start=True, stop=False)
                        lhs1 = mkap(rr1, A, [[S, 4]])
                        nc.tensor.matmul(out=xyz_ps[:], lhsT=lhs1, rhs=t1[:],
                                         start=False, stop=True)
                        xyz = tsbp.tile([4, M], F32, tag="xyzs", name="xyzs")
                        nc.scalar.activation(out=xyz[:], in_=xyz_ps[:],
                                             func=AF.Copy, scale=1.0 / NNEI)
                        res_ps = ps_small.tile([M, AXIS], F32, tag="small",
                                               name="resp", space="PSUM")
                        nc.tensor.matmul(out=res_ps[:], lhsT=xyz[:],
                                         rhs=xyz[:, 0:AXIS], start=True, stop=True)
                        nc.vector.tensor_copy(out=obuf[:, a, :], in_=res_ps[:])
                    src = mkap(obuf, 0, [[AXIS, 64], [1, AXIS]])
                    dst = bass.AP(res_d, (f * S + blk * 64) * M * AXIS,
                                  [[AXIS, M], [M * AXIS, 64], [1, AXIS]])
                    nc.sync.dma_start(out=dst, in_=src)

    nc.finalize()
    return nc


# ---------------------------------------------------------------------------
# cached dispatch (shard_map over 8 cores, built once)
# ---------------------------------------------------------------------------
_EXEC = None


def _get_exec():
    global _EXEC
    if _EXEC is not None:
        return _EXEC
    import concourse.mybir as mybir
    from concourse.bass2jax import (_bass_exec_p, install_neuronx_cc_hook,
                                    partition_id_tensor)
    from jax.experimental.shard_map import shard_map
    from jax.sharding import Mesh, PartitionSpec

    install_neuronx_cc_hook()
    nc = _build_kernel()

    partition_name = (nc.partition_id_tensor.name
                      if nc.partition_id_tensor else None)
    in_names, out_names, out_avals, zero_shapes = [], [], [], []
    for alloc in nc.m.functions[0].allocations:
        if not isinstance(alloc, mybir.MemoryLocationSet):
            continue
        name = alloc.memorylocations[0].name
        if alloc.kind == "ExternalInput":
            if name != partition_name:
                in_names.append(name)
        elif alloc.kind == "ExternalOutput":
            out_names.append(name)
            shape = tuple(alloc.tensor_shape)
            dtype = mybir.dt.np(alloc.dtype)
            out_avals.append(jax.core.ShapedArray(shape, dtype))
            zero_shapes.append((shape, dtype))
    n_params = len(in_names)
    n_outs = len(out_avals)
    all_in_names = list(in_names) + list(out_names)
    if partition_name is not None:
        all_in_names.append(partition_name)
    donate = tuple(range(n_params, n_params + n_outs))

    def _body(*args):
        operands = list(args)
        if partition_name is not None:
            operands.append(partition_id_tensor())
        outs = _bass_exec_p.bind(
            *operands,
            out_avals=tuple(out_avals),
            in_names=tuple(all_in_names),
            out_names=tuple(out_names),
            lowering_input_output_aliases=(),
            sim_require_finite=True,
            sim_require_nnan=True,
            nc=nc,
        )
        return tuple(outs)

    devices = jax.devices()[:NCORES]
    mesh = Mesh(np.asarray(devices), ("core",))
    in_specs = (PartitionSpec("core"),) * (n_params + n_outs)
    out_specs = (PartitionSpec("core"),) * n_outs
    sharded = jax.jit(
        shard_map(_body, mesh=mesh, in_specs=in_specs, out_specs=out_specs,
                  check_rep=False),
        donate_argnums=donate, keep_unused=True)

    from jax.sharding import NamedSharding
    import jax.numpy as jnp
    shardings = [NamedSharding(mesh, PartitionSpec("core"))] * n_outs

    def _mk_zeros():
        return tuple(jnp.zeros((NCORES * shp[0],) + tuple(shp[1:]), dt)
                     for shp, dt in zero_shapes)
    zero_maker = jax.jit(_mk_zeros, out_shardings=tuple(shardings))
    _EXEC = (sharded, in_names, out_names, zero_maker)
    return _EXEC


# ---------------------------------------------------------------------------
# host-side prep + entry point
# ---------------------------------------------------------------------------
def _prep_global_inputs(nlist, coord, atype, mean, stddev, ws, bs):
    """Build the concatenated (8*dim0, ...) arrays for every DRAM input."""
    g = {}
    nl32 = np.asarray(nlist, dtype=np.int32)
    g["nl"] = np.ascontiguousarray(
        nl32.reshape(NF, NCORES, 128, COLS).transpose(1, 0, 2, 3)
    ).reshape(NCORES * NF, 128, COLS)

    coord = np.asarray(coord, dtype=np.float32)
    coord4 = np.zeros((NF, NALL, 4), np.float32)
    coord4[:, :, 0:3] = coord
    g["coord4_0"] = np.tile(coord4[0], (NCORES, 1))
    g["coord4_1"] = np.tile(coord4[1], (NCORES, 1))

    cat = np.zeros((NF, NLOC, 4), np.float32)
    cat[:, :, 0:3] = coord[:, :NLOC, :]
    cat[:, :, 3] = np.asarray(atype)[:, :NLOC].astype(np.float32)
    g["catype"] = np.ascontiguousarray(
        cat.reshape(NF, NCORES, 128, 16).transpose(1, 0, 2, 3)
    ).reshape(NCORES * NF, 128, 16)

    mean = np.asarray(mean, np.float32)
    stddev = np.asarray(stddev, np.float32)
    istd = 1.0 / stddev
    nmean = -mean / stddev
    nrm = np.stack([
        np.tile(istd[0], (4, 1)),
        np.tile(istd[1] - istd[0], (4, 1)),
        np.tile(nmean[0], (4, 1)),
        np.tile(nmean[1] - nmean[0], (4, 1)),
    ]).astype(ml_dtypes.bfloat16)
    g["nrm"] = np.tile(nrm, (NCORES, 1, 1))

    w0, w1, w2 = [np.asarray(w, np.float32) for w in ws]
    b0, b1, b2 = [np.asarray(b, np.float32) for b in bs]
    w0bd = np.zeros((2, 4, 128), np.float32)
    b0s = np.zeros((2, 128, 1), np.float32)
    w1r = np.zeros((2, 121, 50), np.float32)
    b1s = np.zeros((2, 128, 1), np.float32)
    w2r = np.zeros((2, 114, M), np.float32)
    b2s = np.zeros((2, M, 1), np.float32)
    w2sr = np.zeros((2, 121, M), np.float32)
    for t in range(2):
        w2s = w2[t][0:25] + w2[t][25:50]
        for c in range(4):
            w0bd[t, c, 32 * c:32 * c + 25] = w0[t, 0]
            b0s[t, 32 * c:32 * c + 25, 0] = b0[t]
            w1r[t, 32 * c:32 * c + 25, :] = w1[t]
            w2sr[t, 32 * c:32 * c + 25, :] = w2s
        for h in range(2):
            b1s[t, 64 * h:64 * h + 50, 0] = b1[t]
            w2r[t, 64 * h:64 * h + 50, :] = w2[t]
        b2s[t, :, 0] = b2[t]
    cc2 = np.zeros((50, M), np.float32)
    for i in range(50):
        cc2[i, i] = 1.0
        cc2[i, 50 + i] = 1.0
    cc2r = np.zeros((114, M), np.float32)
    cc2r[0:50] = cc2
    cc2r[64:114] = cc2
    cc4 = np.zeros((25, M), np.float32)
    for i in range(25):
        for k in range(4):
            cc4[i, 25 * k + i] = 1.0
    cc4r = np.zeros((121, M), np.float32)
    for c in range(4):
        cc4r[32 * c:32 * c + 25] = cc4
    for nm, arr in (("w0bd", w0bd), ("b0s", b0s), ("w1r", w1r), ("b1s", b1s),
                    ("w2r", w2r), ("b2s", b2s), ("w2sr", w2sr),
                    ("cc2r", cc2r), ("cc4r", cc4r)):
        g[nm] = np.tile(arr, (NCORES,) + (1,) * (arr.ndim - 1))
    return g


def kernel(nlist, extended_coord, extended_atype, mean, stddev,
           w0, b0, w1, b1, w2, b2):
    sharded, in_names, out_names, zero_maker = _get_exec()
    g = _prep_global_inputs(nlist, extended_coord, extended_atype, mean, stddev,
                            [w0, w1, w2], [b0, b1, b2])
    args = [g[n] for n in in_names]
    zeros = zero_maker()
    out_arrs = sharded(*args, *zeros)
    res = out_arrs[out_names.index("res")]  # sharded [8*NF, S, 1600]
    full = np.empty((NF, NLOC, M * AXIS), np.float32)
    for sh in res.addressable_shards:
        c = sh.index[0].start // NF
        arr = np.asarray(sh.data)  # [NF, S, 1600]
        for f in range(NF):
            full[f, c * S:(c + 1) * S, :] = arr[f]
    return full


# revision 5
# speedup vs baseline: 1.4979x; 1.4979x over previous
"""DescrptSeA descriptor on 8 Trainium2 NeuronCores via a hand-written
Bass/Tile kernel (SPMD over the nloc axis, 512 atoms per core).

Per-core device kernel (see _build_kernel): neighbor-coordinate gather via
indirect DMA, smoothed env matrix, 3-layer embedding net (resnet skips folded
into PE matmuls via PSUM accumulation), per-atom contraction to the
[M*AXIS]=1600 descriptor. Host only casts/reshapes inputs and reassembles the
output; the compiled executable is cached across calls.
"""
import numpy as np
import jax
import ml_dtypes

NF = 2
NLOC, NALL = 4096, 8192
S = 512
NNEI = 138
SEL = [46, 92]
M = 100
AXIS = 16
PROT = 1e-6
RMIN, RMAX = 0.5, 6.0
COLS = 552
CH = 368
NCORES = 8


# ---------------------------------------------------------------------------
# device kernel builder
# ---------------------------------------------------------------------------
def _build_kernel():
    import concourse.bass as bass
    import concourse.bacc as bacc
    import concourse.tile as tile
    from concourse import mybir
    from concourse.masks import make_identity

    F32 = mybir.dt.float32
    I32 = mybir.dt.int32
    BF16 = mybir.dt.bfloat16
    AF = mybir.ActivationFunctionType
    ALU = mybir.AluOpType
    AX = mybir.AxisListType

    def ap_of(t):
        return t[:] if not isinstance(t, bass.AP) else t

    def mkap(t, offset_elems, free_dims, parts=None, part_off=0):
        a = ap_of(t)
        pitch = a.ap[0][0]
        p = [pitch, parts if parts is not None else a.ap[0][1]]
        return bass.AP(a.tensor, a.offset + part_off * pitch + offset_elems,
                       [p] + [list(d) for d in free_dims])

    nc = bacc.Bacc(None, target_bir_lowering=False, debug=False)

    nl_d = nc.dram_tensor("nl", [NF, 128, COLS], I32, kind="ExternalInput")
    coord_ds = [nc.dram_tensor(f"coord4_{f}", [NALL, 4], F32, kind="ExternalInput")
                for f in range(NF)]
    ctr_d = nc.dram_tensor("catype", [NF, 128, 16], F32, kind="ExternalInput")
    nrm_d = nc.dram_tensor("nrm", [4, COLS, 4], BF16, kind="ExternalInput")
    w0_d = nc.dram_tensor("w0bd", [2, 4, 128], F32, kind="ExternalInput")
    b0_d = nc.dram_tensor("b0s", [2, 128, 1], F32, kind="ExternalInput")
    w1_d = nc.dram_tensor("w1r", [2, 121, 50], F32, kind="ExternalInput")
    b1_d = nc.dram_tensor("b1s", [2, 128, 1], F32, kind="ExternalInput")
    w2_d = nc.dram_tensor("w2r", [2, 114, M], F32, kind="ExternalInput")
    b2_d = nc.dram_tensor("b2s", [2, M, 1], F32, kind="ExternalInput")
    w2s_d = nc.dram_tensor("w2sr", [2, 121, M], F32, kind="ExternalInput")
    cc2_d = nc.dram_tensor("cc2r", [114, M], F32, kind="ExternalInput")
    cc4_d = nc.dram_tensor("cc4r", [121, M], F32, kind="ExternalInput")
    res_d = nc.dram_tensor("res", [NF, S, M * AXIS], F32, kind="ExternalOutput")

    with tile.TileContext(nc) as tc:
        with (
            tc.tile_pool(name="const", bufs=1) as constp,
            tc.tile_pool(name="frame", bufs=1) as framep,
            tc.tile_pool(name="envt", bufs=1) as envp,
            tc.tile_pool(name="ssb", bufs=2) as ssbp,
            tc.tile_pool(name="mlp", bufs=2) as mlpp,
            tc.tile_pool(name="ggp", bufs=1) as ggp,
            tc.tile_pool(name="tsb", bufs=4) as tsbp,
            tc.tile_pool(name="t0p", bufs=1) as t0p,
            tc.tile_pool(name="outp", bufs=2) as outp_pool,
            tc.tile_pool(name="ps_mlp", bufs=3, space="PSUM") as ps_mlp,
            tc.tile_pool(name="ps_b", bufs=1, space="PSUM") as ps_b,
            tc.tile_pool(name="ps_t", bufs=2, space="PSUM") as ps_t,
            tc.tile_pool(name="ps_small", bufs=2, space="PSUM") as ps_small,
        ):
            ident = constp.tile([128, 128], F32)
            make_identity(nc, ident)
            nrm_sb = []
            for i in range(4):
                t = constp.tile([128, COLS, 4], BF16, tag=f"nrm{i}", name=f"nrm{i}")
                src = bass.AP(nrm_d, i * COLS * 4, [[0, 128], [4, COLS], [1, 4]])
                nc.sync.dma_start(out=t[:], in_=src)
                nrm_sb.append(t)
            nsc0_sb, nscd_sb, nsh0_sb, nshd_sb = nrm_sb
            WS = {}
            for t in range(2):
                for nm, d, shp in (("w0", w0_d, [4, 128]), ("b0", b0_d, [128, 1]),
                                   ("w1", w1_d, [121, 50]), ("b1", b1_d, [128, 1]),
                                   ("w2", w2_d, [114, M]), ("b2", b2_d, [M, 1]),
                                   ("w2s", w2s_d, [121, M])):
                    tl = constp.tile(shp, F32, tag=f"{nm}_{t}", name=f"{nm}_{t}")
                    nc.sync.dma_start(out=tl[:], in_=d[t])
                    WS[(nm, t)] = tl
            cc2_sb = constp.tile([114, M], F32, tag="cc2", name="cc2")
            nc.sync.dma_start(out=cc2_sb[:], in_=cc2_d[:])
            cc4_sb = constp.tile([121, M], F32, tag="cc4", name="cc4")
            nc.sync.dma_start(out=cc4_sb[:], in_=cc4_d[:])

            for f in range(NF):
                # ---- Phase E: env matrix (chunked layout) ----
                it = framep.tile([128, COLS], I32, tag="it", name="it")
                nc.sync.dma_start(out=it[:], in_=nl_d[f])
                mask = envp.tile([128, COLS], F32, tag="mask", name="mask")
                nc.vector.tensor_scalar(out=mask[:], in0=it[:], scalar1=0,
                                        scalar2=None, op0=ALU.is_ge)
                itc = envp.tile([128, COLS], I32, tag="itc", name="itc")
                nc.vector.tensor_scalar(out=itc[:], in0=it[:], scalar1=0,
                                        scalar2=None, op0=ALU.max)
                gt = framep.tile([128, COLS, 4], F32, tag="gt", name="gt")
                for k in range(COLS):
                    nc.gpsimd.indirect_dma_start(
                        out=gt[:, k, :], out_offset=None, in_=coord_ds[f][:],
                        in_offset=bass.IndirectOffsetOnAxis(ap=itc[:, k:k + 1],
                                                            axis=0),
                    )
                ctr = framep.tile([128, 4, 4], F32, tag="ctr", name="ctr")
                nc.sync.dma_start(out=ctr[:],
                                  in_=ctr_d[f].rearrange("p (q c) -> p q c", q=4))

                diff = envp.tile([128, COLS, 3], F32, tag="diff", name="diff")
                ctr_b = mkap(ctr, 0, [[4, 4], [0, NNEI], [1, 3]])
                nc.vector.tensor_tensor(out=diff[:], in0=gt[:, :, 0:3], in1=ctr_b,
                                        op=ALU.subtract)
                sq = envp.tile([128, COLS, 3], F32, tag="sq", name="sq")
                nc.vector.tensor_tensor(out=sq[:], in0=diff[:], in1=diff[:],
                                        op=ALU.mult)
                r2 = envp.tile([128, COLS, 1], F32, tag="r2", name="r2")
                nc.vector.tensor_reduce(out=r2[:], in_=sq[:], axis=AX.X, op=ALU.add)
                r = envp.tile([128, COLS], F32, tag="r", name="r")
                nc.scalar.activation(out=r[:], in_=r2[:, :, 0], func=AF.Sqrt)
                sr = envp.tile([128, COLS], F32, tag="sr", name="sr")
                nc.vector.tensor_scalar(out=sr[:], in0=r[:], scalar1=PROT,
                                        scalar2=None, op0=ALU.add)
                nc.vector.reciprocal(out=sr[:], in_=sr[:])
                sr2 = envp.tile([128, COLS], F32, tag="sr2", name="sr2")
                nc.vector.tensor_tensor(out=sr2[:], in0=sr[:], in1=sr[:], op=ALU.mult)
                uu = envp.tile([128, COLS], F32, tag="uu", name="uu")
                nc.vector.tensor_scalar(out=uu[:], in0=r[:], scalar1=-RMIN,
                                        scalar2=1.0 / (RMAX - RMIN),
                                        op0=ALU.add, op1=ALU.mult)
                nc.vector.tensor_scalar(out=uu[:], in0=uu[:], scalar1=0.0,
                                        scalar2=1.0, op0=ALU.max, op1=ALU.min)
                u2 = envp.tile([128, COLS], F32, tag="u2", name="u2")
                nc.vector.tensor_tensor(out=u2[:], in0=uu[:], in1=uu[:], op=ALU.mult)
                nc.vector.tensor_tensor(out=u2[:], in0=u2[:], in1=uu[:], op=ALU.mult)
                p1 = envp.tile([128, COLS], F32, tag="p1", name="p1")
                nc.vector.tensor_scalar(out=p1[:], in0=uu[:], scalar1=-6.0,
                                        scalar2=15.0, op0=ALU.mult, op1=ALU.add)
                nc.vector.tensor_tensor(out=p1[:], in0=p1[:], in1=uu[:], op=ALU.mult)
                nc.vector.tensor_scalar(out=p1[:], in0=p1[:], scalar1=-10.0,
                                        scalar2=None, op0=ALU.add)
                nc.vector.tensor_tensor(out=p1[:], in0=p1[:], in1=u2[:], op=ALU.mult)
                nc.vector.tensor_scalar(out=p1[:], in0=p1[:], scalar1=1.0,
                                        scalar2=None, op0=ALU.add)
                wm = envp.tile([128, COLS], F32, tag="wm", name="wm")
                nc.vector.tensor_tensor(out=wm[:], in0=p1[:], in1=mask[:],
                                        op=ALU.mult)

                envw = framep.tile([128, COLS, 4], F32, tag="gt", name="envw")
                nc.vector.tensor_copy(out=envw[:, :, 0], in_=sr[:])
                sr2_b = mkap(sr2, 0, [[1, COLS], [0, 3]])
                nc.vector.tensor_tensor(out=envw[:, :, 1:4], in0=diff[:], in1=sr2_b,
                                        op=ALU.mult)
                tpt = envp.tile([128, COLS], F32, tag="r2", name="tpt")
                tpt_src = mkap(ctr, 3, [[4, 4], [0, NNEI]])
                nc.vector.tensor_copy(out=tpt[:], in_=tpt_src)
                tpt_b = mkap(tpt, 0, [[1, COLS], [0, 4]])
                wm_b = mkap(wm, 0, [[1, COLS], [0, 4]])
                x1 = envp.tile([128, COLS, 4], F32, tag="sq", name="x1")
                nc.vector.tensor_tensor(out=x1[:], in0=nscd_sb[:], in1=tpt_b,
                                        op=ALU.mult)
                nc.vector.tensor_tensor(out=x1[:], in0=x1[:], in1=nsc0_sb[:],
                                        op=ALU.add)
                nc.vector.tensor_tensor(out=x1[:], in0=x1[:], in1=wm_b, op=ALU.mult)
                dm = framep.tile([128, COLS, 4], F32, tag="dm", name="dm")
                nc.vector.tensor_tensor(out=dm[:], in0=envw[:], in1=x1[:],
                                        op=ALU.mult)
                y1 = envp.tile([128, COLS, 4], F32, tag="diff", name="y1")
                nc.vector.tensor_tensor(out=y1[:], in0=nshd_sb[:], in1=tpt_b,
                                        op=ALU.mult)
                nc.vector.tensor_tensor(out=y1[:], in0=y1[:], in1=nsh0_sb[:],
                                        op=ALU.add)
                nc.vector.tensor_tensor(out=dm[:], in0=dm[:], in1=y1[:], op=ALU.add)

                # ---- Phase T: rr to slot-major [sel, 4, S] ----
                rr0 = framep.tile([SEL[0], 4, S], F32, tag="rr0", name="rr0")
                rr1 = framep.tile([SEL[1], 4, S], F32, tag="rr1", name="rr1")
                for q in range(4):
                    for ch in range(4):
                        for rr_sb, j0, sel in ((rr0, 0, SEL[0]),
                                               (rr1, SEL[0], SEL[1])):
                            src = mkap(dm, (q * NNEI + j0) * 4 + ch, [[4, sel]])
                            tp = ps_t.tile([128, 128], F32, tag="tpt", name="tpq",
                                           space="PSUM")
                            nc.tensor.transpose(out=tp[:sel, :], in_=src,
                                                identity=ident[:])
                            dst = mkap(rr_sb, ch * S + q, [[4, 128]])
                            nc.vector.tensor_copy(out=dst, in_=tp[:sel, :])

                ssc = framep.tile([128, COLS], F32, tag="ssc", name="ssc")
                nc.vector.tensor_copy(out=ssc[:], in_=dm[:, :, 0])

                # ---- per 64-atom block: MLP + contraction ----
                for blk in range(8):
                    ss_t = {}
                    for seg, (sel, ngrp) in enumerate(((SEL[0], 2), (SEL[1], 4))):
                        sst = ssbp.tile([4, ngrp * CH], F32, tag=f"ss{seg}",
                                        name=f"ss{seg}")
                        j0 = 0 if seg == 0 else SEL[0]
                        src = mkap(ssc, j0, [[NNEI, 4], [1, sel]],
                                   parts=16, part_off=16 * blk)
                        dst = mkap(sst, 0, [[CH, ngrp], [1, CH]])
                        nc.sync.dma_start(out=dst, in_=src)
                        ss_t[seg] = (sst, ngrp, sel)

                    gg_blk = {}
                    for seg in (0, 1):
                        sst, ngrp, sel = ss_t[seg]
                        gg = ggp.tile([M, 64 * sel], F32, tag=f"gg{seg}",
                                      name=f"gg{seg}")
                        gg_blk[seg] = gg
                        for g in range(ngrp):
                            ps0 = ps_mlp.tile([128, CH], F32, tag="psA", name="ps0",
                                              space="PSUM")
                            nc.tensor.matmul(out=ps0[:], lhsT=WS[("w0", seg)][:],
                                             rhs=sst[:, g * CH:(g + 1) * CH],
                                             start=True, stop=True,
                                             tile_position=(0, 0))
                            y0s = mlpp.tile([128, CH], F32, tag="y0s", name="y0s")
                            nc.scalar.activation(out=y0s[:], in_=ps0[:],
                                                 func=AF.Tanh,
                                                 bias=WS[("b0", seg)][:])
                            th1s = []
                            for half in range(2):
                                ps1 = ps_mlp.tile([128, CH], F32, tag="psA",
                                                  name="ps1", space="PSUM")
                                for ci in range(2):
                                    c = half * 2 + ci
                                    nc.tensor.matmul(
                                        out=ps1[64 * ci:64 * ci + 50, :],
                                        lhsT=WS[("w1", seg)][32 * c:32 * c + 25, :],
                                        rhs=y0s[32 * c:32 * c + 25, :],
                                        start=True, stop=True,
                                        tile_position=(32 * c, 64 * ci))
                                th1 = mlpp.tile([128, CH], F32, tag="y1s",
                                                name="th1")
                                nc.scalar.activation(out=th1[:], in_=ps1[:],
                                                     func=AF.Tanh,
                                                     bias=WS[("b1", seg)][:])
                                th1s.append(th1)
                            for c in range(4):
                                th1 = th1s[c // 2]
                                pb = 64 * (c % 2)
                                ps2 = ps_mlp.tile([128, CH], F32, tag="psA",
                                                  name="ps2", space="PSUM")
                                nc.tensor.matmul(out=ps2[:M, :],
                                                 lhsT=WS[("w2", seg)][pb:pb + 50, :],
                                                 rhs=th1[pb:pb + 50, :],
                                                 start=True, stop=False,
                                                 tile_position=(pb, 0))
                                nc.tensor.matmul(
                                    out=ps2[:M, :],
                                    lhsT=WS[("w2s", seg)][32 * c:32 * c + 25, :],
                                    rhs=y0s[32 * c:32 * c + 25, :],
                                    start=False, stop=True,
                                    tile_position=(32 * c, 0))
                                ps3 = ps_b.tile([128, CH], F32, tag="psB",
                                                name="ps3", space="PSUM")
                                nc.tensor.matmul(out=ps3[:M, :],
                                                 lhsT=cc2_sb[pb:pb + 50, :],
                                                 rhs=th1[pb:pb + 50, :],
                                                 start=True, stop=False,
                                                 tile_position=(pb, 0))
                                nc.tensor.matmul(
                                    out=ps3[:M, :],
                                    lhsT=cc4_sb[32 * c:32 * c + 25, :],
                                    rhs=y0s[32 * c:32 * c + 25, :],
                                    start=False, stop=True,
                                    tile_position=(32 * c, 0))
                                o = (c * ngrp + g) * CH
                                nc.scalar.activation(out=gg[:, o:o + CH],
                                                     in_=ps2[:M, :], func=AF.Tanh,
                                                     bias=WS[("b2", seg)][:])
                                nc.vector.tensor_tensor(out=gg[:, o:o + CH],
                                                        in0=gg[:, o:o + CH],
                                                        in1=ps3[:M, :], op=ALU.add)

                    # contraction
                    t0all = t0p.tile([46, 64, M], F32, tag="t0all", name="t0all")
                    for a0 in range(64):
                        tp = ps_t.tile([128, 128], F32, tag="tpt", name="tp0",
                                       space="PSUM")
                        nc.tensor.transpose(out=tp[:46, :M],
                                            in_=gg_blk[0][:, a0 * 46:(a0 + 1) * 46],
                                            identity=ident[0:M, 0:M])
                        nc.vector.tensor_copy(out=t0all[:, a0, :], in_=tp[0:46, :M])
                    obuf = outp_pool.tile([M, 64, AXIS], F32, tag="obuf",
                                          name="obuf")
                    for a in range(64):
                        tp = ps_t.tile([128, 128], F32, tag="tpt", name="tp1",
                                       space="PSUM")
                        nc.tensor.transpose(out=tp[:92, :M],
                                            in_=gg_blk[1][:, a * 92:(a + 1) * 92],
                                            identity=ident[0:M, 0:M])
                        t1 = tsbp.tile([92, M], F32, tag="t1", name="t1")
                        nc.vector.tensor_copy(out=t1[:], in_=tp[:92, :M])

                        A = blk * 64 + a
                        xyz_ps = ps_small.tile([4, M], F32, tag="small",
                                               name="xyzp", space="PSUM")
                        lhs0 = mkap(rr0, A, [[S, 4]])
                        nc.tensor.matmul(out=xyz_ps[:], lhsT=lhs0,
                                         rhs=t0all[:, a, :], start=True, stop=False)
                        lhs1 = mkap(rr1, A, [[S, 4]])
                        nc.tensor.matmul(out=xyz_ps[:], lhsT=lhs1, rhs=t1[:],
                                         start=False, stop=True)
                        xyz = tsbp.tile([4, M], F32, tag="xyzs", name="xyzs")
                        nc.scalar.activation(out=xyz[:], in_=xyz_ps[:],
                                             func=AF.Copy, scale=1.0 / NNEI)
                        res_ps = ps_small.tile([M, AXIS], F32, tag="small",
                                               name="resp", space="PSUM")
                        nc.tensor.matmul(out=res_ps[:], lhsT=xyz[:],
                                         rhs=xyz[:, 0:AXIS], start=True, stop=True)
                        nc.vector.tensor_copy(out=obuf[:, a, :], in_=res_ps[:])
                    src = mkap(obuf, 0, [[AXIS, 64], [1, AXIS]])
                    dst = bass.AP(res_d, (f * S + blk * 64) * M * AXIS,
                                  [[AXIS, M], [M * AXIS, 64], [1, AXIS]])
                    nc.sync.dma_start(out=dst, in_=src)

    nc.finalize()
    return nc


# ---------------------------------------------------------------------------
# cached dispatch (shard_map over 8 cores, built once)
# ---------------------------------------------------------------------------
_EXEC = None


def _get_exec():
    global _EXEC
    if _EXEC is not None:
        return _EXEC
    import concourse.mybir as mybir
    from concourse.bass2jax import (_bass_exec_p, install_neuronx_cc_hook,
                                    partition_id_tensor)
    from jax.experimental.shard_map import shard_map
    from jax.sharding import Mesh, PartitionSpec

    install_neuronx_cc_hook()
    nc = _build_kernel()

    partition_name = (nc.partition_id_tensor.name
                      if nc.partition_id_tensor else None)
    in_names, out_names, out_avals, zero_shapes = [], [], [], []
    for alloc in nc.m.functions[0].allocations:
        if not isinstance(alloc, mybir.MemoryLocationSet):
            continue
        name = alloc.memorylocations[0].name
        if alloc.kind == "ExternalInput":
            if name != partition_name:
                in_names.append(name)
        elif alloc.kind == "ExternalOutput":
            out_names.append(name)
            shape = tuple(alloc.tensor_shape)
            dtype = mybir.dt.np(alloc.dtype)
            out_avals.append(jax.core.ShapedArray(shape, dtype))
            zero_shapes.append((shape, dtype))
    n_params = len(in_names)
    n_outs = len(out_avals)
    all_in_names = list(in_names) + list(out_names)
    if partition_name is not None:
        all_in_names.append(partition_name)
    donate = tuple(range(n_params, n_params + n_outs))

    def _body(*args):
        operands = list(args)
        if partition_name is not None:
            operands.append(partition_id_tensor())
        outs = _bass_exec_p.bind(
            *operands,
            out_avals=tuple(out_avals),
            in_names=tuple(all_in_names),
            out_names=tuple(out_names),
            lowering_input_output_aliases=(),
            sim_require_finite=True,
            sim_require_nnan=True,
            nc=nc,
        )
        return tuple(outs)

    devices = jax.devices()[:NCORES]
    mesh = Mesh(np.asarray(devices), ("core",))
    in_specs = (PartitionSpec("core"),) * (n_params + n_outs)
    out_specs = (PartitionSpec("core"),) * n_outs
    sharded = jax.jit(
        shard_map(_body, mesh=mesh, in_specs=in_specs, out_specs=out_specs,
                  check_rep=False),
        donate_argnums=donate, keep_unused=True)

    from jax.sharding import NamedSharding
    import jax.numpy as jnp
    shardings = [NamedSharding(mesh, PartitionSpec("core"))] * n_outs

    def _mk_zeros():
        return tuple(jnp.zeros((NCORES * shp[0],) + tuple(shp[1:]), dt)
                     for shp, dt in zero_shapes)
    zero_maker = jax.jit(_mk_zeros, out_shardings=tuple(shardings))
    _EXEC = (sharded, in_names, out_names, zero_maker)
    return _EXEC


# ---------------------------------------------------------------------------
# host-side prep + entry point
# ---------------------------------------------------------------------------
def _prep_global_inputs(nlist, coord, atype, mean, stddev, ws, bs):
    """Build the concatenated (8*dim0, ...) arrays for every DRAM input."""
    g = {}
    nl32 = np.asarray(nlist, dtype=np.int32)
    g["nl"] = np.ascontiguousarray(
        nl32.reshape(NF, NCORES, 128, COLS).transpose(1, 0, 2, 3)
    ).reshape(NCORES * NF, 128, COLS)

    coord = np.asarray(coord, dtype=np.float32)
    coord4 = np.zeros((NF, NALL, 4), np.float32)
    coord4[:, :, 0:3] = coord
    g["coord4_0"] = np.tile(coord4[0], (NCORES, 1))
    g["coord4_1"] = np.tile(coord4[1], (NCORES, 1))

    cat = np.zeros((NF, NLOC, 4), np.float32)
    cat[:, :, 0:3] = coord[:, :NLOC, :]
    cat[:, :, 3] = np.asarray(atype)[:, :NLOC].astype(np.float32)
    g["catype"] = np.ascontiguousarray(
        cat.reshape(NF, NCORES, 128, 16).transpose(1, 0, 2, 3)
    ).reshape(NCORES * NF, 128, 16)

    mean = np.asarray(mean, np.float32)
    stddev = np.asarray(stddev, np.float32)
    istd = 1.0 / stddev
    nmean = -mean / stddev
    nrm = np.stack([
        np.tile(istd[0], (4, 1)),
        np.tile(istd[1] - istd[0], (4, 1)),
        np.tile(nmean[0], (4, 1)),
        np.tile(nmean[1] - nmean[0], (4, 1)),
    ]).astype(ml_dtypes.bfloat16)
    g["nrm"] = np.tile(nrm, (NCORES, 1, 1))

    w0, w1, w2 = [np.asarray(w, np.float32) for w in ws]
    b0, b1, b2 = [np.asarray(b, np.float32) for b in bs]
    w0bd = np.zeros((2, 4, 128), np.float32)
    b0s = np.zeros((2, 128, 1), np.float32)
    w1r = np.zeros((2, 121, 50), np.float32)
    b1s = np.zeros((2, 128, 1), np.float32)
    w2r = np.zeros((2, 114, M), np.float32)
    b2s = np.zeros((2, M, 1), np.float32)
    w2sr = np.zeros((2, 121, M), np.float32)
    for t in range(2):
        w2s = w2[t][0:25] + w2[t][25:50]
        for c in range(4):
            w0bd[t, c, 32 * c:32 * c + 25] = w0[t, 0]
            b0s[t, 32 * c:32 * c + 25, 0] = b0[t]
            w1r[t, 32 * c:32 * c + 25, :] = w1[t]
            w2sr[t, 32 * c:32 * c + 25, :] = w2s
        for h in range(2):
            b1s[t, 64 * h:64 * h + 50, 0] = b1[t]
            w2r[t, 64 * h:64 * h + 50, :] = w2[t]
        b2s[t, :, 0] = b2[t]
    cc2 = np.zeros((50, M), np.float32)
    for i in range(50):
        cc2[i, i] = 1.0
        cc2[i, 50 + i] = 1.0
    cc2r = np.zeros((114, M), np.float32)
    cc2r[0:50] = cc2
    cc2r[64:114] = cc2
    cc4 = np.zeros((25, M), np.float32)
    for i in range(25):
        for k in range(4):
            cc4[i, 25 * k + i] = 1.0
    cc4r = np.zeros((121, M), np.float32)
    for c in range(4):
        cc4r[32 * c:32 * c + 25] = cc4
    for nm, arr in (("w0bd", w0bd), ("b0s", b0s), ("w1r", w1r), ("b1s", b1s),
                    ("w2r", w2r), ("b2s", b2s), ("w2sr", w2sr),
                    ("cc2r", cc2r), ("cc4r", cc4r)):
        g[nm] = np.tile(arr, (NCORES,) + (1,) * (arr.ndim - 1))
    return g


def kernel(nlist, extended_coord, extended_atype, mean, stddev,
           w0, b0, w1, b1, w2, b2):
    sharded, in_names, out_names, zero_maker = _get_exec()
    g = _prep_global_inputs(nlist, extended_coord, extended_atype, mean, stddev,
                            [w0, w1, w2], [b0, b1, b2])
    args = [g[n] for n in in_names]
    zeros = zero_maker()
    out_arrs = sharded(*args, *zeros)
    res = np.asarray(out_arrs[out_names.index("res")])  # [8*NF, S, 1600]
    r = res.reshape(NCORES, NF, S, M * AXIS)
    full = np.empty((NF, NLOC, M * AXIS), np.float32)
    for c in range(NCORES):
        for f in range(NF):
            full[f, c * S:(c + 1) * S, :] = r[c, f]
    return full


# revision 7
# speedup vs baseline: 1.5086x; 1.0072x over previous
"""DescrptSeA descriptor on 8 Trainium2 NeuronCores via a hand-written
Bass/Tile kernel (SPMD over the nloc axis, 512 atoms per core).

Per-core device kernel (see _build_kernel): neighbor-coordinate gather via
indirect DMA, smoothed env matrix, 3-layer embedding net (resnet skips folded
into PE matmuls via PSUM accumulation), per-atom contraction to the
[M*AXIS]=1600 descriptor. Host only casts/reshapes inputs and reassembles the
output; the compiled executable is cached across calls.
"""
import numpy as np
import jax
import ml_dtypes

NF = 2
NLOC, NALL = 4096, 8192
S = 512
NNEI = 138
SEL = [46, 92]
M = 100
AXIS = 16
PROT = 1e-6
RMIN, RMAX = 0.5, 6.0
COLS = 552
CH = 368
NCORES = 8


# ---------------------------------------------------------------------------
# device kernel builder
# ---------------------------------------------------------------------------
def _build_kernel():
    import concourse.bass as bass
    import concourse.bacc as bacc
    import concourse.tile as tile
    from concourse import mybir
    from concourse.masks import make_identity

    F32 = mybir.dt.float32
    I32 = mybir.dt.int32
    BF16 = mybir.dt.bfloat16
    AF = mybir.ActivationFunctionType
    ALU = mybir.AluOpType
    AX = mybir.AxisListType

    def ap_of(t):
        return t[:] if not isinstance(t, bass.AP) else t

    def mkap(t, offset_elems, free_dims, parts=None, part_off=0):
        a = ap_of(t)
        pitch = a.ap[0][0]
        p = [pitch, parts if parts is not None else a.ap[0][1]]
        return bass.AP(a.tensor, a.offset + part_off * pitch + offset_elems,
                       [p] + [list(d) for d in free_dims])

    nc = bacc.Bacc(None, target_bir_lowering=False, debug=False)

    I16 = mybir.dt.int16
    nl_d = nc.dram_tensor("nl", [NF, 128, COLS], I16, kind="ExternalInput")
    coord_ds = [nc.dram_tensor(f"coord4_{f}", [NALL, 4], F32, kind="ExternalInput")
                for f in range(NF)]
    ctr_d = nc.dram_tensor("catype", [NF, 128, 16], F32, kind="ExternalInput")
    nrm_d = nc.dram_tensor("nrm", [4, COLS, 4], BF16, kind="ExternalInput")
    w0_d = nc.dram_tensor("w0bd", [2, 4, 128], F32, kind="ExternalInput")
    b0_d = nc.dram_tensor("b0s", [2, 128, 1], F32, kind="ExternalInput")
    w1_d = nc.dram_tensor("w1r", [2, 121, 50], F32, kind="ExternalInput")
    b1_d = nc.dram_tensor("b1s", [2, 128, 1], F32, kind="ExternalInput")
    w2_d = nc.dram_tensor("w2r", [2, 114, M], F32, kind="ExternalInput")
    b2_d = nc.dram_tensor("b2s", [2, M, 1], F32, kind="ExternalInput")
    w2s_d = nc.dram_tensor("w2sr", [2, 121, M], F32, kind="ExternalInput")
    cc2_d = nc.dram_tensor("cc2r", [114, M], F32, kind="ExternalInput")
    cc4_d = nc.dram_tensor("cc4r", [121, M], F32, kind="ExternalInput")
    res_d = nc.dram_tensor("res", [NF, S, M * AXIS], F32, kind="ExternalOutput")

    with tile.TileContext(nc) as tc:
        with (
            tc.tile_pool(name="const", bufs=1) as constp,
            tc.tile_pool(name="frame", bufs=1) as framep,
            tc.tile_pool(name="gather", bufs=2) as gatherp,
            tc.tile_pool(name="envt", bufs=1) as envp,
            tc.tile_pool(name="ssb", bufs=2) as ssbp,
            tc.tile_pool(name="mlp", bufs=2) as mlpp,
            tc.tile_pool(name="ggp", bufs=1) as ggp,
            tc.tile_pool(name="tsb", bufs=4) as tsbp,
            tc.tile_pool(name="t0p", bufs=1) as t0p,
            tc.tile_pool(name="outp", bufs=1) as outp_pool,
            tc.tile_pool(name="ps_mlp", bufs=3, space="PSUM") as ps_mlp,
            tc.tile_pool(name="ps_b", bufs=1, space="PSUM") as ps_b,
            tc.tile_pool(name="ps_t", bufs=2, space="PSUM") as ps_t,
            tc.tile_pool(name="ps_small", bufs=2, space="PSUM") as ps_small,
        ):
            ident = constp.tile([128, 128], F32)
            make_identity(nc, ident)
            nrm_sb = []
            for i in range(4):
                t = constp.tile([128, COLS, 4], BF16, tag=f"nrm{i}", name=f"nrm{i}")
                src = bass.AP(nrm_d, i * COLS * 4, [[0, 128], [4, COLS], [1, 4]])
                nc.sync.dma_start(out=t[:], in_=src)
                nrm_sb.append(t)
            nsc0_sb, nscd_sb, nsh0_sb, nshd_sb = nrm_sb
            WS = {}
            for t in range(2):
                for nm, d, shp in (("w0", w0_d, [4, 128]), ("b0", b0_d, [128, 1]),
                                   ("w1", w1_d, [121, 50]), ("b1", b1_d, [128, 1]),
                                   ("w2", w2_d, [114, M]), ("b2", b2_d, [M, 1]),
                                   ("w2s", w2s_d, [121, M])):
                    tl = constp.tile(shp, F32, tag=f"{nm}_{t}", name=f"{nm}_{t}")
                    nc.sync.dma_start(out=tl[:], in_=d[t])
                    WS[(nm, t)] = tl
            cc2_sb = constp.tile([114, M], F32, tag="cc2", name="cc2")
            nc.sync.dma_start(out=cc2_sb[:], in_=cc2_d[:])
            cc4_sb = constp.tile([121, M], F32, tag="cc4", name="cc4")
            nc.sync.dma_start(out=cc4_sb[:], in_=cc4_d[:])

            for f in range(NF):
                # ---- Phase E: env matrix (chunked layout) ----
                it = gatherp.tile([128, COLS], I16, tag="it", name="it")
                nc.sync.dma_start(out=it[:], in_=nl_d[f])
                mask = envp.tile([128, COLS], F32, tag="mask", name="mask")
                nc.vector.tensor_scalar(out=mask[:], in0=it[:], scalar1=0,
                                        scalar2=None, op0=ALU.is_ge)
                itc = gatherp.tile([128, COLS], I32, tag="itc", name="itc")
                nc.vector.tensor_scalar(out=itc[:], in0=it[:], scalar1=0,
                                        scalar2=None, op0=ALU.max)
                gt = gatherp.tile([128, COLS, 4], F32, tag="gt", name="gt")
                for k in range(COLS):
                    nc.gpsimd.indirect_dma_start(
                        out=gt[:, k, :], out_offset=None, in_=coord_ds[f][:],
                        in_offset=bass.IndirectOffsetOnAxis(ap=itc[:, k:k + 1],
                                                            axis=0),
                    )
                ctr = framep.tile([128, 4, 4], F32, tag="ctr", name="ctr")
                nc.sync.dma_start(out=ctr[:],
                                  in_=ctr_d[f].rearrange("p (q c) -> p q c", q=4))

                diff = envp.tile([128, COLS, 3], F32, tag="diff", name="diff")
                ctr_b = mkap(ctr, 0, [[4, 4], [0, NNEI], [1, 3]])
                nc.vector.tensor_tensor(out=diff[:], in0=gt[:, :, 0:3], in1=ctr_b,
                                        op=ALU.subtract)
                sq = envp.tile([128, COLS, 3], F32, tag="sq", name="sq")
                nc.vector.tensor_tensor(out=sq[:], in0=diff[:], in1=diff[:],
                                        op=ALU.mult)
                r2 = envp.tile([128, COLS, 1], F32, tag="r2", name="r2")
                nc.vector.tensor_reduce(out=r2[:], in_=sq[:], axis=AX.X, op=ALU.add)
                r = envp.tile([128, COLS], F32, tag="r", name="r")
                nc.scalar.activation(out=r[:], in_=r2[:, :, 0], func=AF.Sqrt)
                sr = envp.tile([128, COLS], F32, tag="sr", name="sr")
                nc.vector.tensor_scalar(out=sr[:], in0=r[:], scalar1=PROT,
                                        scalar2=None, op0=ALU.add)
                nc.vector.reciprocal(out=sr[:], in_=sr[:])
                sr2 = envp.tile([128, COLS], F32, tag="sr2", name="sr2")
                nc.vector.tensor_tensor(out=sr2[:], in0=sr[:], in1=sr[:], op=ALU.mult)
                uu = envp.tile([128, COLS], F32, tag="uu", name="uu")
                nc.vector.tensor_scalar(out=uu[:], in0=r[:], scalar1=-RMIN,
                                        scalar2=1.0 / (RMAX - RMIN),
                                        op0=ALU.add, op1=ALU.mult)
                nc.vector.tensor_scalar(out=uu[:], in0=uu[:], scalar1=0.0,
                                        scalar2=1.0, op0=ALU.max, op1=ALU.min)
                u2 = envp.tile([128, COLS], F32, tag="u2", name="u2")
                nc.vector.tensor_tensor(out=u2[:], in0=uu[:], in1=uu[:], op=ALU.mult)
                nc.vector.tensor_tensor(out=u2[:], in0=u2[:], in1=uu[:], op=ALU.mult)
                p1 = envp.tile([128, COLS], F32, tag="p1", name="p1")
                nc.vector.tensor_scalar(out=p1[:], in0=uu[:], scalar1=-6.0,
                                        scalar2=15.0, op0=ALU.mult, op1=ALU.add)
                nc.vector.tensor_tensor(out=p1[:], in0=p1[:], in1=uu[:], op=ALU.mult)
                nc.vector.tensor_scalar(out=p1[:], in0=p1[:], scalar1=-10.0,
                                        scalar2=None, op0=ALU.add)
                nc.vector.tensor_tensor(out=p1[:], in0=p1[:], in1=u2[:], op=ALU.mult)
                nc.vector.tensor_scalar(out=p1[:], in0=p1[:], scalar1=1.0,
                                        scalar2=None, op0=ALU.add)
                wm = envp.tile([128, COLS], F32, tag="wm", name="wm")
                nc.vector.tensor_tensor(out=wm[:], in0=p1[:], in1=mask[:],
                                        op=ALU.mult)

                envw = framep.tile([128, COLS, 4], F32, tag="envw", name="envw")
                nc.vector.tensor_copy(out=envw[:, :, 0], in_=sr[:])
                sr2_b = mkap(sr2, 0, [[1, COLS], [0, 3]])
                nc.vector.tensor_tensor(out=envw[:, :, 1:4], in0=diff[:], in1=sr2_b,
                                        op=ALU.mult)
                tpt = envp.tile([128, COLS], F32, tag="r2", name="tpt")
                tpt_src = mkap(ctr, 3, [[4, 4], [0, NNEI]])
                nc.vector.tensor_copy(out=tpt[:], in_=tpt_src)
                tpt_b = mkap(tpt, 0, [[1, COLS], [0, 4]])
                wm_b = mkap(wm, 0, [[1, COLS], [0, 4]])
                x1 = envp.tile([128, COLS, 4], F32, tag="sq", name="x1")
                nc.vector.tensor_tensor(out=x1[:], in0=nscd_sb[:], in1=tpt_b,
                                        op=ALU.mult)
                nc.vector.tensor_tensor(out=x1[:], in0=x1[:], in1=nsc0_sb[:],
                                        op=ALU.add)
                nc.vector.tensor_tensor(out=x1[:], in0=x1[:], in1=wm_b, op=ALU.mult)
                dm = framep.tile([128, COLS, 4], F32, tag="dm", name="dm")
                nc.vector.tensor_tensor(out=dm[:], in0=envw[:], in1=x1[:],
                                        op=ALU.mult)
                y1 = envp.tile([128, COLS, 4], F32, tag="diff", name="y1")
                nc.vector.tensor_tensor(out=y1[:], in0=nshd_sb[:], in1=tpt_b,
                                        op=ALU.mult)
                nc.vector.tensor_tensor(out=y1[:], in0=y1[:], in1=nsh0_sb[:],
                                        op=ALU.add)
                nc.vector.tensor_tensor(out=dm[:], in0=dm[:], in1=y1[:], op=ALU.add)

                # ---- Phase T: rr to slot-major [sel, 4, S] ----
                rr0 = framep.tile([SEL[0], 4, S], F32, tag="rr0", name="rr0")
                rr1 = framep.tile([SEL[1], 4, S], F32, tag="rr1", name="rr1")
                for q in range(4):
                    for ch in range(4):
                        for rr_sb, j0, sel in ((rr0, 0, SEL[0]),
                                               (rr1, SEL[0], SEL[1])):
                            src = mkap(dm, (q * NNEI + j0) * 4 + ch, [[4, sel]])
                            tp = ps_t.tile([128, 128], F32, tag="tpt", name="tpq",
                                           space="PSUM")
                            nc.tensor.transpose(out=tp[:sel, :], in_=src,
                                                identity=ident[:])
                            dst = mkap(rr_sb, ch * S + q, [[4, 128]])
                            nc.vector.tensor_copy(out=dst, in_=tp[:sel, :])

                ssc = framep.tile([128, COLS], F32, tag="ssc", name="ssc")
                nc.vector.tensor_copy(out=ssc[:], in_=dm[:, :, 0])

                # ---- per 64-atom block: MLP + contraction ----
                for blk in range(8):
                    ss_t = {}
                    for seg, (sel, ngrp) in enumerate(((SEL[0], 2), (SEL[1], 4))):
                        sst = ssbp.tile([4, ngrp * CH], F32, tag=f"ss{seg}",
                                        name=f"ss{seg}")
                        j0 = 0 if seg == 0 else SEL[0]
                        src = mkap(ssc, j0, [[NNEI, 4], [1, sel]],
                                   parts=16, part_off=16 * blk)
                        dst = mkap(sst, 0, [[CH, ngrp], [1, CH]])
                        nc.sync.dma_start(out=dst, in_=src)
                        ss_t[seg] = (sst, ngrp, sel)

                    gg_blk = {}
                    for seg in (0, 1):
                        sst, ngrp, sel = ss_t[seg]
                        gg = ggp.tile([M, 64 * sel], F32, tag=f"gg{seg}",
                                      name=f"gg{seg}")
                        gg_blk[seg] = gg
                        for g in range(ngrp):
                            ps0 = ps_mlp.tile([128, CH], F32, tag="psA", name="ps0",
                                              space="PSUM")
                            nc.tensor.matmul(out=ps0[:], lhsT=WS[("w0", seg)][:],
                                             rhs=sst[:, g * CH:(g + 1) * CH],
                                             start=True, stop=True,
                                             tile_position=(0, 0))
                            y0s = mlpp.tile([128, CH], F32, tag="y0s", name="y0s")
                            nc.scalar.activation(out=y0s[:], in_=ps0[:],
                                                 func=AF.Tanh,
                                                 bias=WS[("b0", seg)][:])
                            th1s = []
                            for half in range(2):
                                ps1 = ps_mlp.tile([128, CH], F32, tag="psA",
                                                  name="ps1", space="PSUM")
                                for ci in range(2):
                                    c = half * 2 + ci
                                    nc.tensor.matmul(
                                        out=ps1[64 * ci:64 * ci + 50, :],
                                        lhsT=WS[("w1", seg)][32 * c:32 * c + 25, :],
                                        rhs=y0s[32 * c:32 * c + 25, :],
                                        start=True, stop=True,
                                        tile_position=(32 * c, 64 * ci))
                                th1 = mlpp.tile([128, CH], F32, tag="y1s",
                                                name="th1")
                                nc.scalar.activation(out=th1[:], in_=ps1[:],
                                                     func=AF.Tanh,
                                                     bias=WS[("b1", seg)][:])
                                th1s.append(th1)
                            for c in range(4):
                                th1 = th1s[c // 2]
                                pb = 64 * (c % 2)
                                ps2 = ps_mlp.tile([128, CH], F32, tag="psA",
                                                  name="ps2", space="PSUM")
                                nc.tensor.matmul(out=ps2[:M, :],
                                                 lhsT=WS[("w2", seg)][pb:pb + 50, :],
                                                 rhs=th1[pb:pb + 50, :],
                                                 start=True, stop=False,
                                                 tile_position=(pb, 0))
                                nc.tensor.matmul(
                                    out=ps2[:M, :],
                                    lhsT=WS[("w2s", seg)][32 * c:32 * c + 25, :],
                                    rhs=y0s[32 * c:32 * c + 25, :],
                                    start=False, stop=True,
                                    tile_position=(32 * c, 0))
                                ps3 = ps_b.tile([128, CH], F32, tag="psB",
                                                name="ps3", space="PSUM")
                                nc.tensor.matmul(out=ps3[:M, :],
                                                 lhsT=cc2_sb[pb:pb + 50, :],
                                                 rhs=th1[pb:pb + 50, :],
                                                 start=True, stop=False,
                                                 tile_position=(pb, 0))
                                nc.tensor.matmul(
                                    out=ps3[:M, :],
                                    lhsT=cc4_sb[32 * c:32 * c + 25, :],
                                    rhs=y0s[32 * c:32 * c + 25, :],
                                    start=False, stop=True,
                                    tile_position=(32 * c, 0))
                                o = (c * ngrp + g) * CH
                                nc.scalar.activation(out=gg[:, o:o + CH],
                                                     in_=ps2[:M, :], func=AF.Tanh,
                                                     bias=WS[("b2", seg)][:])
                                nc.vector.tensor_tensor(out=gg[:, o:o + CH],
                                                        in0=gg[:, o:o + CH],
                                                        in1=ps3[:M, :], op=ALU.add)

                    # contraction
                    t0all = t0p.tile([46, 64, M], F32, tag="t0all", name="t0all")
                    for a0 in range(64):
                        tp = ps_t.tile([128, 128], F32, tag="tpt", name="tp0",
                                       space="PSUM")
                        nc.tensor.transpose(out=tp[:46, :M],
                                            in_=gg_blk[0][:, a0 * 46:(a0 + 1) * 46],
                                            identity=ident[0:M, 0:M])
                        nc.vector.tensor_copy(out=t0all[:, a0, :], in_=tp[0:46, :M])
                    obuf = outp_pool.tile([M, 64, AXIS], F32, tag="obuf",
                                          name="obuf")
                    for a in range(64):
                        tp = ps_t.tile([128, 128], F32, tag="tpt", name="tp1",
                                       space="PSUM")
                        nc.tensor.transpose(out=tp[:92, :M],
                                            in_=gg_blk[1][:, a * 92:(a + 1) * 92],
                                            identity=ident[0:M, 0:M])
                        t1 = tsbp.tile([92, M], F32, tag="t1", name="t1")
                        nc.vector.tensor_copy(out=t1[:], in_=tp[:92, :M])

                        A = blk * 64 + a
                        xyz_ps = ps_small.tile([4, M], F32, tag="small",
                                               name="xyzp", space="PSUM")
                        lhs0 = mkap(rr0, A, [[S, 4]])
                        nc.tensor.matmul(out=xyz_ps[:], lhsT=lhs0,
                                         rhs=t0all[:, a, :], start=True, stop=False)
                        lhs1 = mkap(rr1, A, [[S, 4]])
                        nc.tensor.matmul(out=xyz_ps[:], lhsT=lhs1, rhs=t1[:],
                                         start=False, stop=True)
                        xyz = tsbp.tile([4, M], F32, tag="xyzs", name="xyzs")
                        nc.scalar.activation(out=xyz[:], in_=xyz_ps[:],
                                             func=AF.Copy, scale=1.0 / NNEI)
                        res_ps = ps_small.tile([M, AXIS], F32, tag="small",
                                               name="resp", space="PSUM")
                        nc.tensor.matmul(out=res_ps[:], lhsT=xyz[:],
                                         rhs=xyz[:, 0:AXIS], start=True, stop=True)
                        nc.vector.tensor_copy(out=obuf[:, a, :], in_=res_ps[:])
                    src = mkap(obuf, 0, [[AXIS, 64], [1, AXIS]])
                    dst = bass.AP(res_d, (f * S + blk * 64) * M * AXIS,
                                  [[AXIS, M], [M * AXIS, 64], [1, AXIS]])
                    nc.sync.dma_start(out=dst, in_=src)

    nc.finalize()
    return nc


# ---------------------------------------------------------------------------
# cached dispatch (shard_map over 8 cores, built once)
# ---------------------------------------------------------------------------
_EXEC = None


def _get_exec():
    global _EXEC
    if _EXEC is not None:
        return _EXEC
    import concourse.mybir as mybir
    from concourse.bass2jax import (_bass_exec_p, install_neuronx_cc_hook,
                                    partition_id_tensor)
    from jax.experimental.shard_map import shard_map
    from jax.sharding import Mesh, PartitionSpec

    install_neuronx_cc_hook()
    nc = _build_kernel()

    partition_name = (nc.partition_id_tensor.name
                      if nc.partition_id_tensor else None)
    in_names, out_names, out_avals, zero_shapes = [], [], [], []
    for alloc in nc.m.functions[0].allocations:
        if not isinstance(alloc, mybir.MemoryLocationSet):
            continue
        name = alloc.memorylocations[0].name
        if alloc.kind == "ExternalInput":
            if name != partition_name:
                in_names.append(name)
        elif alloc.kind == "ExternalOutput":
            out_names.append(name)
            shape = tuple(alloc.tensor_shape)
            dtype = mybir.dt.np(alloc.dtype)
            out_avals.append(jax.core.ShapedArray(shape, dtype))
            zero_shapes.append((shape, dtype))
    n_params = len(in_names)
    n_outs = len(out_avals)
    all_in_names = list(in_names) + list(out_names)
    if partition_name is not None:
        all_in_names.append(partition_name)
    donate = tuple(range(n_params, n_params + n_outs))

    def _body(*args):
        operands = list(args)
        if partition_name is not None:
            operands.append(partition_id_tensor())
        outs = _bass_exec_p.bind(
            *operands,
            out_avals=tuple(out_avals),
            in_names=tuple(all_in_names),
            out_names=tuple(out_names),
            lowering_input_output_aliases=(),
            sim_require_finite=True,
            sim_require_nnan=True,
            nc=nc,
        )
        return tuple(outs)

    devices = jax.devices()[:NCORES]
    mesh = Mesh(np.asarray(devices), ("core",))
    in_specs = (PartitionSpec("core"),) * (n_params + n_outs)
    out_specs = (PartitionSpec("core"),) * n_outs
    sharded = jax.jit(
        shard_map(_body, mesh=mesh, in_specs=in_specs, out_specs=out_specs,
                  check_rep=False),
        donate_argnums=donate, keep_unused=True)

    from jax.sharding import NamedSharding
    import jax.numpy as jnp
    shardings = [NamedSharding(mesh, PartitionSpec("core"))] * n_outs

    def _mk_zeros():
        return tuple(jnp.zeros((NCORES * shp[0],) + tuple(shp[1:]), dt)
                     for shp, dt in zero_shapes)
    zero_maker = jax.jit(_mk_zeros, out_shardings=tuple(shardings))
    _EXEC = (sharded, in_names, out_names, zero_maker)
    return _EXEC


# ---------------------------------------------------------------------------
# host-side prep + entry point
# ---------------------------------------------------------------------------
def _prep_global_inputs(nlist, coord, atype, mean, stddev, ws, bs):
    """Build the concatenated (8*dim0, ...) arrays for every DRAM input."""
    g = {}
    nl16 = np.asarray(nlist, dtype=np.int16)
    g["nl"] = np.ascontiguousarray(
        nl16.reshape(NF, NCORES, 128, COLS).transpose(1, 0, 2, 3)
    ).reshape(NCORES * NF, 128, COLS)

    coord = np.asarray(coord, dtype=np.float32)
    coord4 = np.zeros((NF, NALL, 4), np.float32)
    coord4[:, :, 0:3] = coord
    g["coord4_0"] = np.tile(coord4[0], (NCORES, 1))
    g["coord4_1"] = np.tile(coord4[1], (NCORES, 1))

    cat = np.zeros((NF, NLOC, 4), np.float32)
    cat[:, :, 0:3] = coord[:, :NLOC, :]
    cat[:, :, 3] = np.asarray(atype)[:, :NLOC].astype(np.float32)
    g["catype"] = np.ascontiguousarray(
        cat.reshape(NF, NCORES, 128, 16).transpose(1, 0, 2, 3)
    ).reshape(NCORES * NF, 128, 16)

    mean = np.asarray(mean, np.float32)
    stddev = np.asarray(stddev, np.float32)
    istd = 1.0 / stddev
    nmean = -mean / stddev
    nrm = np.stack([
        np.tile(istd[0], (4, 1)),
        np.tile(istd[1] - istd[0], (4, 1)),
        np.tile(nmean[0], (4, 1)),
        np.tile(nmean[1] - nmean[0], (4, 1)),
    ]).astype(ml_dtypes.bfloat16)
    g["nrm"] = np.tile(nrm, (NCORES, 1, 1))

    w0, w1, w2 = [np.asarray(w, np.float32) for w in ws]
    b0, b1, b2 = [np.asarray(b, np.float32) for b in bs]
    w0bd = np.zeros((2, 4, 128), np.float32)
    b0s = np.zeros((2, 128, 1), np.float32)
    w1r = np.zeros((2, 121, 50), np.float32)
    b1s = np.zeros((2, 128, 1), np.float32)
    w2r = np.zeros((2, 114, M), np.float32)
    b2s = np.zeros((2, M, 1), np.float32)
    w2sr = np.zeros((2, 121, M), np.float32)
    for t in range(2):
        w2s = w2[t][0:25] + w2[t][25:50]
        for c in range(4):
            w0bd[t, c, 32 * c:32 * c + 25] = w0[t, 0]
            b0s[t, 32 * c:32 * c + 25, 0] = b0[t]
            w1r[t, 32 * c:32 * c + 25, :] = w1[t]
            w2sr[t, 32 * c:32 * c + 25, :] = w2s
        for h in range(2):
            b1s[t, 64 * h:64 * h + 50, 0] = b1[t]
            w2r[t, 64 * h:64 * h + 50, :] = w2[t]
        b2s[t, :, 0] = b2[t]
    cc2 = np.zeros((50, M), np.float32)
    for i in range(50):
        cc2[i, i] = 1.0
        cc2[i, 50 + i] = 1.0
    cc2r = np.zeros((114, M), np.float32)
    cc2r[0:50] = cc2
    cc2r[64:114] = cc2
    cc4 = np.zeros((25, M), np.float32)
    for i in range(25):
        for k in range(4):
            cc4[i, 25 * k + i] = 1.0
    cc4r = np.zeros((121, M), np.float32)
    for c in range(4):
        cc4r[32 * c:32 * c + 25] = cc4
    for nm, arr in (("w0bd", w0bd), ("b0s", b0s), ("w1r", w1r), ("b1s", b1s),
                    ("w2r", w2r), ("b2s", b2s), ("w2sr", w2sr),
                    ("cc2r", cc2r), ("cc4r", cc4r)):
        g[nm] = np.tile(arr, (NCORES,) + (1,) * (arr.ndim - 1))
    return g


def kernel(nlist, extended_coord, extended_atype, mean, stddev,
           w0, b0, w1, b1, w2, b2):
    sharded, in_names, out_names, zero_maker = _get_exec()
    g = _prep_global_inputs(nlist, extended_coord, extended_atype, mean, stddev,
                            [w0, w1, w2], [b0, b1, b2])
    args = [g[n] for n in in_names]
    zeros = zero_maker()
    out_arrs = sharded(*args, *zeros)
    res = np.asarray(out_arrs[out_names.index("res")])  # [8*NF, S, 1600]
    r = res.reshape(NCORES, NF, S, M * AXIS)
    full = np.empty((NF, NLOC, M * AXIS), np.float32)
    for c in range(NCORES):
        for f in range(NF):
            full[f, c * S:(c + 1) * S, :] = r[c, f]
    return full


# revision 9
# speedup vs baseline: 1.5142x; 1.0037x over previous
"""DescrptSeA descriptor on 8 Trainium2 NeuronCores via a hand-written
Bass/Tile kernel (SPMD over the nloc axis, 512 atoms per core).

Per-core device kernel (see _build_kernel): neighbor-coordinate gather via
indirect DMA, smoothed env matrix, 3-layer embedding net (resnet skips folded
into PE matmuls via PSUM accumulation), per-atom contraction to the
[M*AXIS]=1600 descriptor. Host only casts/reshapes inputs and reassembles the
output; the compiled executable is cached across calls.
"""
import numpy as np
import jax
import ml_dtypes
from concurrent.futures import ThreadPoolExecutor

NF = 2
NLOC, NALL = 4096, 8192
S = 512
NNEI = 138
SEL = [46, 92]
M = 100
AXIS = 16
PROT = 1e-6
RMIN, RMAX = 0.5, 6.0
COLS = 552
CH = 368
NCORES = 8


# ---------------------------------------------------------------------------
# device kernel builder
# ---------------------------------------------------------------------------
def _build_kernel():
    import concourse.bass as bass
    import concourse.bacc as bacc
    import concourse.tile as tile
    from concourse import mybir
    from concourse.masks import make_identity

    F32 = mybir.dt.float32
    I32 = mybir.dt.int32
    BF16 = mybir.dt.bfloat16
    AF = mybir.ActivationFunctionType
    ALU = mybir.AluOpType
    AX = mybir.AxisListType

    def ap_of(t):
        return t[:] if not isinstance(t, bass.AP) else t

    def mkap(t, offset_elems, free_dims, parts=None, part_off=0):
        a = ap_of(t)
        pitch = a.ap[0][0]
        p = [pitch, parts if parts is not None else a.ap[0][1]]
        return bass.AP(a.tensor, a.offset + part_off * pitch + offset_elems,
                       [p] + [list(d) for d in free_dims])

    nc = bacc.Bacc(None, target_bir_lowering=False, debug=False)

    I16 = mybir.dt.int16
    nl_d = nc.dram_tensor("nl", [NF, 128, COLS], I16, kind="ExternalInput")
    coord_ds = [nc.dram_tensor(f"coord4_{f}", [NALL, 4], F32, kind="ExternalInput")
                for f in range(NF)]
    ctr_d = nc.dram_tensor("catype", [NF, 128, 16], F32, kind="ExternalInput")
    nrm_d = nc.dram_tensor("nrm", [4, COLS, 4], BF16, kind="ExternalInput")
    w0_d = nc.dram_tensor("w0bd", [2, 4, 128], F32, kind="ExternalInput")
    b0_d = nc.dram_tensor("b0s", [2, 128, 1], F32, kind="ExternalInput")
    w1_d = nc.dram_tensor("w1r", [2, 121, 50], F32, kind="ExternalInput")
    b1_d = nc.dram_tensor("b1s", [2, 128, 1], F32, kind="ExternalInput")
    w2_d = nc.dram_tensor("w2r", [2, 114, M], F32, kind="ExternalInput")
    b2_d = nc.dram_tensor("b2s", [2, M, 1], F32, kind="ExternalInput")
    w2s_d = nc.dram_tensor("w2sr", [2, 121, M], F32, kind="ExternalInput")
    cc2_d = nc.dram_tensor("cc2r", [114, M], F32, kind="ExternalInput")
    cc4_d = nc.dram_tensor("cc4r", [121, M], F32, kind="ExternalInput")
    res_d = nc.dram_tensor("res", [NF, S, M * AXIS], F32, kind="ExternalOutput")

    with tile.TileContext(nc) as tc:
        with (
            tc.tile_pool(name="const", bufs=1) as constp,
            tc.tile_pool(name="frame", bufs=1) as framep,
            tc.tile_pool(name="gather", bufs=2) as gatherp,
            tc.tile_pool(name="envt", bufs=1) as envp,
            tc.tile_pool(name="ssb", bufs=2) as ssbp,
            tc.tile_pool(name="mlp", bufs=2) as mlpp,
            tc.tile_pool(name="ggp", bufs=1) as ggp,
            tc.tile_pool(name="tsb", bufs=4) as tsbp,
            tc.tile_pool(name="t0p", bufs=1) as t0p,
            tc.tile_pool(name="outp", bufs=1) as outp_pool,
            tc.tile_pool(name="ps_mlp", bufs=3, space="PSUM") as ps_mlp,
            tc.tile_pool(name="ps_b", bufs=1, space="PSUM") as ps_b,
            tc.tile_pool(name="ps_t", bufs=2, space="PSUM") as ps_t,
            tc.tile_pool(name="ps_small", bufs=2, space="PSUM") as ps_small,
        ):
            ident = constp.tile([128, 128], F32)
            make_identity(nc, ident)
            nrm_sb = []
            for i in range(4):
                t = constp.tile([128, COLS, 4], BF16, tag=f"nrm{i}", name=f"nrm{i}")
                src = bass.AP(nrm_d, i * COLS * 4, [[0, 128], [4, COLS], [1, 4]])
                nc.sync.dma_start(out=t[:], in_=src)
                nrm_sb.append(t)
            nsc0_sb, nscd_sb, nsh0_sb, nshd_sb = nrm_sb
            WS = {}
            for t in range(2):
                for nm, d, shp in (("w0", w0_d, [4, 128]), ("b0", b0_d, [128, 1]),
                                   ("w1", w1_d, [121, 50]), ("b1", b1_d, [128, 1]),
                                   ("w2", w2_d, [114, M]), ("b2", b2_d, [M, 1]),
                                   ("w2s", w2s_d, [121, M])):
                    tl = constp.tile(shp, F32, tag=f"{nm}_{t}", name=f"{nm}_{t}")
                    nc.sync.dma_start(out=tl[:], in_=d[t])
                    WS[(nm, t)] = tl
            cc2_sb = constp.tile([114, M], F32, tag="cc2", name="cc2")
            nc.sync.dma_start(out=cc2_sb[:], in_=cc2_d[:])
            cc4_sb = constp.tile([121, M], F32, tag="cc4", name="cc4")
            nc.sync.dma_start(out=cc4_sb[:], in_=cc4_d[:])

            for f in range(NF):
                # ---- Phase E: env matrix (chunked layout) ----
                it = gatherp.tile([128, COLS], I16, tag="it", name="it")
                nc.sync.dma_start(out=it[:], in_=nl_d[f])
                mask = envp.tile([128, COLS], F32, tag="mask", name="mask")
                nc.vector.tensor_scalar(out=mask[:], in0=it[:], scalar1=0,
                                        scalar2=None, op0=ALU.is_ge)
                itc = gatherp.tile([128, COLS], I32, tag="itc", name="itc")
                nc.vector.tensor_scalar(out=itc[:], in0=it[:], scalar1=0,
                                        scalar2=None, op0=ALU.max)
                gt = gatherp.tile([128, COLS, 4], F32, tag="gt", name="gt")
                for k in range(COLS):
                    nc.gpsimd.indirect_dma_start(
                        out=gt[:, k, :], out_offset=None, in_=coord_ds[f][:],
                        in_offset=bass.IndirectOffsetOnAxis(ap=itc[:, k:k + 1],
                                                            axis=0),
                    )
                ctr = framep.tile([128, 4, 4], F32, tag="ctr", name="ctr")
                nc.sync.dma_start(out=ctr[:],
                                  in_=ctr_d[f].rearrange("p (q c) -> p q c", q=4))

                diff = envp.tile([128, COLS, 3], F32, tag="diff", name="diff")
                ctr_b = mkap(ctr, 0, [[4, 4], [0, NNEI], [1, 3]])
                nc.vector.tensor_tensor(out=diff[:], in0=gt[:, :, 0:3], in1=ctr_b,
                                        op=ALU.subtract)
                sq = envp.tile([128, COLS, 3], F32, tag="sq", name="sq")
                nc.vector.tensor_tensor(out=sq[:], in0=diff[:], in1=diff[:],
                                        op=ALU.mult)
                r2 = envp.tile([128, COLS, 1], F32, tag="r2", name="r2")
                nc.vector.tensor_reduce(out=r2[:], in_=sq[:], axis=AX.X, op=ALU.add)
                r = envp.tile([128, COLS], F32, tag="r", name="r")
                nc.scalar.activation(out=r[:], in_=r2[:, :, 0], func=AF.Sqrt)
                sr = envp.tile([128, COLS], F32, tag="sr", name="sr")
                nc.vector.tensor_scalar(out=sr[:], in0=r[:], scalar1=PROT,
                                        scalar2=None, op0=ALU.add)
                nc.vector.reciprocal(out=sr[:], in_=sr[:])
                sr2 = envp.tile([128, COLS], F32, tag="sr2", name="sr2")
                nc.vector.tensor_tensor(out=sr2[:], in0=sr[:], in1=sr[:], op=ALU.mult)
                uu = envp.tile([128, COLS], F32, tag="uu", name="uu")
                nc.vector.tensor_scalar(out=uu[:], in0=r[:], scalar1=-RMIN,
                                        scalar2=1.0 / (RMAX - RMIN),
                                        op0=ALU.add, op1=ALU.mult)
                nc.vector.tensor_scalar(out=uu[:], in0=uu[:], scalar1=0.0,
                                        scalar2=1.0, op0=ALU.max, op1=ALU.min)
                u2 = envp.tile([128, COLS], F32, tag="u2", name="u2")
                nc.vector.tensor_tensor(out=u2[:], in0=uu[:], in1=uu[:], op=ALU.mult)
                nc.vector.tensor_tensor(out=u2[:], in0=u2[:], in1=uu[:], op=ALU.mult)
                p1 = envp.tile([128, COLS], F32, tag="p1", name="p1")
                nc.vector.tensor_scalar(out=p1[:], in0=uu[:], scalar1=-6.0,
                                        scalar2=15.0, op0=ALU.mult, op1=ALU.add)
                nc.vector.tensor_tensor(out=p1[:], in0=p1[:], in1=uu[:], op=ALU.mult)
                nc.vector.tensor_scalar(out=p1[:], in0=p1[:], scalar1=-10.0,
                                        scalar2=None, op0=ALU.add)
                nc.vector.tensor_tensor(out=p1[:], in0=p1[:], in1=u2[:], op=ALU.mult)
                nc.vector.tensor_scalar(out=p1[:], in0=p1[:], scalar1=1.0,
                                        scalar2=None, op0=ALU.add)
                wm = envp.tile([128, COLS], F32, tag="wm", name="wm")
                nc.vector.tensor_tensor(out=wm[:], in0=p1[:], in1=mask[:],
                                        op=ALU.mult)

                envw = framep.tile([128, COLS, 4], F32, tag="envw", name="envw")
                nc.vector.tensor_copy(out=envw[:, :, 0], in_=sr[:])
                sr2_b = mkap(sr2, 0, [[1, COLS], [0, 3]])
                nc.vector.tensor_tensor(out=envw[:, :, 1:4], in0=diff[:], in1=sr2_b,
                                        op=ALU.mult)
                tpt = envp.tile([128, COLS], F32, tag="r2", name="tpt")
                tpt_src = mkap(ctr, 3, [[4, 4], [0, NNEI]])
                nc.vector.tensor_copy(out=tpt[:], in_=tpt_src)
                tpt_b = mkap(tpt, 0, [[1, COLS], [0, 4]])
                wm_b = mkap(wm, 0, [[1, COLS], [0, 4]])
                x1 = envp.tile([128, COLS, 4], F32, tag="sq", name="x1")
                nc.vector.tensor_tensor(out=x1[:], in0=nscd_sb[:], in1=tpt_b,
                                        op=ALU.mult)
                nc.vector.tensor_tensor(out=x1[:], in0=x1[:], in1=nsc0_sb[:],
                                        op=ALU.add)
                nc.vector.tensor_tensor(out=x1[:], in0=x1[:], in1=wm_b, op=ALU.mult)
                dm = framep.tile([128, COLS, 4], F32, tag="dm", name="dm")
                nc.vector.tensor_tensor(out=dm[:], in0=envw[:], in1=x1[:],
                                        op=ALU.mult)
                y1 = envp.tile([128, COLS, 4], F32, tag="diff", name="y1")
                nc.vector.tensor_tensor(out=y1[:], in0=nshd_sb[:], in1=tpt_b,
                                        op=ALU.mult)
                nc.vector.tensor_tensor(out=y1[:], in0=y1[:], in1=nsh0_sb[:],
                                        op=ALU.add)
                nc.vector.tensor_tensor(out=dm[:], in0=dm[:], in1=y1[:], op=ALU.add)

                # ---- Phase T: rr to slot-major [sel, 4, S] ----
                rr0 = framep.tile([SEL[0], 4, S], F32, tag="rr0", name="rr0")
                rr1 = framep.tile([SEL[1], 4, S], F32, tag="rr1", name="rr1")
                for q in range(4):
                    for ch in range(4):
                        for rr_sb, j0, sel in ((rr0, 0, SEL[0]),
                                               (rr1, SEL[0], SEL[1])):
                            src = mkap(dm, (q * NNEI + j0) * 4 + ch, [[4, sel]])
                            tp = ps_t.tile([128, 128], F32, tag="tpt", name="tpq",
                                           space="PSUM")
                            nc.tensor.transpose(out=tp[:sel, :], in_=src,
                                                identity=ident[:])
                            dst = mkap(rr_sb, ch * S + q, [[4, 128]])
                            nc.vector.tensor_copy(out=dst, in_=tp[:sel, :])

                ssc = framep.tile([128, COLS], F32, tag="ssc", name="ssc")
                nc.vector.tensor_copy(out=ssc[:], in_=dm[:, :, 0])

                # ---- per 64-atom block: MLP + contraction ----
                for blk in range(8):
                    ss_t = {}
                    for seg, (sel, ngrp) in enumerate(((SEL[0], 2), (SEL[1], 4))):
                        sst = ssbp.tile([4, ngrp * CH], F32, tag=f"ss{seg}",
                                        name=f"ss{seg}")
                        j0 = 0 if seg == 0 else SEL[0]
                        src = mkap(ssc, j0, [[NNEI, 4], [1, sel]],
                                   parts=16, part_off=16 * blk)
                        dst = mkap(sst, 0, [[CH, ngrp], [1, CH]])
                        nc.sync.dma_start(out=dst, in_=src)
                        ss_t[seg] = (sst, ngrp, sel)

                    gg_blk = {}
                    for seg in (0, 1):
                        sst, ngrp, sel = ss_t[seg]
                        gg = ggp.tile([M, 64 * sel], F32, tag=f"gg{seg}",
                                      name=f"gg{seg}")
                        gg_blk[seg] = gg
                        for g in range(ngrp):
                            ps0 = ps_mlp.tile([128, CH], F32, tag="psA", name="ps0",
                                              space="PSUM")
                            nc.tensor.matmul(out=ps0[:], lhsT=WS[("w0", seg)][:],
                                             rhs=sst[:, g * CH:(g + 1) * CH],
                                             start=True, stop=True,
                                             tile_position=(0, 0))
                            y0s = mlpp.tile([128, CH], F32, tag="y0s", name="y0s")
                            nc.scalar.activation(out=y0s[:], in_=ps0[:],
                                                 func=AF.Tanh,
                                                 bias=WS[("b0", seg)][:])
                            th1s = []
                            for half in range(2):
                                ps1 = ps_mlp.tile([128, CH], F32, tag="psA",
                                                  name="ps1", space="PSUM")
                                for ci in range(2):
                                    c = half * 2 + ci
                                    nc.tensor.matmul(
                                        out=ps1[64 * ci:64 * ci + 50, :],
                                        lhsT=WS[("w1", seg)][32 * c:32 * c + 25, :],
                                        rhs=y0s[32 * c:32 * c + 25, :],
                                        start=True, stop=True,
                                        tile_position=(32 * c, 64 * ci))
                                th1 = mlpp.tile([128, CH], F32, tag="y1s",
                                                name="th1")
                                nc.scalar.activation(out=th1[:], in_=ps1[:],
                                                     func=AF.Tanh,
                                                     bias=WS[("b1", seg)][:])
                                th1s.append(th1)
                            for c in range(4):
                                th1 = th1s[c // 2]
                                pb = 64 * (c % 2)
                                ps2 = ps_mlp.tile([128, CH], F32, tag="psA",
                                                  name="ps2", space="PSUM")
                                nc.tensor.matmul(out=ps2[:M, :],
                                                 lhsT=WS[("w2", seg)][pb:pb + 50, :],
                                                 rhs=th1[pb:pb + 50, :],
                                                 start=True, stop=False,
                                                 tile_position=(pb, 0))
                                nc.tensor.matmul(
                                    out=ps2[:M, :],
                                    lhsT=WS[("w2s", seg)][32 * c:32 * c + 25, :],
                                    rhs=y0s[32 * c:32 * c + 25, :],
                                    start=False, stop=True,
                                    tile_position=(32 * c, 0))
                                ps3 = ps_b.tile([128, CH], F32, tag="psB",
                                                name="ps3", space="PSUM")
                                nc.tensor.matmul(out=ps3[:M, :],
                                                 lhsT=cc2_sb[pb:pb + 50, :],
                                                 rhs=th1[pb:pb + 50, :],
                                                 start=True, stop=False,
                                                 tile_position=(pb, 0))
                                nc.tensor.matmul(
                                    out=ps3[:M, :],
                                    lhsT=cc4_sb[32 * c:32 * c + 25, :],
                                    rhs=y0s[32 * c:32 * c + 25, :],
                                    start=False, stop=True,
                                    tile_position=(32 * c, 0))
                                o = (c * ngrp + g) * CH
                                nc.scalar.activation(out=gg[:, o:o + CH],
                                                     in_=ps2[:M, :], func=AF.Tanh,
                                                     bias=WS[("b2", seg)][:])
                                nc.vector.tensor_tensor(out=gg[:, o:o + CH],
                                                        in0=gg[:, o:o + CH],
                                                        in1=ps3[:M, :], op=ALU.add)

                    # contraction
                    t0all = t0p.tile([46, 64, M], F32, tag="t0all", name="t0all")
                    for a0 in range(64):
                        tp = ps_t.tile([128, 128], F32, tag="tpt", name="tp0",
                                       space="PSUM")
                        nc.tensor.transpose(out=tp[:46, :M],
                                            in_=gg_blk[0][:, a0 * 46:(a0 + 1) * 46],
                                            identity=ident[0:M, 0:M])
                        nc.vector.tensor_copy(out=t0all[:, a0, :], in_=tp[0:46, :M])
                    obuf = outp_pool.tile([M, 64, AXIS], F32, tag="obuf",
                                          name="obuf")
                    for a in range(64):
                        tp = ps_t.tile([128, 128], F32, tag="tpt", name="tp1",
                                       space="PSUM")
                        nc.tensor.transpose(out=tp[:92, :M],
                                            in_=gg_blk[1][:, a * 92:(a + 1) * 92],
                                            identity=ident[0:M, 0:M])
                        t1 = tsbp.tile([92, M], F32, tag="t1", name="t1")
                        nc.vector.tensor_copy(out=t1[:], in_=tp[:92, :M])

                        A = blk * 64 + a
                        xyz_ps = ps_small.tile([4, M], F32, tag="small",
                                               name="xyzp", space="PSUM")
                        lhs0 = mkap(rr0, A, [[S, 4]])
                        nc.tensor.matmul(out=xyz_ps[:], lhsT=lhs0,
                                         rhs=t0all[:, a, :], start=True, stop=False)
                        lhs1 = mkap(rr1, A, [[S, 4]])
                        nc.tensor.matmul(out=xyz_ps[:], lhsT=lhs1, rhs=t1[:],
                                         start=False, stop=True)
                        xyz = tsbp.tile([4, M], F32, tag="xyzs", name="xyzs")
                        nc.scalar.activation(out=xyz[:], in_=xyz_ps[:],
                                             func=AF.Copy, scale=1.0 / NNEI)
                        res_ps = ps_small.tile([M, AXIS], F32, tag="small",
                                               name="resp", space="PSUM")
                        nc.tensor.matmul(out=res_ps[:], lhsT=xyz[:],
                                         rhs=xyz[:, 0:AXIS], start=True, stop=True)
                        nc.vector.tensor_copy(out=obuf[:, a, :], in_=res_ps[:])
                    src = mkap(obuf, 0, [[AXIS, 64], [1, AXIS]])
                    dst = bass.AP(res_d, (f * S + blk * 64) * M * AXIS,
                                  [[AXIS, M], [M * AXIS, 64], [1, AXIS]])
                    nc.sync.dma_start(out=dst, in_=src)

    nc.finalize()
    return nc


# ---------------------------------------------------------------------------
# cached dispatch (shard_map over 8 cores, built once)
# ---------------------------------------------------------------------------
_EXEC = None
_OUT_BUF = None


def _get_exec():
    global _EXEC
    if _EXEC is not None:
        return _EXEC
    import concourse.mybir as mybir
    from concourse.bass2jax import (_bass_exec_p, install_neuronx_cc_hook,
                                    partition_id_tensor)
    from jax.experimental.shard_map import shard_map
    from jax.sharding import Mesh, PartitionSpec

    install_neuronx_cc_hook()
    nc = _build_kernel()

    partition_name = (nc.partition_id_tensor.name
                      if nc.partition_id_tensor else None)
    in_names, out_names, out_avals, zero_shapes = [], [], [], []
    for alloc in nc.m.functions[0].allocations:
        if not isinstance(alloc, mybir.MemoryLocationSet):
            continue
        name = alloc.memorylocations[0].name
        if alloc.kind == "ExternalInput":
            if name != partition_name:
                in_names.append(name)
        elif alloc.kind == "ExternalOutput":
            out_names.append(name)
            shape = tuple(alloc.tensor_shape)
            dtype = mybir.dt.np(alloc.dtype)
            out_avals.append(jax.core.ShapedArray(shape, dtype))
            zero_shapes.append((shape, dtype))
    n_params = len(in_names)
    n_outs = len(out_avals)
    all_in_names = list(in_names) + list(out_names)
    if partition_name is not None:
        all_in_names.append(partition_name)
    donate = tuple(range(n_params, n_params + n_outs))

    def _body(*args):
        operands = list(args)
        if partition_name is not None:
            operands.append(partition_id_tensor())
        outs = _bass_exec_p.bind(
            *operands,
            out_avals=tuple(out_avals),
            in_names=tuple(all_in_names),
            out_names=tuple(out_names),
            lowering_input_output_aliases=(),
            sim_require_finite=True,
            sim_require_nnan=True,
            nc=nc,
        )
        return tuple(outs)

    devices = jax.devices()[:NCORES]
    mesh = Mesh(np.asarray(devices), ("core",))
    in_specs = (PartitionSpec("core"),) * (n_params + n_outs)
    out_specs = (PartitionSpec("core"),) * n_outs
    sharded = jax.jit(
        shard_map(_body, mesh=mesh, in_specs=in_specs, out_specs=out_specs,
                  check_rep=False),
        donate_argnums=donate, keep_unused=True)

    from jax.sharding import NamedSharding
    import jax.numpy as jnp
    shardings = [NamedSharding(mesh, PartitionSpec("core"))] * n_outs

    def _mk_zeros():
        return tuple(jnp.zeros((NCORES * shp[0],) + tuple(shp[1:]), dt)
                     for shp, dt in zero_shapes)
    zero_maker = jax.jit(_mk_zeros, out_shardings=tuple(shardings))
    _EXEC = (sharded, in_names, out_names, zero_maker)
    return _EXEC


# ---------------------------------------------------------------------------
# host-side prep + entry point
# ---------------------------------------------------------------------------
def _prep_global_inputs(nlist, coord, atype, mean, stddev, ws, bs):
    """Build the concatenated (8*dim0, ...) arrays for every DRAM input."""
    g = {}
    nl16 = np.asarray(nlist, dtype=np.int16)
    g["nl"] = np.ascontiguousarray(
        nl16.reshape(NF, NCORES, 128, COLS).transpose(1, 0, 2, 3)
    ).reshape(NCORES * NF, 128, COLS)

    coord = np.asarray(coord, dtype=np.float32)
    coord4 = np.zeros((NF, NALL, 4), np.float32)
    coord4[:, :, 0:3] = coord
    g["coord4_0"] = np.tile(coord4[0], (NCORES, 1))
    g["coord4_1"] = np.tile(coord4[1], (NCORES, 1))

    cat = np.zeros((NF, NLOC, 4), np.float32)
    cat[:, :, 0:3] = coord[:, :NLOC, :]
    cat[:, :, 3] = np.asarray(atype)[:, :NLOC].astype(np.float32)
    g["catype"] = np.ascontiguousarray(
        cat.reshape(NF, NCORES, 128, 16).transpose(1, 0, 2, 3)
    ).reshape(NCORES * NF, 128, 16)

    mean = np.asarray(mean, np.float32)
    stddev = np.asarray(stddev, np.float32)
    istd = 1.0 / stddev
    nmean = -mean / stddev
    nrm = np.stack([
        np.tile(istd[0], (4, 1)),
        np.tile(istd[1] - istd[0], (4, 1)),
        np.tile(nmean[0], (4, 1)),
        np.tile(nmean[1] - nmean[0], (4, 1)),
    ]).astype(ml_dtypes.bfloat16)
    g["nrm"] = np.tile(nrm, (NCORES, 1, 1))

    w0, w1, w2 = [np.asarray(w, np.float32) for w in ws]
    b0, b1, b2 = [np.asarray(b, np.float32) for b in bs]
    w0bd = np.zeros((2, 4, 128), np.float32)
    b0s = np.zeros((2, 128, 1), np.float32)
    w1r = np.zeros((2, 121, 50), np.float32)
    b1s = np.zeros((2, 128, 1), np.float32)
    w2r = np.zeros((2, 114, M), np.float32)
    b2s = np.zeros((2, M, 1), np.float32)
    w2sr = np.zeros((2, 121, M), np.float32)
    for t in range(2):
        w2s = w2[t][0:25] + w2[t][25:50]
        for c in range(4):
            w0bd[t, c, 32 * c:32 * c + 25] = w0[t, 0]
            b0s[t, 32 * c:32 * c + 25, 0] = b0[t]
            w1r[t, 32 * c:32 * c + 25, :] = w1[t]
            w2sr[t, 32 * c:32 * c + 25, :] = w2s
        for h in range(2):
            b1s[t, 64 * h:64 * h + 50, 0] = b1[t]
            w2r[t, 64 * h:64 * h + 50, :] = w2[t]
        b2s[t, :, 0] = b2[t]
    cc2 = np.zeros((50, M), np.float32)
    for i in range(50):
        cc2[i, i] = 1.0
        cc2[i, 50 + i] = 1.0
    cc2r = np.zeros((114, M), np.float32)
    cc2r[0:50] = cc2
    cc2r[64:114] = cc2
    cc4 = np.zeros((25, M), np.float32)
    for i in range(25):
        for k in range(4):
            cc4[i, 25 * k + i] = 1.0
    cc4r = np.zeros((121, M), np.float32)
    for c in range(4):
        cc4r[32 * c:32 * c + 25] = cc4
    for nm, arr in (("w0bd", w0bd), ("b0s", b0s), ("w1r", w1r), ("b1s", b1s),
                    ("w2r", w2r), ("b2s", b2s), ("w2sr", w2sr),
                    ("cc2r", cc2r), ("cc4r", cc4r)):
        g[nm] = np.tile(arr, (NCORES,) + (1,) * (arr.ndim - 1))
    return g


def kernel(nlist, extended_coord, extended_atype, mean, stddev,
           w0, b0, w1, b1, w2, b2):
    sharded, in_names, out_names, zero_maker = _get_exec()
    g = _prep_global_inputs(nlist, extended_coord, extended_atype, mean, stddev,
                            [w0, w1, w2], [b0, b1, b2])
    args = [g[n] for n in in_names]
    zeros = zero_maker()
    out_arrs = sharded(*args, *zeros)
    res = np.asarray(out_arrs[out_names.index("res")])  # [8*NF, S, 1600]
    r = res.reshape(NCORES, NF, S, M * AXIS)
    global _OUT_BUF
    if _OUT_BUF is None:
        _OUT_BUF = np.empty((NF, NLOC, M * AXIS), np.float32)
    full = _OUT_BUF
    for c in range(NCORES):
        for f in range(NF):
            np.copyto(full[f, c * S:(c + 1) * S, :], r[c, f])
    return full


# revision 10
# speedup vs baseline: 2.7551x; 1.8195x over previous
"""DescrptSeA descriptor on 8 Trainium2 NeuronCores via a hand-written
Bass/Tile kernel (SPMD over the nloc axis, 512 atoms per core).

Per-core device kernel (see _build_kernel): neighbor-coordinate gather via
indirect DMA, smoothed env matrix, 3-layer embedding net (resnet skips folded
into PE matmuls via PSUM accumulation), per-atom contraction to the
[M*AXIS]=1600 descriptor. Host only casts/reshapes inputs and reassembles the
output; the compiled executable is cached across calls.
"""
import numpy as np
import jax
import ml_dtypes
from concurrent.futures import ThreadPoolExecutor

NF = 2
NLOC, NALL = 4096, 8192
S = 512
NNEI = 138
SEL = [46, 92]
M = 100
AXIS = 16
PROT = 1e-6
RMIN, RMAX = 0.5, 6.0
COLS = 552
CH = 368
NCORES = 8


# ---------------------------------------------------------------------------
# device kernel builder
# ---------------------------------------------------------------------------
def _build_kernel():
    import concourse.bass as bass
    import concourse.bacc as bacc
    import concourse.tile as tile
    from concourse import mybir
    from concourse.masks import make_identity

    F32 = mybir.dt.float32
    I32 = mybir.dt.int32
    BF16 = mybir.dt.bfloat16
    AF = mybir.ActivationFunctionType
    ALU = mybir.AluOpType
    AX = mybir.AxisListType

    def ap_of(t):
        return t[:] if not isinstance(t, bass.AP) else t

    def mkap(t, offset_elems, free_dims, parts=None, part_off=0):
        a = ap_of(t)
        pitch = a.ap[0][0]
        p = [pitch, parts if parts is not None else a.ap[0][1]]
        return bass.AP(a.tensor, a.offset + part_off * pitch + offset_elems,
                       [p] + [list(d) for d in free_dims])

    nc = bacc.Bacc(None, target_bir_lowering=False, debug=False)

    I16 = mybir.dt.int16
    nl_d = nc.dram_tensor("nl", [NF, 128, COLS], I16, kind="ExternalInput")
    coord_ds = [nc.dram_tensor(f"coord4_{f}", [NALL, 4], F32, kind="ExternalInput")
                for f in range(NF)]
    ctr_d = nc.dram_tensor("catype", [NF, 128, 16], F32, kind="ExternalInput")
    nrm_d = nc.dram_tensor("nrm", [4, COLS, 4], BF16, kind="ExternalInput")
    w0_d = nc.dram_tensor("w0bd", [2, 4, 128], F32, kind="ExternalInput")
    b0_d = nc.dram_tensor("b0s", [2, 128, 1], F32, kind="ExternalInput")
    w1_d = nc.dram_tensor("w1r", [2, 121, 50], F32, kind="ExternalInput")
    b1_d = nc.dram_tensor("b1s", [2, 128, 1], F32, kind="ExternalInput")
    w2_d = nc.dram_tensor("w2r", [2, 114, M], F32, kind="ExternalInput")
    b2_d = nc.dram_tensor("b2s", [2, M, 1], F32, kind="ExternalInput")
    w2s_d = nc.dram_tensor("w2sr", [2, 121, M], F32, kind="ExternalInput")
    cc2_d = nc.dram_tensor("cc2r", [114, M], F32, kind="ExternalInput")
    cc4_d = nc.dram_tensor("cc4r", [121, M], F32, kind="ExternalInput")
    res_d = nc.dram_tensor("res", [NF, S, M * AXIS], BF16, kind="ExternalOutput")

    with tile.TileContext(nc) as tc:
        with (
            tc.tile_pool(name="const", bufs=1) as constp,
            tc.tile_pool(name="frame", bufs=1) as framep,
            tc.tile_pool(name="gather", bufs=2) as gatherp,
            tc.tile_pool(name="envt", bufs=1) as envp,
            tc.tile_pool(name="ssb", bufs=2) as ssbp,
            tc.tile_pool(name="mlp", bufs=2) as mlpp,
            tc.tile_pool(name="ggp", bufs=1) as ggp,
            tc.tile_pool(name="tsb", bufs=4) as tsbp,
            tc.tile_pool(name="t0p", bufs=1) as t0p,
            tc.tile_pool(name="outp", bufs=1) as outp_pool,
            tc.tile_pool(name="ps_mlp", bufs=3, space="PSUM") as ps_mlp,
            tc.tile_pool(name="ps_b", bufs=1, space="PSUM") as ps_b,
            tc.tile_pool(name="ps_t", bufs=2, space="PSUM") as ps_t,
            tc.tile_pool(name="ps_small", bufs=2, space="PSUM") as ps_small,
        ):
            ident = constp.tile([128, 128], F32)
            make_identity(nc, ident)
            nrm_sb = []
            for i in range(4):
                t = constp.tile([128, COLS, 4], BF16, tag=f"nrm{i}", name=f"nrm{i}")
                src = bass.AP(nrm_d, i * COLS * 4, [[0, 128], [4, COLS], [1, 4]])
                nc.sync.dma_start(out=t[:], in_=src)
                nrm_sb.append(t)
            nsc0_sb, nscd_sb, nsh0_sb, nshd_sb = nrm_sb
            WS = {}
            for t in range(2):
                for nm, d, shp in (("w0", w0_d, [4, 128]), ("b0", b0_d, [128, 1]),
                                   ("w1", w1_d, [121, 50]), ("b1", b1_d, [128, 1]),
                                   ("w2", w2_d, [114, M]), ("b2", b2_d, [M, 1]),
                                   ("w2s", w2s_d, [121, M])):
                    tl = constp.tile(shp, F32, tag=f"{nm}_{t}", name=f"{nm}_{t}")
                    nc.sync.dma_start(out=tl[:], in_=d[t])
                    WS[(nm, t)] = tl
            cc2_sb = constp.tile([114, M], F32, tag="cc2", name="cc2")
            nc.sync.dma_start(out=cc2_sb[:], in_=cc2_d[:])
            cc4_sb = constp.tile([121, M], F32, tag="cc4", name="cc4")
            nc.sync.dma_start(out=cc4_sb[:], in_=cc4_d[:])

            for f in range(NF):
                # ---- Phase E: env matrix (chunked layout) ----
                it = gatherp.tile([128, COLS], I16, tag="it", name="it")
                nc.sync.dma_start(out=it[:], in_=nl_d[f])
                mask = envp.tile([128, COLS], F32, tag="mask", name="mask")
                nc.vector.tensor_scalar(out=mask[:], in0=it[:], scalar1=0,
                                        scalar2=None, op0=ALU.is_ge)
                itc = gatherp.tile([128, COLS], I32, tag="itc", name="itc")
                nc.vector.tensor_scalar(out=itc[:], in0=it[:], scalar1=0,
                                        scalar2=None, op0=ALU.max)
                gt = gatherp.tile([128, COLS, 4], F32, tag="gt", name="gt")
                for k in range(COLS):
                    nc.gpsimd.indirect_dma_start(
                        out=gt[:, k, :], out_offset=None, in_=coord_ds[f][:],
                        in_offset=bass.IndirectOffsetOnAxis(ap=itc[:, k:k + 1],
                                                            axis=0),
                    )
                ctr = framep.tile([128, 4, 4], F32, tag="ctr", name="ctr")
                nc.sync.dma_start(out=ctr[:],
                                  in_=ctr_d[f].rearrange("p (q c) -> p q c", q=4))

                diff = envp.tile([128, COLS, 3], F32, tag="diff", name="diff")
                ctr_b = mkap(ctr, 0, [[4, 4], [0, NNEI], [1, 3]])
                nc.vector.tensor_tensor(out=diff[:], in0=gt[:, :, 0:3], in1=ctr_b,
                                        op=ALU.subtract)
                sq = envp.tile([128, COLS, 3], F32, tag="sq", name="sq")
                nc.vector.tensor_tensor(out=sq[:], in0=diff[:], in1=diff[:],
                                        op=ALU.mult)
                r2 = envp.tile([128, COLS, 1], F32, tag="r2", name="r2")
                nc.vector.tensor_reduce(out=r2[:], in_=sq[:], axis=AX.X, op=ALU.add)
                r = envp.tile([128, COLS], F32, tag="r", name="r")
                nc.scalar.activation(out=r[:], in_=r2[:, :, 0], func=AF.Sqrt)
                sr = envp.tile([128, COLS], F32, tag="sr", name="sr")
                nc.vector.tensor_scalar(out=sr[:], in0=r[:], scalar1=PROT,
                                        scalar2=None, op0=ALU.add)
                nc.vector.reciprocal(out=sr[:], in_=sr[:])
                sr2 = envp.tile([128, COLS], F32, tag="sr2", name="sr2")
                nc.vector.tensor_tensor(out=sr2[:], in0=sr[:], in1=sr[:], op=ALU.mult)
                uu = envp.tile([128, COLS], F32, tag="uu", name="uu")
                nc.vector.tensor_scalar(out=uu[:], in0=r[:], scalar1=-RMIN,
                                        scalar2=1.0 / (RMAX - RMIN),
                                        op0=ALU.add, op1=ALU.mult)
                nc.vector.tensor_scalar(out=uu[:], in0=uu[:], scalar1=0.0,
                                        scalar2=1.0, op0=ALU.max, op1=ALU.min)
                u2 = envp.tile([128, COLS], F32, tag="u2", name="u2")
                nc.vector.tensor_tensor(out=u2[:], in0=uu[:], in1=uu[:], op=ALU.mult)
                nc.vector.tensor_tensor(out=u2[:], in0=u2[:], in1=uu[:], op=ALU.mult)
                p1 = envp.tile([128, COLS], F32, tag="p1", name="p1")
                nc.vector.tensor_scalar(out=p1[:], in0=uu[:], scalar1=-6.0,
                                        scalar2=15.0, op0=ALU.mult, op1=ALU.add)
                nc.vector.tensor_tensor(out=p1[:], in0=p1[:], in1=uu[:], op=ALU.mult)
                nc.vector.tensor_scalar(out=p1[:], in0=p1[:], scalar1=-10.0,
                                        scalar2=None, op0=ALU.add)
                nc.vector.tensor_tensor(out=p1[:], in0=p1[:], in1=u2[:], op=ALU.mult)
                nc.vector.tensor_scalar(out=p1[:], in0=p1[:], scalar1=1.0,
                                        scalar2=None, op0=ALU.add)
                wm = envp.tile([128, COLS], F32, tag="wm", name="wm")
                nc.vector.tensor_tensor(out=wm[:], in0=p1[:], in1=mask[:],
                                        op=ALU.mult)

                envw = framep.tile([128, COLS, 4], F32, tag="envw", name="envw")
                nc.vector.tensor_copy(out=envw[:, :, 0], in_=sr[:])
                sr2_b = mkap(sr2, 0, [[1, COLS], [0, 3]])
                nc.vector.tensor_tensor(out=envw[:, :, 1:4], in0=diff[:], in1=sr2_b,
                                        op=ALU.mult)
                tpt = envp.tile([128, COLS], F32, tag="r2", name="tpt")
                tpt_src = mkap(ctr, 3, [[4, 4], [0, NNEI]])
                nc.vector.tensor_copy(out=tpt[:], in_=tpt_src)
                tpt_b = mkap(tpt, 0, [[1, COLS], [0, 4]])
                wm_b = mkap(wm, 0, [[1, COLS], [0, 4]])
                x1 = envp.tile([128, COLS, 4], F32, tag="sq", name="x1")
                nc.vector.tensor_tensor(out=x1[:], in0=nscd_sb[:], in1=tpt_b,
                                        op=ALU.mult)
                nc.vector.tensor_tensor(out=x1[:], in0=x1[:], in1=nsc0_sb[:],
                                        op=ALU.add)
                nc.vector.tensor_tensor(out=x1[:], in0=x1[:], in1=wm_b, op=ALU.mult)
                dm = framep.tile([128, COLS, 4], F32, tag="dm", name="dm")
                nc.vector.tensor_tensor(out=dm[:], in0=envw[:], in1=x1[:],
                                        op=ALU.mult)
                y1 = envp.tile([128, COLS, 4], F32, tag="diff", name="y1")
                nc.vector.tensor_tensor(out=y1[:], in0=nshd_sb[:], in1=tpt_b,
                                        op=ALU.mult)
                nc.vector.tensor_tensor(out=y1[:], in0=y1[:], in1=nsh0_sb[:],
                                        op=ALU.add)
                nc.vector.tensor_tensor(out=dm[:], in0=dm[:], in1=y1[:], op=ALU.add)

                # ---- Phase T: rr to slot-major [sel, 4, S] ----
                rr0 = framep.tile([SEL[0], 4, S], F32, tag="rr0", name="rr0")
                rr1 = framep.tile([SEL[1], 4, S], F32, tag="rr1", name="rr1")
                for q in range(4):
                    for ch in range(4):
                        for rr_sb, j0, sel in ((rr0, 0, SEL[0]),
                                               (rr1, SEL[0], SEL[1])):
                            src = mkap(dm, (q * NNEI + j0) * 4 + ch, [[4, sel]])
                            tp = ps_t.tile([128, 128], F32, tag="tpt", name="tpq",
                                           space="PSUM")
                            nc.tensor.transpose(out=tp[:sel, :], in_=src,
                                                identity=ident[:])
                            dst = mkap(rr_sb, ch * S + q, [[4, 128]])
                            nc.vector.tensor_copy(out=dst, in_=tp[:sel, :])

                ssc = framep.tile([128, COLS], F32, tag="ssc", name="ssc")
                nc.vector.tensor_copy(out=ssc[:], in_=dm[:, :, 0])

                # ---- per 64-atom block: MLP + contraction ----
                for blk in range(8):
                    ss_t = {}
                    for seg, (sel, ngrp) in enumerate(((SEL[0], 2), (SEL[1], 4))):
                        sst = ssbp.tile([4, ngrp * CH], F32, tag=f"ss{seg}",
                                        name=f"ss{seg}")
                        j0 = 0 if seg == 0 else SEL[0]
                        src = mkap(ssc, j0, [[NNEI, 4], [1, sel]],
                                   parts=16, part_off=16 * blk)
                        dst = mkap(sst, 0, [[CH, ngrp], [1, CH]])
                        nc.sync.dma_start(out=dst, in_=src)
                        ss_t[seg] = (sst, ngrp, sel)

                    gg_blk = {}
                    for seg in (0, 1):
                        sst, ngrp, sel = ss_t[seg]
                        gg = ggp.tile([M, 64 * sel], F32, tag=f"gg{seg}",
                                      name=f"gg{seg}")
                        gg_blk[seg] = gg
                        for g in range(ngrp):
                            ps0 = ps_mlp.tile([128, CH], F32, tag="psA", name="ps0",
                                              space="PSUM")
                            nc.tensor.matmul(out=ps0[:], lhsT=WS[("w0", seg)][:],
                                             rhs=sst[:, g * CH:(g + 1) * CH],
                                             start=True, stop=True,
                                             tile_position=(0, 0))
                            y0s = mlpp.tile([128, CH], F32, tag="y0s", name="y0s")
                            nc.scalar.activation(out=y0s[:], in_=ps0[:],
                                                 func=AF.Tanh,
                                                 bias=WS[("b0", seg)][:])
                            th1s = []
                            for half in range(2):
                                ps1 = ps_mlp.tile([128, CH], F32, tag="psA",
                                                  name="ps1", space="PSUM")
                                for ci in range(2):
                                    c = half * 2 + ci
                                    nc.tensor.matmul(
                                        out=ps1[64 * ci:64 * ci + 50, :],
                                        lhsT=WS[("w1", seg)][32 * c:32 * c + 25, :],
                                        rhs=y0s[32 * c:32 * c + 25, :],
                                        start=True, stop=True,
                                        tile_position=(32 * c, 64 * ci))
                                th1 = mlpp.tile([128, CH], F32, tag="y1s",
                                                name="th1")
                                nc.scalar.activation(out=th1[:], in_=ps1[:],
                                                     func=AF.Tanh,
                                                     bias=WS[("b1", seg)][:])
                                th1s.append(th1)
                            for c in range(4):
                                th1 = th1s[c // 2]
                                pb = 64 * (c % 2)
                                ps2 = ps_mlp.tile([128, CH], F32, tag="psA",
                                                  name="ps2", space="PSUM")
                                nc.tensor.matmul(out=ps2[:M, :],
                                                 lhsT=WS[("w2", seg)][pb:pb + 50, :],
                                                 rhs=th1[pb:pb + 50, :],
                                                 start=True, stop=False,
                                                 tile_position=(pb, 0))
                                nc.tensor.matmul(
                                    out=ps2[:M, :],
                                    lhsT=WS[("w2s", seg)][32 * c:32 * c + 25, :],
                                    rhs=y0s[32 * c:32 * c + 25, :],
                                    start=False, stop=True,
                                    tile_position=(32 * c, 0))
                                ps3 = ps_b.tile([128, CH], F32, tag="psB",
                                                name="ps3", space="PSUM")
                                nc.tensor.matmul(out=ps3[:M, :],
                                                 lhsT=cc2_sb[pb:pb + 50, :],
                                                 rhs=th1[pb:pb + 50, :],
                                                 start=True, stop=False,
                                                 tile_position=(pb, 0))
                                nc.tensor.matmul(
                                    out=ps3[:M, :],
                                    lhsT=cc4_sb[32 * c:32 * c + 25, :],
                                    rhs=y0s[32 * c:32 * c + 25, :],
                                    start=False, stop=True,
                                    tile_position=(32 * c, 0))
                                o = (c * ngrp + g) * CH
                                nc.scalar.activation(out=gg[:, o:o + CH],
                                                     in_=ps2[:M, :], func=AF.Tanh,
                                                     bias=WS[("b2", seg)][:])
                                nc.vector.tensor_tensor(out=gg[:, o:o + CH],
                                                        in0=gg[:, o:o + CH],
                                                        in1=ps3[:M, :], op=ALU.add)

                    # contraction
                    t0all = t0p.tile([46, 64, M], F32, tag="t0all", name="t0all")
                    for a0 in range(64):
                        tp = ps_t.tile([128, 128], F32, tag="tpt", name="tp0",
                                       space="PSUM")
                        nc.tensor.transpose(out=tp[:46, :M],
                                            in_=gg_blk[0][:, a0 * 46:(a0 + 1) * 46],
                                            identity=ident[0:M, 0:M])
                        nc.vector.tensor_copy(out=t0all[:, a0, :], in_=tp[0:46, :M])
                    obuf = outp_pool.tile([M, 64, AXIS], BF16, tag="obuf",
                                          name="obuf")
                    for a in range(64):
                        tp = ps_t.tile([128, 128], F32, tag="tpt", name="tp1",
                                       space="PSUM")
                        nc.tensor.transpose(out=tp[:92, :M],
                                            in_=gg_blk[1][:, a * 92:(a + 1) * 92],
                                            identity=ident[0:M, 0:M])
                        t1 = tsbp.tile([92, M], F32, tag="t1", name="t1")
                        nc.vector.tensor_copy(out=t1[:], in_=tp[:92, :M])

                        A = blk * 64 + a
                        xyz_ps = ps_small.tile([4, M], F32, tag="small",
                                               name="xyzp", space="PSUM")
                        lhs0 = mkap(rr0, A, [[S, 4]])
                        nc.tensor.matmul(out=xyz_ps[:], lhsT=lhs0,
                                         rhs=t0all[:, a, :], start=True, stop=False)
                        lhs1 = mkap(rr1, A, [[S, 4]])
                        nc.tensor.matmul(out=xyz_ps[:], lhsT=lhs1, rhs=t1[:],
                                         start=False, stop=True)
                        xyz = tsbp.tile([4, M], F32, tag="xyzs", name="xyzs")
                        nc.scalar.activation(out=xyz[:], in_=xyz_ps[:],
                                             func=AF.Copy, scale=1.0 / NNEI)
                        res_ps = ps_small.tile([M, AXIS], F32, tag="small",
                                               name="resp", space="PSUM")
                        nc.tensor.matmul(out=res_ps[:], lhsT=xyz[:],
                                         rhs=xyz[:, 0:AXIS], start=True, stop=True)
                        nc.vector.tensor_copy(out=obuf[:, a, :], in_=res_ps[:])
                    src = mkap(obuf, 0, [[AXIS, 64], [1, AXIS]])
                    dst = bass.AP(res_d, (f * S + blk * 64) * M * AXIS,
                                  [[AXIS, M], [M * AXIS, 64], [1, AXIS]])
                    nc.sync.dma_start(out=dst, in_=src)

    nc.finalize()
    return nc


# ---------------------------------------------------------------------------
# cached dispatch (shard_map over 8 cores, built once)
# ---------------------------------------------------------------------------
_EXEC = None
_OUT_BUF = None


def _get_exec():
    global _EXEC
    if _EXEC is not None:
        return _EXEC
    import concourse.mybir as mybir
    from concourse.bass2jax import (_bass_exec_p, install_neuronx_cc_hook,
                                    partition_id_tensor)
    from jax.experimental.shard_map import shard_map
    from jax.sharding import Mesh, PartitionSpec

    install_neuronx_cc_hook()
    nc = _build_kernel()

    partition_name = (nc.partition_id_tensor.name
                      if nc.partition_id_tensor else None)
    in_names, out_names, out_avals, zero_shapes = [], [], [], []
    for alloc in nc.m.functions[0].allocations:
        if not isinstance(alloc, mybir.MemoryLocationSet):
            continue
        name = alloc.memorylocations[0].name
        if alloc.kind == "ExternalInput":
            if name != partition_name:
                in_names.append(name)
        elif alloc.kind == "ExternalOutput":
            out_names.append(name)
            shape = tuple(alloc.tensor_shape)
            dtype = mybir.dt.np(alloc.dtype)
            out_avals.append(jax.core.ShapedArray(shape, dtype))
            zero_shapes.append((shape, dtype))
    n_params = len(in_names)
    n_outs = len(out_avals)
    all_in_names = list(in_names) + list(out_names)
    if partition_name is not None:
        all_in_names.append(partition_name)
    donate = tuple(range(n_params, n_params + n_outs))

    def _body(*args):
        operands = list(args)
        if partition_name is not None:
            operands.append(partition_id_tensor())
        outs = _bass_exec_p.bind(
            *operands,
            out_avals=tuple(out_avals),
            in_names=tuple(all_in_names),
            out_names=tuple(out_names),
            lowering_input_output_aliases=(),
            sim_require_finite=True,
            sim_require_nnan=True,
            nc=nc,
        )
        return tuple(outs)

    devices = jax.devices()[:NCORES]
    mesh = Mesh(np.asarray(devices), ("core",))
    in_specs = (PartitionSpec("core"),) * (n_params + n_outs)
    out_specs = (PartitionSpec("core"),) * n_outs
    sharded = jax.jit(
        shard_map(_body, mesh=mesh, in_specs=in_specs, out_specs=out_specs,
                  check_rep=False),
        donate_argnums=donate, keep_unused=True)

    from jax.sharding import NamedSharding
    import jax.numpy as jnp
    shardings = [NamedSharding(mesh, PartitionSpec("core"))] * n_outs

    def _mk_zeros():
        return tuple(jnp.zeros((NCORES * shp[0],) + tuple(shp[1:]), dt)
                     for shp, dt in zero_shapes)
    zero_maker = jax.jit(_mk_zeros, out_shardings=tuple(shardings))
    _EXEC = (sharded, in_names, out_names, zero_maker)
    return _EXEC


# ---------------------------------------------------------------------------
# host-side prep + entry point
# ---------------------------------------------------------------------------
def _prep_global_inputs(nlist, coord, atype, mean, stddev, ws, bs):
    """Build the concatenated (8*dim0, ...) arrays for every DRAM input."""
    g = {}
    nl16 = np.asarray(nlist, dtype=np.int16)
    g["nl"] = np.ascontiguousarray(
        nl16.reshape(NF, NCORES, 128, COLS).transpose(1, 0, 2, 3)
    ).reshape(NCORES * NF, 128, COLS)

    coord = np.asarray(coord, dtype=np.float32)
    coord4 = np.zeros((NF, NALL, 4), np.float32)
    coord4[:, :, 0:3] = coord
    g["coord4_0"] = np.tile(coord4[0], (NCORES, 1))
    g["coord4_1"] = np.tile(coord4[1], (NCORES, 1))

    cat = np.zeros((NF, NLOC, 4), np.float32)
    cat[:, :, 0:3] = coord[:, :NLOC, :]
    cat[:, :, 3] = np.asarray(atype)[:, :NLOC].astype(np.float32)
    g["catype"] = np.ascontiguousarray(
        cat.reshape(NF, NCORES, 128, 16).transpose(1, 0, 2, 3)
    ).reshape(NCORES * NF, 128, 16)

    mean = np.asarray(mean, np.float32)
    stddev = np.asarray(stddev, np.float32)
    istd = 1.0 / stddev
    nmean = -mean / stddev
    nrm = np.stack([
        np.tile(istd[0], (4, 1)),
        np.tile(istd[1] - istd[0], (4, 1)),
        np.tile(nmean[0], (4, 1)),
        np.tile(nmean[1] - nmean[0], (4, 1)),
    ]).astype(ml_dtypes.bfloat16)
    g["nrm"] = np.tile(nrm, (NCORES, 1, 1))

    w0, w1, w2 = [np.asarray(w, np.float32) for w in ws]
    b0, b1, b2 = [np.asarray(b, np.float32) for b in bs]
    w0bd = np.zeros((2, 4, 128), np.float32)
    b0s = np.zeros((2, 128, 1), np.float32)
    w1r = np.zeros((2, 121, 50), np.float32)
    b1s = np.zeros((2, 128, 1), np.float32)
    w2r = np.zeros((2, 114, M), np.float32)
    b2s = np.zeros((2, M, 1), np.float32)
    w2sr = np.zeros((2, 121, M), np.float32)
    for t in range(2):
        w2s = w2[t][0:25] + w2[t][25:50]
        for c in range(4):
            w0bd[t, c, 32 * c:32 * c + 25] = w0[t, 0]
            b0s[t, 32 * c:32 * c + 25, 0] = b0[t]
            w1r[t, 32 * c:32 * c + 25, :] = w1[t]
            w2sr[t, 32 * c:32 * c + 25, :] = w2s
        for h in range(2):
            b1s[t, 64 * h:64 * h + 50, 0] = b1[t]
            w2r[t, 64 * h:64 * h + 50, :] = w2[t]
        b2s[t, :, 0] = b2[t]
    cc2 = np.zeros((50, M), np.float32)
    for i in range(50):
        cc2[i, i] = 1.0
        cc2[i, 50 + i] = 1.0
    cc2r = np.zeros((114, M), np.float32)
    cc2r[0:50] = cc2
    cc2r[64:114] = cc2
    cc4 = np.zeros((25, M), np.float32)
    for i in range(25):
        for k in range(4):
            cc4[i, 25 * k + i] = 1.0
    cc4r = np.zeros((121, M), np.float32)
    for c in range(4):
        cc4r[32 * c:32 * c + 25] = cc4
    for nm, arr in (("w0bd", w0bd), ("b0s", b0s), ("w1r", w1r), ("b1s", b1s),
                    ("w2r", w2r), ("b2s", b2s), ("w2sr", w2sr),
                    ("cc2r", cc2r), ("cc4r", cc4r)):
        g[nm] = np.tile(arr, (NCORES,) + (1,) * (arr.ndim - 1))
    return g


def kernel(nlist, extended_coord, extended_atype, mean, stddev,
           w0, b0, w1, b1, w2, b2):
    sharded, in_names, out_names, zero_maker = _get_exec()
    g = _prep_global_inputs(nlist, extended_coord, extended_atype, mean, stddev,
                            [w0, w1, w2], [b0, b1, b2])
    args = [g[n] for n in in_names]
    zeros = zero_maker()
    out_arrs = sharded(*args, *zeros)
    res = np.asarray(out_arrs[out_names.index("res")])  # [8*NF, S, 1600]
    r = res.reshape(NCORES, NF, S, M * AXIS)
    global _OUT_BUF
    if _OUT_BUF is None:
        _OUT_BUF = np.empty((NF, NLOC, M * AXIS), np.float32)
    full = _OUT_BUF
    for c in range(NCORES):
        for f in range(NF):
            np.copyto(full[f, c * S:(c + 1) * S, :], r[c, f])
    return full


# revision 12
# speedup vs baseline: 2.7865x; 1.0114x over previous
"""DescrptSeA descriptor on 8 Trainium2 NeuronCores via a hand-written
Bass/Tile kernel (SPMD over the nloc axis, 512 atoms per core).

Per-core device kernel (see _build_kernel): neighbor-coordinate gather via
indirect DMA, smoothed env matrix, 3-layer embedding net (resnet skips folded
into PE matmuls via PSUM accumulation), per-atom contraction to the
[M*AXIS]=1600 descriptor. Host only casts/reshapes inputs and reassembles the
output; the compiled executable is cached across calls.
"""
import numpy as np
import jax
import ml_dtypes
from concurrent.futures import ThreadPoolExecutor

NF = 2
NLOC, NALL = 4096, 8192
S = 512
NNEI = 138
SEL = [46, 92]
M = 100
AXIS = 16
PROT = 1e-6
RMIN, RMAX = 0.5, 6.0
COLS = 552
CH = 368
NCORES = 8


# ---------------------------------------------------------------------------
# device kernel builder
# ---------------------------------------------------------------------------
def _build_kernel():
    import concourse.bass as bass
    import concourse.bacc as bacc
    import concourse.tile as tile
    from concourse import mybir
    from concourse.masks import make_identity

    F32 = mybir.dt.float32
    I32 = mybir.dt.int32
    BF16 = mybir.dt.bfloat16
    AF = mybir.ActivationFunctionType
    ALU = mybir.AluOpType
    AX = mybir.AxisListType

    def ap_of(t):
        return t[:] if not isinstance(t, bass.AP) else t

    def mkap(t, offset_elems, free_dims, parts=None, part_off=0):
        a = ap_of(t)
        pitch = a.ap[0][0]
        p = [pitch, parts if parts is not None else a.ap[0][1]]
        return bass.AP(a.tensor, a.offset + part_off * pitch + offset_elems,
                       [p] + [list(d) for d in free_dims])

    nc = bacc.Bacc(None, target_bir_lowering=False, debug=False)

    I16 = mybir.dt.int16
    nl_d = nc.dram_tensor("nl", [NF, 128, COLS], I16, kind="ExternalInput")
    coord_ds = [nc.dram_tensor(f"coord4_{f}", [NALL, 4], F32, kind="ExternalInput")
                for f in range(NF)]
    ctr_d = nc.dram_tensor("catype", [NF, 128, 16], F32, kind="ExternalInput")
    nrm_d = nc.dram_tensor("nrm", [4, COLS, 4], BF16, kind="ExternalInput")
    w0_d = nc.dram_tensor("w0bd", [2, 4, 128], F32, kind="ExternalInput")
    b0_d = nc.dram_tensor("b0s", [2, 128, 1], F32, kind="ExternalInput")
    w1_d = nc.dram_tensor("w1r", [2, 121, 50], F32, kind="ExternalInput")
    b1_d = nc.dram_tensor("b1s", [2, 128, 1], F32, kind="ExternalInput")
    w2_d = nc.dram_tensor("w2r", [2, 114, M], F32, kind="ExternalInput")
    b2_d = nc.dram_tensor("b2s", [2, M, 1], F32, kind="ExternalInput")
    w2sc_d = nc.dram_tensor("w2sc", [2, 25, M], F32, kind="ExternalInput")
    res_d = nc.dram_tensor("res", [NF, S, M * AXIS], BF16, kind="ExternalOutput")

    with tile.TileContext(nc) as tc:
        with (
            tc.tile_pool(name="const", bufs=1) as constp,
            tc.tile_pool(name="frame", bufs=1) as framep,
            tc.tile_pool(name="gather", bufs=2) as gatherp,
            tc.tile_pool(name="envt", bufs=1) as envp,
            tc.tile_pool(name="ssb", bufs=2) as ssbp,
            tc.tile_pool(name="mlp", bufs=2) as mlpp,
            tc.tile_pool(name="ggp", bufs=1) as ggp,
            tc.tile_pool(name="tsb", bufs=4) as tsbp,
            tc.tile_pool(name="t0p", bufs=1) as t0p,
            tc.tile_pool(name="outp", bufs=1) as outp_pool,
            tc.tile_pool(name="ps_mlp", bufs=3, space="PSUM") as ps_mlp,
            tc.tile_pool(name="ps_b", bufs=1, space="PSUM") as ps_b,
            tc.tile_pool(name="ps_t", bufs=2, space="PSUM") as ps_t,
            tc.tile_pool(name="ps_small", bufs=2, space="PSUM") as ps_small,
        ):
            ident = constp.tile([128, 128], F32)
            make_identity(nc, ident)
            nrm_sb = []
            for i in range(4):
                t = constp.tile([128, COLS, 4], BF16, tag=f"nrm{i}", name=f"nrm{i}")
                src = bass.AP(nrm_d, i * COLS * 4, [[0, 128], [4, COLS], [1, 4]])
                nc.sync.dma_start(out=t[:], in_=src)
                nrm_sb.append(t)
            nsc0_sb, nscd_sb, nsh0_sb, nshd_sb = nrm_sb
            WS = {}
            for t in range(2):
                for nm, d, shp in (("w0", w0_d, [4, 128]), ("b0", b0_d, [128, 1]),
                                   ("w1", w1_d, [121, 50]), ("b1", b1_d, [128, 1]),
                                   ("w2", w2_d, [114, M]), ("b2", b2_d, [M, 1])):
                    tl = constp.tile(shp, F32, tag=f"{nm}_{t}", name=f"{nm}_{t}")
                    nc.sync.dma_start(out=tl[:], in_=d[t])
                    WS[(nm, t)] = tl
            # cc2r = [I50;I50] at row-bases {0,64}; cc4r = I25 x4 at 32c
            cc2_sb = constp.tile([114, M], F32, tag="cc2", name="cc2")
            nc.vector.memset(cc2_sb[:], 0.0)
            for rb in (0, 64):
                for cb in (0, 50):
                    nc.vector.tensor_copy(out=cc2_sb[rb:rb + 50, cb:cb + 50],
                                          in_=ident[0:50, 0:50])
            cc4_sb = constp.tile([121, M], F32, tag="cc4", name="cc4")
            nc.vector.memset(cc4_sb[:], 0.0)
            for c in range(4):
                for k in range(4):
                    nc.vector.tensor_copy(
                        out=cc4_sb[32 * c:32 * c + 25, 25 * k:25 * k + 25],
                        in_=ident[0:25, 0:25])
            # w2s (host-compacted w2[0:25]+w2[25:50]) replicated at 32c
            for t in range(2):
                w2st = constp.tile([121, M], F32, tag=f"w2s_{t}", name=f"w2s_{t}")
                for c in range(4):
                    nc.sync.dma_start(out=w2st[32 * c:32 * c + 25, :],
                                      in_=w2sc_d[t])
                WS[("w2s", t)] = w2st

            for f in range(NF):
                # ---- Phase E: env matrix (chunked layout) ----
                it = gatherp.tile([128, COLS], I16, tag="it", name="it")
                nc.sync.dma_start(out=it[:], in_=nl_d[f])
                mask = envp.tile([128, COLS], F32, tag="mask", name="mask")
                nc.vector.tensor_scalar(out=mask[:], in0=it[:], scalar1=0,
                                        scalar2=None, op0=ALU.is_ge)
                itc = gatherp.tile([128, COLS], I32, tag="itc", name="itc")
                nc.vector.tensor_scalar(out=itc[:], in0=it[:], scalar1=0,
                                        scalar2=None, op0=ALU.max)
                gt = gatherp.tile([128, COLS, 4], F32, tag="gt", name="gt")
                for k in range(COLS):
                    nc.gpsimd.indirect_dma_start(
                        out=gt[:, k, :], out_offset=None, in_=coord_ds[f][:],
                        in_offset=bass.IndirectOffsetOnAxis(ap=itc[:, k:k + 1],
                                                            axis=0),
                    )
                ctr = framep.tile([128, 4, 4], F32, tag="ctr", name="ctr")
                nc.sync.dma_start(out=ctr[:],
                                  in_=ctr_d[f].rearrange("p (q c) -> p q c", q=4))

                diff = envp.tile([128, COLS, 3], F32, tag="diff", name="diff")
                ctr_b = mkap(ctr, 0, [[4, 4], [0, NNEI], [1, 3]])
                nc.vector.tensor_tensor(out=diff[:], in0=gt[:, :, 0:3], in1=ctr_b,
                                        op=ALU.subtract)
                sq = envp.tile([128, COLS, 3], F32, tag="sq", name="sq")
                nc.vector.tensor_tensor(out=sq[:], in0=diff[:], in1=diff[:],
                                        op=ALU.mult)
                r2 = envp.tile([128, COLS, 1], F32, tag="r2", name="r2")
                nc.vector.tensor_reduce(out=r2[:], in_=sq[:], axis=AX.X, op=ALU.add)
                r = envp.tile([128, COLS], F32, tag="r", name="r")
                nc.scalar.activation(out=r[:], in_=r2[:, :, 0], func=AF.Sqrt)
                sr = envp.tile([128, COLS], F32, tag="sr", name="sr")
                nc.vector.tensor_scalar(out=sr[:], in0=r[:], scalar1=PROT,
                                        scalar2=None, op0=ALU.add)
                nc.vector.reciprocal(out=sr[:], in_=sr[:])
                sr2 = envp.tile([128, COLS], F32, tag="sr2", name="sr2")
                nc.vector.tensor_tensor(out=sr2[:], in0=sr[:], in1=sr[:], op=ALU.mult)
                uu = envp.tile([128, COLS], F32, tag="uu", name="uu")
                nc.vector.tensor_scalar(out=uu[:], in0=r[:], scalar1=-RMIN,
                                        scalar2=1.0 / (RMAX - RMIN),
                                        op0=ALU.add, op1=ALU.mult)
                nc.vector.tensor_scalar(out=uu[:], in0=uu[:], scalar1=0.0,
                                        scalar2=1.0, op0=ALU.max, op1=ALU.min)
                u2 = envp.tile([128, COLS], F32, tag="u2", name="u2")
                nc.vector.tensor_tensor(out=u2[:], in0=uu[:], in1=uu[:], op=ALU.mult)
                nc.vector.tensor_tensor(out=u2[:], in0=u2[:], in1=uu[:], op=ALU.mult)
                p1 = envp.tile([128, COLS], F32, tag="p1", name="p1")
                nc.vector.tensor_scalar(out=p1[:], in0=uu[:], scalar1=-6.0,
                                        scalar2=15.0, op0=ALU.mult, op1=ALU.add)
                nc.vector.tensor_tensor(out=p1[:], in0=p1[:], in1=uu[:], op=ALU.mult)
                nc.vector.tensor_scalar(out=p1[:], in0=p1[:], scalar1=-10.0,
                                        scalar2=None, op0=ALU.add)
                nc.vector.tensor_tensor(out=p1[:], in0=p1[:], in1=u2[:], op=ALU.mult)
                nc.vector.tensor_scalar(out=p1[:], in0=p1[:], scalar1=1.0,
                                        scalar2=None, op0=ALU.add)
                wm = envp.tile([128, COLS], F32, tag="wm", name="wm")
                nc.vector.tensor_tensor(out=wm[:], in0=p1[:], in1=mask[:],
                                        op=ALU.mult)

                envw = framep.tile([128, COLS, 4], F32, tag="envw", name="envw")
                nc.vector.tensor_copy(out=envw[:, :, 0], in_=sr[:])
                sr2_b = mkap(sr2, 0, [[1, COLS], [0, 3]])
                nc.vector.tensor_tensor(out=envw[:, :, 1:4], in0=diff[:], in1=sr2_b,
                                        op=ALU.mult)
                tpt = envp.tile([128, COLS], F32, tag="r2", name="tpt")
                tpt_src = mkap(ctr, 3, [[4, 4], [0, NNEI]])
                nc.vector.tensor_copy(out=tpt[:], in_=tpt_src)
                tpt_b = mkap(tpt, 0, [[1, COLS], [0, 4]])
                wm_b = mkap(wm, 0, [[1, COLS], [0, 4]])
                x1 = envp.tile([128, COLS, 4], F32, tag="sq", name="x1")
                nc.vector.tensor_tensor(out=x1[:], in0=nscd_sb[:], in1=tpt_b,
                                        op=ALU.mult)
                nc.vector.tensor_tensor(out=x1[:], in0=x1[:], in1=nsc0_sb[:],
                                        op=ALU.add)
                nc.vector.tensor_tensor(out=x1[:], in0=x1[:], in1=wm_b, op=ALU.mult)
                dm = framep.tile([128, COLS, 4], F32, tag="dm", name="dm")
                nc.vector.tensor_tensor(out=dm[:], in0=envw[:], in1=x1[:],
                                        op=ALU.mult)
                y1 = envp.tile([128, COLS, 4], F32, tag="diff", name="y1")
                nc.vector.tensor_tensor(out=y1[:], in0=nshd_sb[:], in1=tpt_b,
                                        op=ALU.mult)
                nc.vector.tensor_tensor(out=y1[:], in0=y1[:], in1=nsh0_sb[:],
                                        op=ALU.add)
                nc.vector.tensor_tensor(out=dm[:], in0=dm[:], in1=y1[:], op=ALU.add)

                # ---- Phase T: rr to slot-major [sel, 4, S] ----
                rr0 = framep.tile([SEL[0], 4, S], F32, tag="rr0", name="rr0")
                rr1 = framep.tile([SEL[1], 4, S], F32, tag="rr1", name="rr1")
                for q in range(4):
                    for ch in range(4):
                        for rr_sb, j0, sel in ((rr0, 0, SEL[0]),
                                               (rr1, SEL[0], SEL[1])):
                            src = mkap(dm, (q * NNEI + j0) * 4 + ch, [[4, sel]])
                            tp = ps_t.tile([128, 128], F32, tag="tpt", name="tpq",
                                           space="PSUM")
                            nc.tensor.transpose(out=tp[:sel, :], in_=src,
                                                identity=ident[:])
                            dst = mkap(rr_sb, ch * S + q, [[4, 128]])
                            nc.vector.tensor_copy(out=dst, in_=tp[:sel, :])

                ssc = framep.tile([128, COLS], F32, tag="ssc", name="ssc")
                nc.vector.tensor_copy(out=ssc[:], in_=dm[:, :, 0])

                # ---- per 64-atom block: MLP + contraction ----
                for blk in range(8):
                    ss_t = {}
                    for seg, (sel, ngrp) in enumerate(((SEL[0], 2), (SEL[1], 4))):
                        sst = ssbp.tile([4, ngrp * CH], F32, tag=f"ss{seg}",
                                        name=f"ss{seg}")
                        j0 = 0 if seg == 0 else SEL[0]
                        src = mkap(ssc, j0, [[NNEI, 4], [1, sel]],
                                   parts=16, part_off=16 * blk)
                        dst = mkap(sst, 0, [[CH, ngrp], [1, CH]])
                        nc.sync.dma_start(out=dst, in_=src)
                        ss_t[seg] = (sst, ngrp, sel)

                    gg_blk = {}
                    for seg in (0, 1):
                        sst, ngrp, sel = ss_t[seg]
                        gg = ggp.tile([M, 64 * sel], F32, tag=f"gg{seg}",
                                      name=f"gg{seg}")
                        gg_blk[seg] = gg
                        for g in range(ngrp):
                            ps0 = ps_mlp.tile([128, CH], F32, tag="psA", name="ps0",
                                              space="PSUM")
                            nc.tensor.matmul(out=ps0[:], lhsT=WS[("w0", seg)][:],
                                             rhs=sst[:, g * CH:(g + 1) * CH],
                                             start=True, stop=True,
                                             tile_position=(0, 0))
                            y0s = mlpp.tile([128, CH], F32, tag="y0s", name="y0s")
                            nc.scalar.activation(out=y0s[:], in_=ps0[:],
                                                 func=AF.Tanh,
                                                 bias=WS[("b0", seg)][:])
                            th1s = []
                            for half in range(2):
                                ps1 = ps_mlp.tile([128, CH], F32, tag="psA",
                                                  name="ps1", space="PSUM")
                                for ci in range(2):
                                    c = half * 2 + ci
                                    nc.tensor.matmul(
                                        out=ps1[64 * ci:64 * ci + 50, :],
                                        lhsT=WS[("w1", seg)][32 * c:32 * c + 25, :],
                                        rhs=y0s[32 * c:32 * c + 25, :],
                                        start=True, stop=True,
                                        tile_position=(32 * c, 64 * ci))
                                th1 = mlpp.tile([128, CH], F32, tag="y1s",
                                                name="th1")
                                nc.scalar.activation(out=th1[:], in_=ps1[:],
                                                     func=AF.Tanh,
                                                     bias=WS[("b1", seg)][:])
                                th1s.append(th1)
                            for c in range(4):
                                th1 = th1s[c // 2]
                                pb = 64 * (c % 2)
                                ps2 = ps_mlp.tile([128, CH], F32, tag="psA",
                                                  name="ps2", space="PSUM")
                                nc.tensor.matmul(out=ps2[:M, :],
                                                 lhsT=WS[("w2", seg)][pb:pb + 50, :],
                                                 rhs=th1[pb:pb + 50, :],
                                                 start=True, stop=False,
                                                 tile_position=(pb, 0))
                                nc.tensor.matmul(
                                    out=ps2[:M, :],
                                    lhsT=WS[("w2s", seg)][32 * c:32 * c + 25, :],
                                    rhs=y0s[32 * c:32 * c + 25, :],
                                    start=False, stop=True,
                                    tile_position=(32 * c, 0))
                                ps3 = ps_b.tile([128, CH], F32, tag="psB",
                                                name="ps3", space="PSUM")
                                nc.tensor.matmul(out=ps3[:M, :],
                                                 lhsT=cc2_sb[pb:pb + 50, :],
                                                 rhs=th1[pb:pb + 50, :],
                                                 start=True, stop=False,
                                                 tile_position=(pb, 0))
                                nc.tensor.matmul(
                                    out=ps3[:M, :],
                                    lhsT=cc4_sb[32 * c:32 * c + 25, :],
                                    rhs=y0s[32 * c:32 * c + 25, :],
                                    start=False, stop=True,
                                    tile_position=(32 * c, 0))
                                o = (c * ngrp + g) * CH
                                nc.scalar.activation(out=gg[:, o:o + CH],
                                                     in_=ps2[:M, :], func=AF.Tanh,
                                                     bias=WS[("b2", seg)][:])
                                nc.vector.tensor_tensor(out=gg[:, o:o + CH],
                                                        in0=gg[:, o:o + CH],
                                                        in1=ps3[:M, :], op=ALU.add)

                    # contraction
                    t0all = t0p.tile([46, 64, M], F32, tag="t0all", name="t0all")
                    for a0 in range(64):
                        tp = ps_t.tile([128, 128], F32, tag="tpt", name="tp0",
                                       space="PSUM")
                        nc.tensor.transpose(out=tp[:46, :M],
                                            in_=gg_blk[0][:, a0 * 46:(a0 + 1) * 46],
                                            identity=ident[0:M, 0:M])
                        nc.vector.tensor_copy(out=t0all[:, a0, :], in_=tp[0:46, :M])
                    obuf = outp_pool.tile([M, 64, AXIS], BF16, tag="obuf",
                                          name="obuf")
                    for a in range(64):
                        tp = ps_t.tile([128, 128], F32, tag="tpt", name="tp1",
                                       space="PSUM")
                        nc.tensor.transpose(out=tp[:92, :M],
                                            in_=gg_blk[1][:, a * 92:(a + 1) * 92],
                                            identity=ident[0:M, 0:M])
                        t1 = tsbp.tile([92, M], F32, tag="t1", name="t1")
                        nc.vector.tensor_copy(out=t1[:], in_=tp[:92, :M])

                        A = blk * 64 + a
                        xyz_ps = ps_small.tile([4, M], F32, tag="small",
                                               name="xyzp", space="PSUM")
                        lhs0 = mkap(rr0, A, [[S, 4]])
                        nc.tensor.matmul(out=xyz_ps[:], lhsT=lhs0,
                                         rhs=t0all[:, a, :], start=True, stop=False)
                        lhs1 = mkap(rr1, A, [[S, 4]])
                        nc.tensor.matmul(out=xyz_ps[:], lhsT=lhs1, rhs=t1[:],
                                         start=False, stop=True)
                        xyz = tsbp.tile([4, M], F32, tag="xyzs", name="xyzs")
                        nc.scalar.activation(out=xyz[:], in_=xyz_ps[:],
                                             func=AF.Copy, scale=1.0 / NNEI)
                        res_ps = ps_small.tile([M, AXIS], F32, tag="small",
                                               name="resp", space="PSUM")
                        nc.tensor.matmul(out=res_ps[:], lhsT=xyz[:],
                                         rhs=xyz[:, 0:AXIS], start=True, stop=True)
                        nc.vector.tensor_copy(out=obuf[:, a, :], in_=res_ps[:])
                    src = mkap(obuf, 0, [[AXIS, 64], [1, AXIS]])
                    dst = bass.AP(res_d, (f * S + blk * 64) * M * AXIS,
                                  [[AXIS, M], [M * AXIS, 64], [1, AXIS]])
                    nc.sync.dma_start(out=dst, in_=src)

    nc.finalize()
    return nc


# ---------------------------------------------------------------------------
# cached dispatch (shard_map over 8 cores, built once)
# ---------------------------------------------------------------------------
_EXEC = None
_OUT_BUF = None


def _get_exec():
    global _EXEC
    if _EXEC is not None:
        return _EXEC
    import concourse.mybir as mybir
    from concourse.bass2jax import (_bass_exec_p, install_neuronx_cc_hook,
                                    partition_id_tensor)
    from jax.experimental.shard_map import shard_map
    from jax.sharding import Mesh, PartitionSpec

    install_neuronx_cc_hook()
    nc = _build_kernel()

    partition_name = (nc.partition_id_tensor.name
                      if nc.partition_id_tensor else None)
    in_names, out_names, out_avals, zero_shapes = [], [], [], []
    for alloc in nc.m.functions[0].allocations:
        if not isinstance(alloc, mybir.MemoryLocationSet):
            continue
        name = alloc.memorylocations[0].name
        if alloc.kind == "ExternalInput":
            if name != partition_name:
                in_names.append(name)
        elif alloc.kind == "ExternalOutput":
            out_names.append(name)
            shape = tuple(alloc.tensor_shape)
            dtype = mybir.dt.np(alloc.dtype)
            out_avals.append(jax.core.ShapedArray(shape, dtype))
            zero_shapes.append((shape, dtype))
    n_params = len(in_names)
    n_outs = len(out_avals)
    all_in_names = list(in_names) + list(out_names)
    if partition_name is not None:
        all_in_names.append(partition_name)
    donate = tuple(range(n_params, n_params + n_outs))

    def _body(*args):
        operands = list(args)
        if partition_name is not None:
            operands.append(partition_id_tensor())
        outs = _bass_exec_p.bind(
            *operands,
            out_avals=tuple(out_avals),
            in_names=tuple(all_in_names),
            out_names=tuple(out_names),
            lowering_input_output_aliases=(),
            sim_require_finite=True,
            sim_require_nnan=True,
            nc=nc,
        )
        return tuple(outs)

    devices = jax.devices()[:NCORES]
    mesh = Mesh(np.asarray(devices), ("core",))
    in_specs = (PartitionSpec("core"),) * (n_params + n_outs)
    out_specs = (PartitionSpec("core"),) * n_outs
    sharded = jax.jit(
        shard_map(_body, mesh=mesh, in_specs=in_specs, out_specs=out_specs,
                  check_rep=False),
        donate_argnums=donate, keep_unused=True)

    from jax.sharding import NamedSharding
    import jax.numpy as jnp
    shardings = [NamedSharding(mesh, PartitionSpec("core"))] * n_outs

    def _mk_zeros():
        return tuple(jnp.zeros((NCORES * shp[0],) + tuple(shp[1:]), dt)
                     for shp, dt in zero_shapes)
    zero_maker = jax.jit(_mk_zeros, out_shardings=tuple(shardings))
    _EXEC = (sharded, in_names, out_names, zero_maker)
    return _EXEC


# ---------------------------------------------------------------------------
# host-side prep + entry point
# ---------------------------------------------------------------------------
def _prep_global_inputs(nlist, coord, atype, mean, stddev, ws, bs):
    """Build the concatenated (8*dim0, ...) arrays for every DRAM input."""
    g = {}
    nl16 = np.asarray(nlist, dtype=np.int16)
    g["nl"] = np.ascontiguousarray(
        nl16.reshape(NF, NCORES, 128, COLS).transpose(1, 0, 2, 3)
    ).reshape(NCORES * NF, 128, COLS)

    coord = np.asarray(coord, dtype=np.float32)
    coord4 = np.zeros((NF, NALL, 4), np.float32)
    coord4[:, :, 0:3] = coord
    g["coord4_0"] = np.tile(coord4[0], (NCORES, 1))
    g["coord4_1"] = np.tile(coord4[1], (NCORES, 1))

    cat = np.zeros((NF, NLOC, 4), np.float32)
    cat[:, :, 0:3] = coord[:, :NLOC, :]
    cat[:, :, 3] = np.asarray(atype)[:, :NLOC].astype(np.float32)
    g["catype"] = np.ascontiguousarray(
        cat.reshape(NF, NCORES, 128, 16).transpose(1, 0, 2, 3)
    ).reshape(NCORES * NF, 128, 16)

    mean = np.asarray(mean, np.float32)
    stddev = np.asarray(stddev, np.float32)
    istd = 1.0 / stddev
    nmean = -mean / stddev
    nrm = np.stack([
        np.tile(istd[0], (4, 1)),
        np.tile(istd[1] - istd[0], (4, 1)),
        np.tile(nmean[0], (4, 1)),
        np.tile(nmean[1] - nmean[0], (4, 1)),
    ]).astype(ml_dtypes.bfloat16)
    g["nrm"] = np.tile(nrm, (NCORES, 1, 1))

    w0, w1, w2 = [np.asarray(w, np.float32) for w in ws]
    b0, b1, b2 = [np.asarray(b, np.float32) for b in bs]
    w0bd = np.zeros((2, 4, 128), np.float32)
    b0s = np.zeros((2, 128, 1), np.float32)
    w1r = np.zeros((2, 121, 50), np.float32)
    b1s = np.zeros((2, 128, 1), np.float32)
    w2r = np.zeros((2, 114, M), np.float32)
    b2s = np.zeros((2, M, 1), np.float32)
    for t in range(2):
        for c in range(4):
            w0bd[t, c, 32 * c:32 * c + 25] = w0[t, 0]
            b0s[t, 32 * c:32 * c + 25, 0] = b0[t]
            w1r[t, 32 * c:32 * c + 25, :] = w1[t]
        for h in range(2):
            b1s[t, 64 * h:64 * h + 50, 0] = b1[t]
            w2r[t, 64 * h:64 * h + 50, :] = w2[t]
        b2s[t, :, 0] = b2[t]
    w2sc = (w2[:, 0:25, :] + w2[:, 25:50, :]).astype(np.float32)
    for nm, arr in (("w0bd", w0bd), ("b0s", b0s), ("w1r", w1r), ("b1s", b1s),
                    ("w2r", w2r), ("b2s", b2s), ("w2sc", w2sc)):
        g[nm] = np.tile(arr, (NCORES,) + (1,) * (arr.ndim - 1))
    return g


def kernel(nlist, extended_coord, extended_atype, mean, stddev,
           w0, b0, w1, b1, w2, b2):
    sharded, in_names, out_names, zero_maker = _get_exec()
    g = _prep_global_inputs(nlist, extended_coord, extended_atype, mean, stddev,
                            [w0, w1, w2], [b0, b1, b2])
    args = [g[n] for n in in_names]
    zeros = zero_maker()
    out_arrs = sharded(*args, *zeros)
    res = np.asarray(out_arrs[out_names.index("res")])  # [8*NF, S, 1600]
    r = res.reshape(NCORES, NF, S, M * AXIS)
    global _OUT_BUF
    if _OUT_BUF is None:
        _OUT_BUF = np.empty((NF, NLOC, M * AXIS), np.float32)
    full = _OUT_BUF
    for c in range(NCORES):
        for f in range(NF):
            np.copyto(full[f, c * S:(c + 1) * S, :], r[c, f])
    return full


# revision 14
# speedup vs baseline: 3.0069x; 1.0791x over previous
"""DescrptSeA descriptor on 8 Trainium2 NeuronCores via a hand-written
Bass/Tile kernel (SPMD over the nloc axis, 512 atoms per core).

Per-core device kernel (see _build_kernel): neighbor-coordinate gather via
indirect DMA, smoothed env matrix, 3-layer embedding net (resnet skips folded
into PE matmuls via PSUM accumulation), per-atom contraction to the
[M*AXIS]=1600 descriptor. Host only casts/reshapes inputs and reassembles the
output; the compiled executable is cached across calls.
"""
import numpy as np
import jax
import ml_dtypes
from concurrent.futures import ThreadPoolExecutor

NF = 2
NLOC, NALL = 4096, 8192
S = 512
NNEI = 138
SEL = [46, 92]
M = 100
AXIS = 16
PROT = 1e-6
RMIN, RMAX = 0.5, 6.0
COLS = 552
CH = 368
NCORES = 8


# ---------------------------------------------------------------------------
# device kernel builder
# ---------------------------------------------------------------------------
def _build_kernel():
    import concourse.bass as bass
    import concourse.bacc as bacc
    import concourse.tile as tile
    from concourse import mybir
    from concourse.masks import make_identity

    F32 = mybir.dt.float32
    I32 = mybir.dt.int32
    BF16 = mybir.dt.bfloat16
    AF = mybir.ActivationFunctionType
    ALU = mybir.AluOpType
    AX = mybir.AxisListType

    def ap_of(t):
        return t[:] if not isinstance(t, bass.AP) else t

    def mkap(t, offset_elems, free_dims, parts=None, part_off=0):
        a = ap_of(t)
        pitch = a.ap[0][0]
        p = [pitch, parts if parts is not None else a.ap[0][1]]
        return bass.AP(a.tensor, a.offset + part_off * pitch + offset_elems,
                       [p] + [list(d) for d in free_dims])

    nc = bacc.Bacc(None, target_bir_lowering=False, debug=False)

    I16 = mybir.dt.int16
    nl_d = nc.dram_tensor("nl", [NF, 128, COLS], I16, kind="ExternalInput")
    coordsh_d = nc.dram_tensor("coordsh", [NF * (NALL // NCORES), 4], F32,
                               kind="ExternalInput")
    ctr_d = nc.dram_tensor("catype", [NF, 128, 16], F32, kind="ExternalInput")
    nrm_d = nc.dram_tensor("nrm", [4, COLS, 4], BF16, kind="ExternalInput")
    w0_d = nc.dram_tensor("w0bd", [2, 4, 128], F32, kind="ExternalInput")
    b0_d = nc.dram_tensor("b0s", [2, 128, 1], F32, kind="ExternalInput")
    w1_d = nc.dram_tensor("w1r", [2, 121, 50], F32, kind="ExternalInput")
    b1_d = nc.dram_tensor("b1s", [2, 128, 1], F32, kind="ExternalInput")
    w2_d = nc.dram_tensor("w2r", [2, 114, M], F32, kind="ExternalInput")
    b2_d = nc.dram_tensor("b2s", [2, M, 1], F32, kind="ExternalInput")
    w2sc_d = nc.dram_tensor("w2sc", [2, 25, M], F32, kind="ExternalInput")
    res_d = nc.dram_tensor("res", [NF, S, M * AXIS], BF16, kind="ExternalOutput")

    with tile.TileContext(nc) as tc:
        with (
            tc.tile_pool(name="const", bufs=1) as constp,
            tc.tile_pool(name="frame", bufs=1) as framep,
            tc.tile_pool(name="gather", bufs=2) as gatherp,
            tc.tile_pool(name="envt", bufs=1) as envp,
            tc.tile_pool(name="ssb", bufs=2) as ssbp,
            tc.tile_pool(name="mlp", bufs=2) as mlpp,
            tc.tile_pool(name="ggp", bufs=1) as ggp,
            tc.tile_pool(name="tsb", bufs=4) as tsbp,
            tc.tile_pool(name="t0p", bufs=1) as t0p,
            tc.tile_pool(name="outp", bufs=1) as outp_pool,
            tc.tile_pool(name="ps_mlp", bufs=3, space="PSUM") as ps_mlp,
            tc.tile_pool(name="ps_b", bufs=1, space="PSUM") as ps_b,
            tc.tile_pool(name="ps_t", bufs=2, space="PSUM") as ps_t,
            tc.tile_pool(name="ps_small", bufs=2, space="PSUM") as ps_small,
            tc.tile_pool(name="dramp", bufs=1, space="DRAM") as dramp,
        ):
            # all-gather the full coordinate table from the per-core shards
            SHR = NALL // NCORES
            agout = []
            for f in range(NF):
                agi = dramp.tile([SHR, 4], F32, tag=f"agi{f}", name=f"agi{f}")
                nc.gpsimd.dma_start(out=agi[:],
                                    in_=coordsh_d[f * SHR:(f + 1) * SHR, :])
                ago = dramp.tile([NALL, 4], F32, tag=f"ago{f}", name=f"ago{f}")
                nc.gpsimd.collective_compute(
                    "AllGather", mybir.AluOpType.bypass,
                    replica_groups=[list(range(NCORES))],
                    ins=[agi.opt()], outs=[ago.opt()])
                agout.append(ago)
            ident = constp.tile([128, 128], F32)
            make_identity(nc, ident)
            nrm_sb = []
            for i in range(4):
                t = constp.tile([128, COLS, 4], BF16, tag=f"nrm{i}", name=f"nrm{i}")
                src = bass.AP(nrm_d, i * COLS * 4, [[0, 128], [4, COLS], [1, 4]])
                nc.sync.dma_start(out=t[:], in_=src)
                nrm_sb.append(t)
            nsc0_sb, nscd_sb, nsh0_sb, nshd_sb = nrm_sb
            WS = {}
            for t in range(2):
                for nm, d, shp in (("w0", w0_d, [4, 128]), ("b0", b0_d, [128, 1]),
                                   ("w1", w1_d, [121, 50]), ("b1", b1_d, [128, 1]),
                                   ("w2", w2_d, [114, M]), ("b2", b2_d, [M, 1])):
                    tl = constp.tile(shp, F32, tag=f"{nm}_{t}", name=f"{nm}_{t}")
                    nc.sync.dma_start(out=tl[:], in_=d[t])
                    WS[(nm, t)] = tl
            # cc2r = [I50;I50] at row-bases {0,64}; cc4r = I25 x4 at 32c
            cc2_sb = constp.tile([114, M], F32, tag="cc2", name="cc2")
            nc.vector.memset(cc2_sb[:], 0.0)
            for rb in (0, 64):
                for cb in (0, 50):
                    nc.vector.tensor_copy(out=cc2_sb[rb:rb + 50, cb:cb + 50],
                                          in_=ident[0:50, 0:50])
            cc4_sb = constp.tile([121, M], F32, tag="cc4", name="cc4")
            nc.vector.memset(cc4_sb[:], 0.0)
            for c in range(4):
                for k in range(4):
                    nc.vector.tensor_copy(
                        out=cc4_sb[32 * c:32 * c + 25, 25 * k:25 * k + 25],
                        in_=ident[0:25, 0:25])
            # w2s (host-compacted w2[0:25]+w2[25:50]) replicated at 32c
            for t in range(2):
                w2st = constp.tile([121, M], F32, tag=f"w2s_{t}", name=f"w2s_{t}")
                for c in range(4):
                    nc.sync.dma_start(out=w2st[32 * c:32 * c + 25, :],
                                      in_=w2sc_d[t])
                WS[("w2s", t)] = w2st

            def process_frame(f):
                # ---- Phase E: env matrix (chunked layout) ----
                it = gatherp.tile([128, COLS], I16, tag="it", name="it")
                nc.sync.dma_start(out=it[:], in_=nl_d[f])
                mask = envp.tile([128, COLS], F32, tag="mask", name="mask")
                nc.vector.tensor_scalar(out=mask[:], in0=it[:], scalar1=0,
                                        scalar2=None, op0=ALU.is_ge)
                itc = gatherp.tile([128, COLS], I32, tag="itc", name="itc")
                nc.vector.tensor_scalar(out=itc[:], in0=it[:], scalar1=0,
                                        scalar2=None, op0=ALU.max)
                gt = gatherp.tile([128, COLS, 4], F32, tag="gt", name="gt")
                for k in range(COLS):
                    nc.gpsimd.indirect_dma_start(
                        out=gt[:, k, :], out_offset=None, in_=agout[f][:],
                        in_offset=bass.IndirectOffsetOnAxis(ap=itc[:, k:k + 1],
                                                            axis=0),
                    )
                ctr = framep.tile([128, 4, 4], F32, tag="ctr", name="ctr")
                nc.sync.dma_start(out=ctr[:],
                                  in_=ctr_d[f].rearrange("p (q c) -> p q c", q=4))

                diff = envp.tile([128, COLS, 3], F32, tag="diff", name="diff")
                ctr_b = mkap(ctr, 0, [[4, 4], [0, NNEI], [1, 3]])
                nc.vector.tensor_tensor(out=diff[:], in0=gt[:, :, 0:3], in1=ctr_b,
                                        op=ALU.subtract)
                sq = envp.tile([128, COLS, 3], F32, tag="sq", name="sq")
                nc.vector.tensor_tensor(out=sq[:], in0=diff[:], in1=diff[:],
                                        op=ALU.mult)
                r2 = envp.tile([128, COLS, 1], F32, tag="r2", name="r2")
                nc.vector.tensor_reduce(out=r2[:], in_=sq[:], axis=AX.X, op=ALU.add)
                r = envp.tile([128, COLS], F32, tag="r", name="r")
                nc.scalar.activation(out=r[:], in_=r2[:, :, 0], func=AF.Sqrt)
                sr = envp.tile([128, COLS], F32, tag="sr", name="sr")
                nc.vector.tensor_scalar(out=sr[:], in0=r[:], scalar1=PROT,
                                        scalar2=None, op0=ALU.add)
                nc.vector.reciprocal(out=sr[:], in_=sr[:])
                sr2 = envp.tile([128, COLS], F32, tag="sr2", name="sr2")
                nc.vector.tensor_tensor(out=sr2[:], in0=sr[:], in1=sr[:], op=ALU.mult)
                uu = envp.tile([128, COLS], F32, tag="uu", name="uu")
                nc.vector.tensor_scalar(out=uu[:], in0=r[:], scalar1=-RMIN,
                                        scalar2=1.0 / (RMAX - RMIN),
                                        op0=ALU.add, op1=ALU.mult)
                nc.vector.tensor_scalar(out=uu[:], in0=uu[:], scalar1=0.0,
                                        scalar2=1.0, op0=ALU.max, op1=ALU.min)
                u2 = envp.tile([128, COLS], F32, tag="u2", name="u2")
                nc.vector.tensor_tensor(out=u2[:], in0=uu[:], in1=uu[:], op=ALU.mult)
                nc.vector.tensor_tensor(out=u2[:], in0=u2[:], in1=uu[:], op=ALU.mult)
                p1 = envp.tile([128, COLS], F32, tag="p1", name="p1")
                nc.vector.tensor_scalar(out=p1[:], in0=uu[:], scalar1=-6.0,
                                        scalar2=15.0, op0=ALU.mult, op1=ALU.add)
                nc.vector.tensor_tensor(out=p1[:], in0=p1[:], in1=uu[:], op=ALU.mult)
                nc.vector.tensor_scalar(out=p1[:], in0=p1[:], scalar1=-10.0,
                                        scalar2=None, op0=ALU.add)
                nc.vector.tensor_tensor(out=p1[:], in0=p1[:], in1=u2[:], op=ALU.mult)
                nc.vector.tensor_scalar(out=p1[:], in0=p1[:], scalar1=1.0,
                                        scalar2=None, op0=ALU.add)
                wm = envp.tile([128, COLS], F32, tag="wm", name="wm")
                nc.vector.tensor_tensor(out=wm[:], in0=p1[:], in1=mask[:],
                                        op=ALU.mult)

                envw = framep.tile([128, COLS, 4], F32, tag="envw", name="envw")
                nc.vector.tensor_copy(out=envw[:, :, 0], in_=sr[:])
                sr2_b = mkap(sr2, 0, [[1, COLS], [0, 3]])
                nc.vector.tensor_tensor(out=envw[:, :, 1:4], in0=diff[:], in1=sr2_b,
                                        op=ALU.mult)
                tpt = envp.tile([128, COLS], F32, tag="r2", name="tpt")
                tpt_src = mkap(ctr, 3, [[4, 4], [0, NNEI]])
                nc.vector.tensor_copy(out=tpt[:], in_=tpt_src)
                tpt_b = mkap(tpt, 0, [[1, COLS], [0, 4]])
                wm_b = mkap(wm, 0, [[1, COLS], [0, 4]])
                x1 = envp.tile([128, COLS, 4], F32, tag="sq", name="x1")
                nc.vector.tensor_tensor(out=x1[:], in0=nscd_sb[:], in1=tpt_b,
                                        op=ALU.mult)
                nc.vector.tensor_tensor(out=x1[:], in0=x1[:], in1=nsc0_sb[:],
                                        op=ALU.add)
                nc.vector.tensor_tensor(out=x1[:], in0=x1[:], in1=wm_b, op=ALU.mult)
                dm = framep.tile([128, COLS, 4], F32, tag="dm", name="dm")
                nc.vector.tensor_tensor(out=dm[:], in0=envw[:], in1=x1[:],
                                        op=ALU.mult)
                y1 = envp.tile([128, COLS, 4], F32, tag="diff", name="y1")
                nc.vector.tensor_tensor(out=y1[:], in0=nshd_sb[:], in1=tpt_b,
                                        op=ALU.mult)
                nc.vector.tensor_tensor(out=y1[:], in0=y1[:], in1=nsh0_sb[:],
                                        op=ALU.add)
                nc.vector.tensor_tensor(out=dm[:], in0=dm[:], in1=y1[:], op=ALU.add)

                # ---- Phase T: rr to slot-major [sel, 4, S] ----
                rr0 = framep.tile([SEL[0], 4, S], F32, tag="rr0", name="rr0")
                rr1 = framep.tile([SEL[1], 4, S], F32, tag="rr1", name="rr1")
                for q in range(4):
                    for ch in range(4):
                        for rr_sb, j0, sel in ((rr0, 0, SEL[0]),
                                               (rr1, SEL[0], SEL[1])):
                            src = mkap(dm, (q * NNEI + j0) * 4 + ch, [[4, sel]])
                            tp = ps_t.tile([128, 128], F32, tag="tpt", name="tpq",
                                           space="PSUM")
                            nc.tensor.transpose(out=tp[:sel, :], in_=src,
                                                identity=ident[:])
                            dst = mkap(rr_sb, ch * S + q, [[4, 128]])
                            nc.vector.tensor_copy(out=dst, in_=tp[:sel, :])

                ssc = framep.tile([128, COLS], F32, tag="ssc", name="ssc")
                nc.vector.tensor_copy(out=ssc[:], in_=dm[:, :, 0])

                # ---- per 64-atom block: MLP + contraction ----
                for blk in range(8):
                    ss_t = {}
                    for seg, (sel, ngrp) in enumerate(((SEL[0], 2), (SEL[1], 4))):
                        sst = ssbp.tile([4, ngrp * CH], F32, tag=f"ss{seg}",
                                        name=f"ss{seg}")
                        j0 = 0 if seg == 0 else SEL[0]
                        src = mkap(ssc, j0, [[NNEI, 4], [1, sel]],
                                   parts=16, part_off=16 * blk)
                        dst = mkap(sst, 0, [[CH, ngrp], [1, CH]])
                        nc.sync.dma_start(out=dst, in_=src)
                        ss_t[seg] = (sst, ngrp, sel)

                    gg_blk = {}
                    for seg in (0, 1):
                        sst, ngrp, sel = ss_t[seg]
                        gg = ggp.tile([M, 64 * sel], F32, tag=f"gg{seg}",
                                      name=f"gg{seg}")
                        gg_blk[seg] = gg
                        for g in range(ngrp):
                            ps0 = ps_mlp.tile([128, CH], F32, tag="psA", name="ps0",
                                              space="PSUM")
                            nc.tensor.matmul(out=ps0[:], lhsT=WS[("w0", seg)][:],
                                             rhs=sst[:, g * CH:(g + 1) * CH],
                                             start=True, stop=True,
                                             tile_position=(0, 0))
                            y0s = mlpp.tile([128, CH], F32, tag="y0s", name="y0s")
                            nc.scalar.activation(out=y0s[:], in_=ps0[:],
                                                 func=AF.Tanh,
                                                 bias=WS[("b0", seg)][:])
                            th1s = []
                            for half in range(2):
                                ps1 = ps_mlp.tile([128, CH], F32, tag="psA",
                                                  name="ps1", space="PSUM")
                                for ci in range(2):
                                    c = half * 2 + ci
                                    nc.tensor.matmul(
                                        out=ps1[64 * ci:64 * ci + 50, :],
                                        lhsT=WS[("w1", seg)][32 * c:32 * c + 25, :],
                                        rhs=y0s[32 * c:32 * c + 25, :],
                                        start=True, stop=True,
                                        tile_position=(32 * c, 64 * ci))
                                th1 = mlpp.tile([128, CH], F32, tag="y1s",
                                                name="th1")
                                nc.scalar.activation(out=th1[:], in_=ps1[:],
                                                     func=AF.Tanh,
                                                     bias=WS[("b1", seg)][:])
                                th1s.append(th1)
                            for c in range(4):
                                th1 = th1s[c // 2]
                                pb = 64 * (c % 2)
                                ps2 = ps_mlp.tile([128, CH], F32, tag="psA",
                                                  name="ps2", space="PSUM")
                                nc.tensor.matmul(out=ps2[:M, :],
                                                 lhsT=WS[("w2", seg)][pb:pb + 50, :],
                                                 rhs=th1[pb:pb + 50, :],
                                                 start=True, stop=False,
                                                 tile_position=(pb, 0))
                                nc.tensor.matmul(
                                    out=ps2[:M, :],
                                    lhsT=WS[("w2s", seg)][32 * c:32 * c + 25, :],
                                    rhs=y0s[32 * c:32 * c + 25, :],
                                    start=False, stop=True,
                                    tile_position=(32 * c, 0))
                                ps3 = ps_b.tile([128, CH], F32, tag="psB",
                                                name="ps3", space="PSUM")
                                nc.tensor.matmul(out=ps3[:M, :],
                                                 lhsT=cc2_sb[pb:pb + 50, :],
                                                 rhs=th1[pb:pb + 50, :],
                                                 start=True, stop=False,
                                                 tile_position=(pb, 0))
                                nc.tensor.matmul(
                                    out=ps3[:M, :],
                                    lhsT=cc4_sb[32 * c:32 * c + 25, :],
                                    rhs=y0s[32 * c:32 * c + 25, :],
                                    start=False, stop=True,
                                    tile_position=(32 * c, 0))
                                o = (c * ngrp + g) * CH
                                nc.scalar.activation(out=gg[:, o:o + CH],
                                                     in_=ps2[:M, :], func=AF.Tanh,
                                                     bias=WS[("b2", seg)][:])
                                nc.vector.tensor_tensor(out=gg[:, o:o + CH],
                                                        in0=gg[:, o:o + CH],
                                                        in1=ps3[:M, :], op=ALU.add)

                    # contraction
                    t0all = t0p.tile([46, 64, M], F32, tag="t0all", name="t0all")
                    for a0 in range(64):
                        tp = ps_t.tile([128, 128], F32, tag="tpt", name="tp0",
                                       space="PSUM")
                        nc.tensor.transpose(out=tp[:46, :M],
                                            in_=gg_blk[0][:, a0 * 46:(a0 + 1) * 46],
                                            identity=ident[0:M, 0:M])
                        nc.vector.tensor_copy(out=t0all[:, a0, :], in_=tp[0:46, :M])
                    obuf = outp_pool.tile([M, 64, AXIS], BF16, tag="obuf",
                                          name="obuf")
                    for a in range(64):
                        tp = ps_t.tile([128, 128], F32, tag="tpt", name="tp1",
                                       space="PSUM")
                        nc.tensor.transpose(out=tp[:92, :M],
                                            in_=gg_blk[1][:, a * 92:(a + 1) * 92],
                                            identity=ident[0:M, 0:M])
                        t1 = tsbp.tile([92, M], F32, tag="t1", name="t1")
                        nc.vector.tensor_copy(out=t1[:], in_=tp[:92, :M])

                        A = blk * 64 + a
                        xyz_ps = ps_small.tile([4, M], F32, tag="small",
                                               name="xyzp", space="PSUM")
                        lhs0 = mkap(rr0, A, [[S, 4]])
                        nc.tensor.matmul(out=xyz_ps[:], lhsT=lhs0,
                                         rhs=t0all[:, a, :], start=True, stop=False)
                        lhs1 = mkap(rr1, A, [[S, 4]])
                        nc.tensor.matmul(out=xyz_ps[:], lhsT=lhs1, rhs=t1[:],
                                         start=False, stop=True)
                        xyz = tsbp.tile([4, M], F32, tag="xyzs", name="xyzs")
                        nc.scalar.activation(out=xyz[:], in_=xyz_ps[:],
                                             func=AF.Copy, scale=1.0 / NNEI)
                        res_ps = ps_small.tile([M, AXIS], F32, tag="small",
                                               name="resp", space="PSUM")
                        nc.tensor.matmul(out=res_ps[:], lhsT=xyz[:],
                                         rhs=xyz[:, 0:AXIS], start=True, stop=True)
                        nc.vector.tensor_copy(out=obuf[:, a, :], in_=res_ps[:])
                    src = mkap(obuf, 0, [[AXIS, 64], [1, AXIS]])
                    dst = bass.AP(res_d, (f * S + blk * 64) * M * AXIS,
                                  [[AXIS, M], [M * AXIS, 64], [1, AXIS]])
                    nc.sync.dma_start(out=dst, in_=src)

            for f in range(NF):
                process_frame(f)

    nc.finalize()
    return nc


# ---------------------------------------------------------------------------
# cached dispatch (shard_map over 8 cores, built once)
# ---------------------------------------------------------------------------
_EXEC = None
_OUT_BUF = None


def _get_exec():
    global _EXEC
    if _EXEC is not None:
        return _EXEC
    import concourse.mybir as mybir
    from concourse.bass2jax import (_bass_exec_p, install_neuronx_cc_hook,
                                    partition_id_tensor)
    from jax.experimental.shard_map import shard_map
    from jax.sharding import Mesh, PartitionSpec

    install_neuronx_cc_hook()
    nc = _build_kernel()

    partition_name = (nc.partition_id_tensor.name
                      if nc.partition_id_tensor else None)
    in_names, out_names, out_avals, zero_shapes = [], [], [], []
    for alloc in nc.m.functions[0].allocations:
        if not isinstance(alloc, mybir.MemoryLocationSet):
            continue
        name = alloc.memorylocations[0].name
        if alloc.kind == "ExternalInput":
            if name != partition_name:
                in_names.append(name)
        elif alloc.kind == "ExternalOutput":
            out_names.append(name)
            shape = tuple(alloc.tensor_shape)
            dtype = mybir.dt.np(alloc.dtype)
            out_avals.append(jax.core.ShapedArray(shape, dtype))
            zero_shapes.append((shape, dtype))
    n_params = len(in_names)
    n_outs = len(out_avals)
    all_in_names = list(in_names) + list(out_names)
    if partition_name is not None:
        all_in_names.append(partition_name)
    donate = tuple(range(n_params, n_params + n_outs))

    def _body(*args):
        operands = list(args)
        if partition_name is not None:
            operands.append(partition_id_tensor())
        outs = _bass_exec_p.bind(
            *operands,
            out_avals=tuple(out_avals),
            in_names=tuple(all_in_names),
            out_names=tuple(out_names),
            lowering_input_output_aliases=(),
            sim_require_finite=True,
            sim_require_nnan=True,
            nc=nc,
        )
        return tuple(outs)

    devices = jax.devices()[:NCORES]
    mesh = Mesh(np.asarray(devices), ("core",))
    in_specs = (PartitionSpec("core"),) * (n_params + n_outs)
    out_specs = (PartitionSpec("core"),) * n_outs
    sharded = jax.jit(
        shard_map(_body, mesh=mesh, in_specs=in_specs, out_specs=out_specs,
                  check_rep=False),
        donate_argnums=donate, keep_unused=True)

    from jax.sharding import NamedSharding
    import jax.numpy as jnp
    shardings = [NamedSharding(mesh, PartitionSpec("core"))] * n_outs

    def _mk_zeros():
        return tuple(jnp.zeros((NCORES * shp[0],) + tuple(shp[1:]), dt)
                     for shp, dt in zero_shapes)
    zero_maker = jax.jit(_mk_zeros, out_shardings=tuple(shardings))
    _EXEC = (sharded, in_names, out_names, zero_maker)
    return _EXEC


# ---------------------------------------------------------------------------
# host-side prep + entry point
# ---------------------------------------------------------------------------
def _prep_global_inputs(nlist, coord, atype, mean, stddev, ws, bs):
    """Build the concatenated (8*dim0, ...) arrays for every DRAM input."""
    g = {}
    nl16 = np.asarray(nlist, dtype=np.int16)
    g["nl"] = np.ascontiguousarray(
        nl16.reshape(NF, NCORES, 128, COLS).transpose(1, 0, 2, 3)
    ).reshape(NCORES * NF, 128, COLS)

    coord = np.asarray(coord, dtype=np.float32)
    coord4 = np.zeros((NF, NALL, 4), np.float32)
    coord4[:, :, 0:3] = coord
    shr = NALL // NCORES
    g["coordsh"] = np.ascontiguousarray(
        coord4.reshape(NF, NCORES, shr, 4).transpose(1, 0, 2, 3)
    ).reshape(NCORES * NF * shr, 4)

    cat = np.zeros((NF, NLOC, 4), np.float32)
    cat[:, :, 0:3] = coord[:, :NLOC, :]
    cat[:, :, 3] = np.asarray(atype)[:, :NLOC].astype(np.float32)
    g["catype"] = np.ascontiguousarray(
        cat.reshape(NF, NCORES, 128, 16).transpose(1, 0, 2, 3)
    ).reshape(NCORES * NF, 128, 16)

    mean = np.asarray(mean, np.float32)
    stddev = np.asarray(stddev, np.float32)
    istd = 1.0 / stddev
    nmean = -mean / stddev
    nrm = np.stack([
        np.tile(istd[0], (4, 1)),
        np.tile(istd[1] - istd[0], (4, 1)),
        np.tile(nmean[0], (4, 1)),
        np.tile(nmean[1] - nmean[0], (4, 1)),
    ]).astype(ml_dtypes.bfloat16)
    g["nrm"] = np.tile(nrm, (NCORES, 1, 1))

    w0, w1, w2 = [np.asarray(w, np.float32) for w in ws]
    b0, b1, b2 = [np.asarray(b, np.float32) for b in bs]
    w0bd = np.zeros((2, 4, 128), np.float32)
    b0s = np.zeros((2, 128, 1), np.float32)
    w1r = np.zeros((2, 121, 50), np.float32)
    b1s = np.zeros((2, 128, 1), np.float32)
    w2r = np.zeros((2, 114, M), np.float32)
    b2s = np.zeros((2, M, 1), np.float32)
    for t in range(2):
        for c in range(4):
            w0bd[t, c, 32 * c:32 * c + 25] = w0[t, 0]
            b0s[t, 32 * c:32 * c + 25, 0] = b0[t]
            w1r[t, 32 * c:32 * c + 25, :] = w1[t]
        for h in range(2):
            b1s[t, 64 * h:64 * h + 50, 0] = b1[t]
            w2r[t, 64 * h:64 * h + 50, :] = w2[t]
        b2s[t, :, 0] = b2[t]
    w2sc = (w2[:, 0:25, :] + w2[:, 25:50, :]).astype(np.float32)
    for nm, arr in (("w0bd", w0bd), ("b0s", b0s), ("w1r", w1r), ("b1s", b1s),
                    ("w2r", w2r), ("b2s", b2s), ("w2sc", w2sc)):
        g[nm] = np.tile(arr, (NCORES,) + (1,) * (arr.ndim - 1))
    return g


def kernel(nlist, extended_coord, extended_atype, mean, stddev,
           w0, b0, w1, b1, w2, b2):
    sharded, in_names, out_names, zero_maker = _get_exec()
    g = _prep_global_inputs(nlist, extended_coord, extended_atype, mean, stddev,
                            [w0, w1, w2], [b0, b1, b2])
    args = [g[n] for n in in_names]
    zeros = zero_maker()
    out_arrs = sharded(*args, *zeros)
    res = np.asarray(out_arrs[out_names.index("res")])  # [8*NF, S, 1600]
    r = res.reshape(NCORES, NF, S, M * AXIS)
    global _OUT_BUF
    if _OUT_BUF is None:
        _OUT_BUF = np.empty((NF, NLOC, M * AXIS), np.float32)
    full = _OUT_BUF
    for c in range(NCORES):
        for f in range(NF):
            np.copyto(full[f, c * S:(c + 1) * S, :], r[c, f])
    return full


# revision 15
# speedup vs baseline: 3.0213x; 1.0048x over previous
"""DescrptSeA descriptor on 8 Trainium2 NeuronCores via a hand-written
Bass/Tile kernel (SPMD over the nloc axis, 512 atoms per core).

Per-core device kernel (see _build_kernel): neighbor-coordinate gather via
indirect DMA, smoothed env matrix, 3-layer embedding net (resnet skips folded
into PE matmuls via PSUM accumulation), per-atom contraction to the
[M*AXIS]=1600 descriptor. Host only casts/reshapes inputs and reassembles the
output; the compiled executable is cached across calls.
"""
import numpy as np
import jax
import ml_dtypes
from concurrent.futures import ThreadPoolExecutor

NF = 2
NLOC, NALL = 4096, 8192
S = 512
NNEI = 138
SEL = [46, 92]
M = 100
AXIS = 16
PROT = 1e-6
RMIN, RMAX = 0.5, 6.0
COLS = 552
CH = 368
NCORES = 8


# ---------------------------------------------------------------------------
# device kernel builder
# ---------------------------------------------------------------------------
def _build_kernel():
    import concourse.bass as bass
    import concourse.bacc as bacc
    import concourse.tile as tile
    from concourse import mybir
    from concourse.masks import make_identity

    F32 = mybir.dt.float32
    I32 = mybir.dt.int32
    BF16 = mybir.dt.bfloat16
    AF = mybir.ActivationFunctionType
    ALU = mybir.AluOpType
    AX = mybir.AxisListType

    def ap_of(t):
        return t[:] if not isinstance(t, bass.AP) else t

    def mkap(t, offset_elems, free_dims, parts=None, part_off=0):
        a = ap_of(t)
        pitch = a.ap[0][0]
        p = [pitch, parts if parts is not None else a.ap[0][1]]
        return bass.AP(a.tensor, a.offset + part_off * pitch + offset_elems,
                       [p] + [list(d) for d in free_dims])

    nc = bacc.Bacc(None, target_bir_lowering=False, debug=False)

    I16 = mybir.dt.int16
    nl_d = nc.dram_tensor("nl", [NF, 128, COLS], I16, kind="ExternalInput")
    coordsh_d = nc.dram_tensor("coordsh", [NF * (NALL // NCORES), 4], F32,
                               kind="ExternalInput")
    ctr_d = nc.dram_tensor("catype", [NF, 128, 16], F32, kind="ExternalInput")
    nrm_d = nc.dram_tensor("nrm", [4, COLS, 4], BF16, kind="ExternalInput")
    w0_d = nc.dram_tensor("w0bd", [2, 4, 128], F32, kind="ExternalInput")
    b0_d = nc.dram_tensor("b0s", [2, 128, 1], F32, kind="ExternalInput")
    w1_d = nc.dram_tensor("w1c", [2, 25, 50], F32, kind="ExternalInput")
    b1_d = nc.dram_tensor("b1s", [2, 128, 1], F32, kind="ExternalInput")
    w2_d = nc.dram_tensor("w2c", [2, 50, M], F32, kind="ExternalInput")
    b2_d = nc.dram_tensor("b2s", [2, M, 1], F32, kind="ExternalInput")
    w2sc_d = nc.dram_tensor("w2sc", [2, 25, M], F32, kind="ExternalInput")
    res_d = nc.dram_tensor("res", [NF, S, M * AXIS], BF16, kind="ExternalOutput")

    with tile.TileContext(nc) as tc:
        with (
            tc.tile_pool(name="const", bufs=1) as constp,
            tc.tile_pool(name="frame", bufs=1) as framep,
            tc.tile_pool(name="gather", bufs=2) as gatherp,
            tc.tile_pool(name="envt", bufs=1) as envp,
            tc.tile_pool(name="ssb", bufs=2) as ssbp,
            tc.tile_pool(name="mlp", bufs=2) as mlpp,
            tc.tile_pool(name="ggp", bufs=1) as ggp,
            tc.tile_pool(name="tsb", bufs=4) as tsbp,
            tc.tile_pool(name="t0p", bufs=1) as t0p,
            tc.tile_pool(name="outp", bufs=1) as outp_pool,
            tc.tile_pool(name="ps_mlp", bufs=3, space="PSUM") as ps_mlp,
            tc.tile_pool(name="ps_b", bufs=1, space="PSUM") as ps_b,
            tc.tile_pool(name="ps_t", bufs=2, space="PSUM") as ps_t,
            tc.tile_pool(name="ps_small", bufs=2, space="PSUM") as ps_small,
            tc.tile_pool(name="dramp", bufs=1, space="DRAM") as dramp,
        ):
            # all-gather the full coordinate table from the per-core shards
            SHR = NALL // NCORES
            agout = []
            for f in range(NF):
                agi = dramp.tile([SHR, 4], F32, tag=f"agi{f}", name=f"agi{f}")
                nc.gpsimd.dma_start(out=agi[:],
                                    in_=coordsh_d[f * SHR:(f + 1) * SHR, :])
                ago = dramp.tile([NALL, 4], F32, tag=f"ago{f}", name=f"ago{f}")
                nc.gpsimd.collective_compute(
                    "AllGather", mybir.AluOpType.bypass,
                    replica_groups=[list(range(NCORES))],
                    ins=[agi.opt()], outs=[ago.opt()])
                agout.append(ago)
            ident = constp.tile([128, 128], F32)
            make_identity(nc, ident)
            nrm_sb = []
            for i in range(4):
                t = constp.tile([128, COLS, 4], BF16, tag=f"nrm{i}", name=f"nrm{i}")
                src = bass.AP(nrm_d, i * COLS * 4, [[0, 128], [4, COLS], [1, 4]])
                nc.sync.dma_start(out=t[:], in_=src)
                nrm_sb.append(t)
            nsc0_sb, nscd_sb, nsh0_sb, nshd_sb = nrm_sb
            WS = {}
            for t in range(2):
                for nm, d, shp in (("w0", w0_d, [4, 128]), ("b0", b0_d, [128, 1]),
                                   ("b1", b1_d, [128, 1]), ("b2", b2_d, [M, 1])):
                    tl = constp.tile(shp, F32, tag=f"{nm}_{t}", name=f"{nm}_{t}")
                    nc.sync.dma_start(out=tl[:], in_=d[t])
                    WS[(nm, t)] = tl
                w1t = constp.tile([121, 50], F32, tag=f"w1_{t}", name=f"w1_{t}")
                for c in range(4):
                    nc.sync.dma_start(out=w1t[32 * c:32 * c + 25, :], in_=w1_d[t])
                WS[("w1", t)] = w1t
                w2t = constp.tile([114, M], F32, tag=f"w2_{t}", name=f"w2_{t}")
                for h in range(2):
                    nc.sync.dma_start(out=w2t[64 * h:64 * h + 50, :], in_=w2_d[t])
                WS[("w2", t)] = w2t
            # cc2r = [I50;I50] at row-bases {0,64}; cc4r = I25 x4 at 32c
            cc2_sb = constp.tile([114, M], F32, tag="cc2", name="cc2")
            nc.vector.memset(cc2_sb[:], 0.0)
            for rb in (0, 64):
                for cb in (0, 50):
                    nc.vector.tensor_copy(out=cc2_sb[rb:rb + 50, cb:cb + 50],
                                          in_=ident[0:50, 0:50])
            cc4_sb = constp.tile([121, M], F32, tag="cc4", name="cc4")
            nc.vector.memset(cc4_sb[:], 0.0)
            for c in range(4):
                for k in range(4):
                    nc.vector.tensor_copy(
                        out=cc4_sb[32 * c:32 * c + 25, 25 * k:25 * k + 25],
                        in_=ident[0:25, 0:25])
            # w2s (host-compacted w2[0:25]+w2[25:50]) replicated at 32c
            for t in range(2):
                w2st = constp.tile([121, M], F32, tag=f"w2s_{t}", name=f"w2s_{t}")
                for c in range(4):
                    nc.sync.dma_start(out=w2st[32 * c:32 * c + 25, :],
                                      in_=w2sc_d[t])
                WS[("w2s", t)] = w2st

            def process_frame(f):
                # ---- Phase E: env matrix (chunked layout) ----
                it = gatherp.tile([128, COLS], I16, tag="it", name="it")
                nc.sync.dma_start(out=it[:], in_=nl_d[f])
                mask = envp.tile([128, COLS], F32, tag="mask", name="mask")
                nc.vector.tensor_scalar(out=mask[:], in0=it[:], scalar1=0,
                                        scalar2=None, op0=ALU.is_ge)
                itc = gatherp.tile([128, COLS], I32, tag="itc", name="itc")
                nc.vector.tensor_scalar(out=itc[:], in0=it[:], scalar1=0,
                                        scalar2=None, op0=ALU.max)
                gt = gatherp.tile([128, COLS, 4], F32, tag="gt", name="gt")
                for k in range(COLS):
                    nc.gpsimd.indirect_dma_start(
                        out=gt[:, k, :], out_offset=None, in_=agout[f][:],
                        in_offset=bass.IndirectOffsetOnAxis(ap=itc[:, k:k + 1],
                                                            axis=0),
                    )
                ctr = framep.tile([128, 4, 4], F32, tag="ctr", name="ctr")
                nc.sync.dma_start(out=ctr[:],
                                  in_=ctr_d[f].rearrange("p (q c) -> p q c", q=4))

                diff = envp.tile([128, COLS, 3], F32, tag="diff", name="diff")
                ctr_b = mkap(ctr, 0, [[4, 4], [0, NNEI], [1, 3]])
                nc.vector.tensor_tensor(out=diff[:], in0=gt[:, :, 0:3], in1=ctr_b,
                                        op=ALU.subtract)
                sq = envp.tile([128, COLS, 3], F32, tag="sq", name="sq")
                nc.vector.tensor_tensor(out=sq[:], in0=diff[:], in1=diff[:],
                                        op=ALU.mult)
                r2 = envp.tile([128, COLS, 1], F32, tag="r2", name="r2")
                nc.vector.tensor_reduce(out=r2[:], in_=sq[:], axis=AX.X, op=ALU.add)
                r = envp.tile([128, COLS], F32, tag="r", name="r")
                nc.scalar.activation(out=r[:], in_=r2[:, :, 0], func=AF.Sqrt)
                sr = envp.tile([128, COLS], F32, tag="sr", name="sr")
                nc.vector.tensor_scalar(out=sr[:], in0=r[:], scalar1=PROT,
                                        scalar2=None, op0=ALU.add)
                nc.vector.reciprocal(out=sr[:], in_=sr[:])
                sr2 = envp.tile([128, COLS], F32, tag="sr2", name="sr2")
                nc.vector.tensor_tensor(out=sr2[:], in0=sr[:], in1=sr[:], op=ALU.mult)
                uu = envp.tile([128, COLS], F32, tag="uu", name="uu")
                nc.vector.tensor_scalar(out=uu[:], in0=r[:], scalar1=-RMIN,
                                        scalar2=1.0 / (RMAX - RMIN),
                                        op0=ALU.add, op1=ALU.mult)
                nc.vector.tensor_scalar(out=uu[:], in0=uu[:], scalar1=0.0,
                                        scalar2=1.0, op0=ALU.max, op1=ALU.min)
                u2 = envp.tile([128, COLS], F32, tag="u2", name="u2")
                nc.vector.tensor_tensor(out=u2[:], in0=uu[:], in1=uu[:], op=ALU.mult)
                nc.vector.tensor_tensor(out=u2[:], in0=u2[:], in1=uu[:], op=ALU.mult)
                p1 = envp.tile([128, COLS], F32, tag="p1", name="p1")
                nc.vector.tensor_scalar(out=p1[:], in0=uu[:], scalar1=-6.0,
                                        scalar2=15.0, op0=ALU.mult, op1=ALU.add)
                nc.vector.tensor_tensor(out=p1[:], in0=p1[:], in1=uu[:], op=ALU.mult)
                nc.vector.tensor_scalar(out=p1[:], in0=p1[:], scalar1=-10.0,
                                        scalar2=None, op0=ALU.add)
                nc.vector.tensor_tensor(out=p1[:], in0=p1[:], in1=u2[:], op=ALU.mult)
                nc.vector.tensor_scalar(out=p1[:], in0=p1[:], scalar1=1.0,
                                        scalar2=None, op0=ALU.add)
                wm = envp.tile([128, COLS], F32, tag="wm", name="wm")
                nc.vector.tensor_tensor(out=wm[:], in0=p1[:], in1=mask[:],
                                        op=ALU.mult)

                envw = framep.tile([128, COLS, 4], F32, tag="envw", name="envw")
                nc.vector.tensor_copy(out=envw[:, :, 0], in_=sr[:])
                sr2_b = mkap(sr2, 0, [[1, COLS], [0, 3]])
                nc.vector.tensor_tensor(out=envw[:, :, 1:4], in0=diff[:], in1=sr2_b,
                                        op=ALU.mult)
                tpt = envp.tile([128, COLS], F32, tag="r2", name="tpt")
                tpt_src = mkap(ctr, 3, [[4, 4], [0, NNEI]])
                nc.vector.tensor_copy(out=tpt[:], in_=tpt_src)
                tpt_b = mkap(tpt, 0, [[1, COLS], [0, 4]])
                wm_b = mkap(wm, 0, [[1, COLS], [0, 4]])
                x1 = envp.tile([128, COLS, 4], F32, tag="sq", name="x1")
                nc.vector.tensor_tensor(out=x1[:], in0=nscd_sb[:], in1=tpt_b,
                                        op=ALU.mult)
                nc.vector.tensor_tensor(out=x1[:], in0=x1[:], in1=nsc0_sb[:],
                                        op=ALU.add)
                nc.vector.tensor_tensor(out=x1[:], in0=x1[:], in1=wm_b, op=ALU.mult)
                dm = framep.tile([128, COLS, 4], F32, tag="dm", name="dm")
                nc.vector.tensor_tensor(out=dm[:], in0=envw[:], in1=x1[:],
                                        op=ALU.mult)
                y1 = envp.tile([128, COLS, 4], F32, tag="diff", name="y1")
                nc.vector.tensor_tensor(out=y1[:], in0=nshd_sb[:], in1=tpt_b,
                                        op=ALU.mult)
                nc.vector.tensor_tensor(out=y1[:], in0=y1[:], in1=nsh0_sb[:],
                                        op=ALU.add)
                nc.vector.tensor_tensor(out=dm[:], in0=dm[:], in1=y1[:], op=ALU.add)

                # ---- Phase T: rr to slot-major [sel, 4, S] ----
                rr0 = framep.tile([SEL[0], 4, S], F32, tag="rr0", name="rr0")
                rr1 = framep.tile([SEL[1], 4, S], F32, tag="rr1", name="rr1")
                for q in range(4):
                    for ch in range(4):
                        for rr_sb, j0, sel in ((rr0, 0, SEL[0]),
                                               (rr1, SEL[0], SEL[1])):
                            src = mkap(dm, (q * NNEI + j0) * 4 + ch, [[4, sel]])
                            tp = ps_t.tile([128, 128], F32, tag="tpt", name="tpq",
                                           space="PSUM")
                            nc.tensor.transpose(out=tp[:sel, :], in_=src,
                                                identity=ident[:])
                            dst = mkap(rr_sb, ch * S + q, [[4, 128]])
                            nc.vector.tensor_copy(out=dst, in_=tp[:sel, :])

                ssc = framep.tile([128, COLS], F32, tag="ssc", name="ssc")
                nc.vector.tensor_copy(out=ssc[:], in_=dm[:, :, 0])

                # ---- per 64-atom block: MLP + contraction ----
                for blk in range(8):
                    ss_t = {}
                    for seg, (sel, ngrp) in enumerate(((SEL[0], 2), (SEL[1], 4))):
                        sst = ssbp.tile([4, ngrp * CH], F32, tag=f"ss{seg}",
                                        name=f"ss{seg}")
                        j0 = 0 if seg == 0 else SEL[0]
                        src = mkap(ssc, j0, [[NNEI, 4], [1, sel]],
                                   parts=16, part_off=16 * blk)
                        dst = mkap(sst, 0, [[CH, ngrp], [1, CH]])
                        nc.sync.dma_start(out=dst, in_=src)
                        ss_t[seg] = (sst, ngrp, sel)

                    gg_blk = {}
                    for seg in (0, 1):
                        sst, ngrp, sel = ss_t[seg]
                        gg = ggp.tile([M, 64 * sel], F32, tag=f"gg{seg}",
                                      name=f"gg{seg}")
                        gg_blk[seg] = gg
                        for g in range(ngrp):
                            ps0 = ps_mlp.tile([128, CH], F32, tag="psA", name="ps0",
                                              space="PSUM")
                            nc.tensor.matmul(out=ps0[:], lhsT=WS[("w0", seg)][:],
                                             rhs=sst[:, g * CH:(g + 1) * CH],
                                             start=True, stop=True,
                                             tile_position=(0, 0))
                            y0s = mlpp.tile([128, CH], F32, tag="y0s", name="y0s")
                            nc.scalar.activation(out=y0s[:], in_=ps0[:],
                                                 func=AF.Tanh,
                                                 bias=WS[("b0", seg)][:])
                            th1s = []
                            for half in range(2):
                                ps1 = ps_mlp.tile([128, CH], F32, tag="psA",
                                                  name="ps1", space="PSUM")
                                for ci in range(2):
                                    c = half * 2 + ci
                                    nc.tensor.matmul(
                                        out=ps1[64 * ci:64 * ci + 50, :],
                                        lhsT=WS[("w1", seg)][32 * c:32 * c + 25, :],
                                        rhs=y0s[32 * c:32 * c + 25, :],
                                        start=True, stop=True,
                                        tile_position=(32 * c, 64 * ci))
                                th1 = mlpp.tile([128, CH], F32, tag="y1s",
                                                name="th1")
                                nc.scalar.activation(out=th1[:], in_=ps1[:],
                                                     func=AF.Tanh,
                                                     bias=WS[("b1", seg)][:])
                                th1s.append(th1)
                            for c in range(4):
                                th1 = th1s[c // 2]
                                pb = 64 * (c % 2)
                                ps2 = ps_mlp.tile([128, CH], F32, tag="psA",
                                                  name="ps2", space="PSUM")
                                nc.tensor.matmul(out=ps2[:M, :],
                                                 lhsT=WS[("w2", seg)][pb:pb + 50, :],
                                                 rhs=th1[pb:pb + 50, :],
                                                 start=True, stop=False,
                                                 tile_position=(pb, 0))
                                nc.tensor.matmul(
                                    out=ps2[:M, :],
                                    lhsT=WS[("w2s", seg)][32 * c:32 * c + 25, :],
                                    rhs=y0s[32 * c:32 * c + 25, :],
                                    start=False, stop=True,
                                    tile_position=(32 * c, 0))
                                ps3 = ps_b.tile([128, CH], F32, tag="psB",
                                                name="ps3", space="PSUM")
                                nc.tensor.matmul(out=ps3[:M, :],
                                                 lhsT=cc2_sb[pb:pb + 50, :],
                                                 rhs=th1[pb:pb + 50, :],
                                                 start=True, stop=False,
                                                 tile_position=(pb, 0))
                                nc.tensor.matmul(
                                    out=ps3[:M, :],
                                    lhsT=cc4_sb[32 * c:32 * c + 25, :],
                                    rhs=y0s[32 * c:32 * c + 25, :],
                                    start=False, stop=True,
                                    tile_position=(32 * c, 0))
                                o = (c * ngrp + g) * CH
                                nc.scalar.activation(out=gg[:, o:o + CH],
                                                     in_=ps2[:M, :], func=AF.Tanh,
                                                     bias=WS[("b2", seg)][:])
                                nc.vector.tensor_tensor(out=gg[:, o:o + CH],
                                                        in0=gg[:, o:o + CH],
                                                        in1=ps3[:M, :], op=ALU.add)

                    # contraction
                    t0all = t0p.tile([46, 64, M], F32, tag="t0all", name="t0all")
                    for a0 in range(64):
                        tp = ps_t.tile([128, 128], F32, tag="tpt", name="tp0",
                                       space="PSUM")
                        nc.tensor.transpose(out=tp[:46, :M],
                                            in_=gg_blk[0][:, a0 * 46:(a0 + 1) * 46],
                                            identity=ident[0:M, 0:M])
                        nc.vector.tensor_copy(out=t0all[:, a0, :], in_=tp[0:46, :M])
                    obuf = outp_pool.tile([M, 64, AXIS], BF16, tag="obuf",
                                          name="obuf")
                    for a in range(64):
                        tp = ps_t.tile([128, 128], F32, tag="tpt", name="tp1",
                                       space="PSUM")
                        nc.tensor.transpose(out=tp[:92, :M],
                                            in_=gg_blk[1][:, a * 92:(a + 1) * 92],
                                            identity=ident[0:M, 0:M])
                        t1 = tsbp.tile([92, M], F32, tag="t1", name="t1")
                        nc.vector.tensor_copy(out=t1[:], in_=tp[:92, :M])

                        A = blk * 64 + a
                        xyz_ps = ps_small.tile([4, M], F32, tag="small",
                                               name="xyzp", space="PSUM")
                        lhs0 = mkap(rr0, A, [[S, 4]])
                        nc.tensor.matmul(out=xyz_ps[:], lhsT=lhs0,
                                         rhs=t0all[:, a, :], start=True, stop=False)
                        lhs1 = mkap(rr1, A, [[S, 4]])
                        nc.tensor.matmul(out=xyz_ps[:], lhsT=lhs1, rhs=t1[:],
                                         start=False, stop=True)
                        xyz = tsbp.tile([4, M], F32, tag="xyzs", name="xyzs")
                        nc.scalar.activation(out=xyz[:], in_=xyz_ps[:],
                                             func=AF.Copy, scale=1.0 / NNEI)
                        res_ps = ps_small.tile([M, AXIS], F32, tag="small",
                                               name="resp", space="PSUM")
                        nc.tensor.matmul(out=res_ps[:], lhsT=xyz[:],
                                         rhs=xyz[:, 0:AXIS], start=True, stop=True)
                        nc.vector.tensor_copy(out=obuf[:, a, :], in_=res_ps[:])
                    src = mkap(obuf, 0, [[AXIS, 64], [1, AXIS]])
                    dst = bass.AP(res_d, (f * S + blk * 64) * M * AXIS,
                                  [[AXIS, M], [M * AXIS, 64], [1, AXIS]])
                    nc.sync.dma_start(out=dst, in_=src)

            for f in range(NF):
                process_frame(f)

    nc.finalize()
    return nc


# ---------------------------------------------------------------------------
# cached dispatch (shard_map over 8 cores, built once)
# ---------------------------------------------------------------------------
_EXEC = None
_OUT_BUF = None


def _get_exec():
    global _EXEC
    if _EXEC is not None:
        return _EXEC
    import concourse.mybir as mybir
    from concourse.bass2jax import (_bass_exec_p, install_neuronx_cc_hook,
                                    partition_id_tensor)
    from jax.experimental.shard_map import shard_map
    from jax.sharding import Mesh, PartitionSpec

    install_neuronx_cc_hook()
    nc = _build_kernel()

    partition_name = (nc.partition_id_tensor.name
                      if nc.partition_id_tensor else None)
    in_names, out_names, out_avals, zero_shapes = [], [], [], []
    for alloc in nc.m.functions[0].allocations:
        if not isinstance(alloc, mybir.MemoryLocationSet):
            continue
        name = alloc.memorylocations[0].name
        if alloc.kind == "ExternalInput":
            if name != partition_name:
                in_names.append(name)
        elif alloc.kind == "ExternalOutput":
            out_names.append(name)
            shape = tuple(alloc.tensor_shape)
            dtype = mybir.dt.np(alloc.dtype)
            out_avals.append(jax.core.ShapedArray(shape, dtype))
            zero_shapes.append((shape, dtype))
    n_params = len(in_names)
    n_outs = len(out_avals)
    all_in_names = list(in_names) + list(out_names)
    if partition_name is not None:
        all_in_names.append(partition_name)
    donate = tuple(range(n_params, n_params + n_outs))

    def _body(*args):
        operands = list(args)
        if partition_name is not None:
            operands.append(partition_id_tensor())
        outs = _bass_exec_p.bind(
            *operands,
            out_avals=tuple(out_avals),
            in_names=tuple(all_in_names),
            out_names=tuple(out_names),
            lowering_input_output_aliases=(),
            sim_require_finite=True,
            sim_require_nnan=True,
            nc=nc,
        )
        return tuple(outs)

    devices = jax.devices()[:NCORES]
    mesh = Mesh(np.asarray(devices), ("core",))
    in_specs = (PartitionSpec("core"),) * (n_params + n_outs)
    out_specs = (PartitionSpec("core"),) * n_outs
    sharded = jax.jit(
        shard_map(_body, mesh=mesh, in_specs=in_specs, out_specs=out_specs,
                  check_rep=False),
        donate_argnums=donate, keep_unused=True)

    from jax.sharding import NamedSharding
    import jax.numpy as jnp
    shardings = [NamedSharding(mesh, PartitionSpec("core"))] * n_outs

    def _mk_zeros():
        return tuple(jnp.zeros((NCORES * shp[0],) + tuple(shp[1:]), dt)
                     for shp, dt in zero_shapes)
    zero_maker = jax.jit(_mk_zeros, out_shardings=tuple(shardings))
    _EXEC = (sharded, in_names, out_names, zero_maker)
    return _EXEC


# ---------------------------------------------------------------------------
# host-side prep + entry point
# ---------------------------------------------------------------------------
def _prep_global_inputs(nlist, coord, atype, mean, stddev, ws, bs):
    """Build the concatenated (8*dim0, ...) arrays for every DRAM input."""
    g = {}
    nl16 = np.asarray(nlist, dtype=np.int16)
    g["nl"] = np.ascontiguousarray(
        nl16.reshape(NF, NCORES, 128, COLS).transpose(1, 0, 2, 3)
    ).reshape(NCORES * NF, 128, COLS)

    coord = np.asarray(coord, dtype=np.float32)
    coord4 = np.zeros((NF, NALL, 4), np.float32)
    coord4[:, :, 0:3] = coord
    shr = NALL // NCORES
    g["coordsh"] = np.ascontiguousarray(
        coord4.reshape(NF, NCORES, shr, 4).transpose(1, 0, 2, 3)
    ).reshape(NCORES * NF * shr, 4)

    cat = np.zeros((NF, NLOC, 4), np.float32)
    cat[:, :, 0:3] = coord[:, :NLOC, :]
    cat[:, :, 3] = np.asarray(atype)[:, :NLOC].astype(np.float32)
    g["catype"] = np.ascontiguousarray(
        cat.reshape(NF, NCORES, 128, 16).transpose(1, 0, 2, 3)
    ).reshape(NCORES * NF, 128, 16)

    mean = np.asarray(mean, np.float32)
    stddev = np.asarray(stddev, np.float32)
    istd = 1.0 / stddev
    nmean = -mean / stddev
    nrm = np.stack([
        np.tile(istd[0], (4, 1)),
        np.tile(istd[1] - istd[0], (4, 1)),
        np.tile(nmean[0], (4, 1)),
        np.tile(nmean[1] - nmean[0], (4, 1)),
    ]).astype(ml_dtypes.bfloat16)
    g["nrm"] = np.tile(nrm, (NCORES, 1, 1))

    w0, w1, w2 = [np.asarray(w, np.float32) for w in ws]
    b0, b1, b2 = [np.asarray(b, np.float32) for b in bs]
    w0bd = np.zeros((2, 4, 128), np.float32)
    b0s = np.zeros((2, 128, 1), np.float32)
    b1s = np.zeros((2, 128, 1), np.float32)
    b2s = np.zeros((2, M, 1), np.float32)
    for t in range(2):
        for c in range(4):
            w0bd[t, c, 32 * c:32 * c + 25] = w0[t, 0]
            b0s[t, 32 * c:32 * c + 25, 0] = b0[t]
        for h in range(2):
            b1s[t, 64 * h:64 * h + 50, 0] = b1[t]
        b2s[t, :, 0] = b2[t]
    w2sc = (w2[:, 0:25, :] + w2[:, 25:50, :]).astype(np.float32)
    for nm, arr in (("w0bd", w0bd), ("b0s", b0s), ("w1c", w1), ("b1s", b1s),
                    ("w2c", w2), ("b2s", b2s), ("w2sc", w2sc)):
        g[nm] = np.tile(arr, (NCORES,) + (1,) * (arr.ndim - 1))
    return g


def kernel(nlist, extended_coord, extended_atype, mean, stddev,
           w0, b0, w1, b1, w2, b2):
    sharded, in_names, out_names, zero_maker = _get_exec()
    g = _prep_global_inputs(nlist, extended_coord, extended_atype, mean, stddev,
                            [w0, w1, w2], [b0, b1, b2])
    args = [g[n] for n in in_names]
    zeros = zero_maker()
    out_arrs = sharded(*args, *zeros)
    res = np.asarray(out_arrs[out_names.index("res")])  # [8*NF, S, 1600]
    r = res.reshape(NCORES, NF, S, M * AXIS)
    global _OUT_BUF
    if _OUT_BUF is None:
        _OUT_BUF = np.empty((NF, NLOC, M * AXIS), np.float32)
    full = _OUT_BUF
    for c in range(NCORES):
        for f in range(NF):
            np.copyto(full[f, c * S:(c + 1) * S, :], r[c, f])
    return full
